# revision 93
# baseline (speedup 1.0000x reference)
"""Trainium2 Bass kernel for AttentiveNonLocalBlock2D (AllGather design).

Sequence-parallel over N=H*W across 8 cores, per the sharding hint's
tensor-parallel scheme: each core computes the gate + projections ONLY for
its own 1152-pixel chunk, then phi [32,1152] and G^T [128,9*64] are
AllGathered (DRAM-staged collectives) to form the full phi [32,9216] /
G [128,72*64] every core needs for its n-slice of the attention.

Per core:
  Phase A (one pool scope, no mid barriers): identity-matmul PE p-state
    warm-up under the input DMAs; 3x stride-2 conv gating unit (fp16 PE,
    lrelu = 0.6x+0.4|x| via ACT Abs + DVE stt); conv3 emits pre-transposed
    y3T halves; bilinear-upsample columns for the OWN chunk only via the
    per-core krC input (y3T^T @ krC) -> sigmoid -> fp16 gate-mul ->
    phi/theta/G^T projections.  The AllGathers + Exp table load launch
    outside the phase-A pools so no close-barrier gates pass-1 on them.
  Pass 1: 55 exp instructions (52x1536 cols = 1.33 m-tiles each, then
    tile-aligned 768/1152/1152 tails; the fp8 cache is contiguous so spans
    may cross m-tile boundaries): PE score matmuls fT = phi_tile^T
    theta_chunk into a 2-buffer PSUM ring, ACT exp(f - 2.5) written
    straight to a float8e5 cache (e5m2: wide range so the softmax
    denominator doesn't lose its tail to subnormal flushing; e4m3 loses
    ~10% of Z's mass).  Z[m] partials via DVE dead-store tensor_scalar
    accumulation over the cache (2x SBUF mode), except the final two
    tile-aligned instrs which use ACT's f32 accumulator (shortest path
    into the last Z-AllReduce); Z is AllReduced in 6 segments.
  Pass 2: per segment G is scaled by GSCALE/Z and split into fp8e4 high +
    residual parts; fp8 DoubleRow matmuls (2 m-tiles/instr, 0.5 cy/col,
    e4 stationary x e5 moving) accumulate into per-segment PSUM banks,
    pace-interleaved between later pass-1 instrs (margins keep not-ready
    units from parking at the PE queue head, which would starve ACT);
    DVE adds across segments, final read-out divides by GSCALE and adds
    the gated residual.  The last two segments drain after the loop behind
    a PE warm-up burst that keeps the p-state up through the final Z
    AllReduce latency.
  Host concatenates the per-core n-chunks.

Single-device build (the TimelineSim timing variant) replaces each
collective with one DRAM-hop DMA (upload straight to the gathered buffer);
landing DMAs are modeled in full.
"""

import sys

if "/opt/trn_rl_repo" not in sys.path:
    sys.path.insert(0, "/opt/trn_rl_repo")

import numpy as np

NCORES = 8
C, CI, H, W = 64, 32, 96, 96
N = H * W            # 9216
CH = N // NCORES     # 1152 pixels per core
MT = N // 128        # 72 m-tiles of 128
TPC = MT // NCORES   # 9 own m-tiles per core
EXP_BIAS = -2.5      # keeps exp(f+bias) <= ~16k < 57344 (e5m2 max) while
                     # minimizing subnormal flushing of tiny softmax terms
GSCALE = 64.0 * float(np.exp(-2.5 + 7.5))
                     # pre-scale so G*GSCALE/Z clears the e4m3 subnormal
                     # floor; tracks EXP_BIAS (Z scales with exp(bias))
SEGS = ((0, 22), (22, 40), (40, 54), (54, 64), (64, 70), (70, 72))
EIW = 1536           # exp-instruction width (cols)
# 52 x 1536-col instrs, then tile-aligned tails (768, 1152, 1152): the last
# two instrs cover exactly tiles 70 / 71 so their Z comes from the ACT f32
# accumulator (saves the DVE round trip on the final Z-AllReduce chain)
INSTRS = tuple([(i * EIW, EIW) for i in range(52)]
               + [(52 * EIW, 768), (70 * CH, CH), (71 * CH, CH)])
MARGINS = (7, 8, 7, 6, 99, 99)  # exp-instrs between AR issue and pass-2
                                # emit; last two segs drain after the loop
BUDGET = (3, 4)      # pass-2 units per exp instr (early, late)
RESID = True         # add an fp8 residual pass for G (extra accuracy)
# n-chunk subtiles for the two PSUM ring halves (bank-boundary aligned)
SUBS0 = ((0, 512), (512, 512), (1024, 128))
SUBS1 = ((0, 384), (384, 512), (896, 256))
YSUBS = ((0, 512), (512, 512), (1024, 128))  # pass-2 output subtiles

_compiled = {}


def _zmode(j):
    """Z accumulation engine per tile: DVE only (the dead-store
    tensor_scalar opcode does not exist on GPSIMD, and ACT's accumulator
    cannot be used because exp instructions span m-tile boundaries)."""
    return "dve"


def _seg_of(j):
    for k, (j0, j1) in enumerate(SEGS):
        if j0 <= j < j1:
            return k, j0
    raise ValueError(j)


def _build(single=False):
    import concourse.bacc as bacc
    import concourse.bass as bass
    import concourse.mybir as mybir
    import concourse.tile as tile
    from concourse import masks

    f16 = mybir.dt.float16
    f32 = mybir.dt.float32
    f8 = mybir.dt.float8e4
    f8w = mybir.dt.float8e5   # exp cache: wide range so tiny softmax terms
                              # aren't flushed (Z would lose ~10% of its mass)
    DR = mybir.MatmulPerfMode.DoubleRow
    AF = mybir.ActivationFunctionType
    ALU = mybir.AluOpType

    nc = bacc.Bacc("TRN2", target_bir_lowering=False, debug=False,
                   num_devices=1 if single else NCORES)

    xpad_io = nc.dram_tensor("xpad", [C, 98, 98], f16, kind="ExternalInput")
    w1_io = nc.dram_tensor("w1", [C, 9 * C], f16, kind="ExternalInput")
    w2_io = nc.dram_tensor("w2", [C, 9 * C], f16, kind="ExternalInput")
    w3_io = nc.dram_tensor("w3", [C, 9 * C], f16, kind="ExternalInput")
    twT_io = nc.dram_tensor("twT", [C, CI], f16, kind="ExternalInput")
    pwT_io = nc.dram_tensor("pwT", [C, CI], f16, kind="ExternalInput")
    gw_io = nc.dram_tensor("gw", [CI, C], f32, kind="ExternalInput")
    WwT_io = nc.dram_tensor("WwT", [CI, C], f32, kind="ExternalInput")
    xch_io = nc.dram_tensor("xch", [C, CH], f16, kind="ExternalInput")
    krC_io = nc.dram_tensor("krC", [144, CH], f16, kind="ExternalInput")
    out_io = nc.dram_tensor("out", [C, CH], f32, kind="ExternalOutput")

    groups = [list(range(NCORES))]

    with tile.TileContext(nc) as tc:
        with tc.tile_pool(name="persist", bufs=1) as pp, \
             tc.tile_pool(name="dram", bufs=1, space="DRAM") as dp:
            # per-segment Z tiles so the AR DMA reads never alias later writes
            zsumk = [pp.tile([128, j1 - j0], f32, name=f"zsum{k}")
                     for k, (j0, j1) in enumerate(SEGS)]
            zredk = [pp.tile([128, j1 - j0], f32, name=f"zred{k}")
                     for k, (j0, j1) in enumerate(SEGS)]
            nb5 = pp.tile([128, 1], f32)
            nc.gpsimd.memset(nb5[:], EXP_BIAS)
            zin = [dp.tile([128, j1 - j0], f32, name=f"zin{k}")
                   for k, (j0, j1) in enumerate(SEGS)]
            zout = [dp.tile([128, j1 - j0], f32, addr_space="Shared",
                            name=f"zout{k}")
                    for k, (j0, j1) in enumerate(SEGS)]
            phin = dp.tile([CI, CH], f16, name="phin")
            phout = dp.tile([NCORES, CI, CH], f16, addr_space="Shared",
                            name="phout")
            gin = dp.tile([128, TPC * C], f16, name="gin")
            gout = dp.tile([NCORES, 128, TPC * C], f16, addr_space="Shared",
                           name="gout")

            with tc.tile_pool(name="hand", bufs=1) as hp:
                phi16 = hp.tile([CI, N], f16)
                th16 = hp.tile([CI, CH], f16)
                G16 = hp.tile([128, MT * C], f16)
                G3 = G16[:].rearrange("p (j c) -> p j c", c=C)
                G8 = hp.tile([128, MT * C], f8)
                G83 = G8[:].rearrange("p (j c) -> p j c", c=C)
                R8 = hp.tile([128, MT * C], f8)
                R83 = R8[:].rearrange("p (j c) -> p j c", c=C)
                xgc16 = hp.tile([C, CH], f16)
                outsb = hp.tile([C, CH], f32)
                # (exp-table load is implicit before the first pass-1 exp;
                # it hides behind the phi AllGather landing wait)
                zdeadV = hp.tile([128, CH], f8w)  # dead stores for Z accum
                zdeadP = hp.tile([128, CH], f8w)  # (same dtype as the cache)
                phiown = hp.tile([CI, CH], f16)
                gown = hp.tile([128, TPC * C], f16)
                s8 = hp.tile([128, MT * CH], f8w)
                s83 = s8[:].rearrange("p (j n) -> p j n", n=CH)

                # ==================== PHASE A ====================
                # single merged pool scope: no mid-phase close barrier
                # between the convs and the gate/projection pipeline
                with tc.tile_pool(name="pa", bufs=1) as pa, \
                     tc.tile_pool(name="paps", bufs=2, space="PSUM") as paps:
                    y3Ta = pa.tile([72, C], f16)
                    y3Tb = pa.tile([72, C], f16)
                    # preload the Sigmoid table while input DMAs fly
                    tld0 = pa.tile([128, 1], f32)
                    nc.scalar.activation(tld0[:], nb5[:], AF.Sigmoid)
                    # ramp the PE p-state during the input-DMA wait so conv1
                    # runs at full speed from its first matmul (identity
                    # needs no DMA)
                    ident = pa.tile([C, C], f16)
                    masks.make_identity(nc, ident[:])
                    wmps = paps.tile([C, C], f32, tag="warm", name="wmps",
                                     bufs=1)
                    for _ in range(140):
                        nc.tensor.matmul(wmps[:], ident[:], ident[:],
                                         start=True, stop=True,
                                         skip_group_check=True)

                    # conv-critical DMAs first: HWDGE is one serial queue,
                    # and conv1 must run gapless to keep the PE p-state up
                    xpad = pa.tile([C, 98, 98], f16)
                    w1sb = pa.tile([C, 9 * C], f16)
                    nc.sync.dma_start(xpad[:, 0:18, :], xpad_io[:, 0:18, :])
                    nc.sync.dma_start(w1sb[:], w1_io[:])
                    nc.sync.dma_start(xpad[:, 18:50, :], xpad_io[:, 18:50, :])
                    nc.sync.dma_start(xpad[:, 50:98, :], xpad_io[:, 50:98, :])
                    w2sb = pa.tile([C, 9 * C], f16)
                    nc.sync.dma_start(w2sb[:], w2_io[:])
                    w3sb = pa.tile([C, 9 * C], f16)
                    nc.sync.dma_start(w3sb[:], w3_io[:])
                    twT16 = pa.tile([C, CI], f16)
                    nc.sync.dma_start(twT16[:], twT_io[:])
                    pwT16 = pa.tile([C, CI], f16)
                    nc.sync.dma_start(pwT16[:], pwT_io[:])
                    gwsb = pa.tile([CI, C], f32)
                    nc.sync.dma_start(gwsb[:], gw_io[:])
                    WwTsb = pa.tile([CI, C], f32)
                    nc.sync.dma_start(WwTsb[:], WwT_io[:])
                    krCa = pa.tile([72, CH], f16)
                    nc.sync.dma_start(krCa[:], krC_io[0:72, :])
                    krCb = pa.tile([72, CH], f16)
                    nc.sync.dma_start(krCb[:], krC_io[72:144, :])
                    xchsb = pa.tile([C, CH], f16)
                    nc.sync.dma_start(xchsb[:], xch_io[:])

                    # conv1: 96x96 -> 48x48, stride 2, pad 1, lrelu(0.2)
                    y1p = pa.tile([C, 50, 50], f16)
                    nc.gpsimd.memset(y1p[:], 0.0)
                    for g in range(6):
                        ps1 = paps.tile([C, 8, 48], f32, tag="cv", name="ps1")
                        for t in range(9):
                            dy, dx = t // 3, t % 3
                            nc.tensor.matmul(
                                ps1[:], w1sb[:, t * C:(t + 1) * C],
                                xpad[:, 16 * g + dy: 16 * g + dy + 16: 2,
                                     dx: dx + 96: 2],
                                start=(t == 0), stop=(t == 8))
                        # lrelu(x) = 0.6*x + 0.4*|x| (only one PSUM input
                        # allowed per DVE op; ACT is idle during the convs)
                        ab1 = pa.tile([C, 8 * 48], f32, tag="ab1", name="ab1",
                                      bufs=2)
                        nc.scalar.activation(ab1[:], ps1[:], AF.Abs,
                                             scale=0.4)
                        nc.vector.scalar_tensor_tensor(
                            y1p[:, 1 + 8 * g: 9 + 8 * g, 1:49], ps1[:], 0.6,
                            ab1[:], op0=ALU.mult, op1=ALU.add)

                    # conv2: 48x48 -> 24x24
                    y2p = pa.tile([C, 26, 26], f16)
                    nc.gpsimd.memset(y2p[:], 0.0)
                    for g in range(2):
                        ps2 = paps.tile([C, 12, 24], f32, tag="cv", name="ps2")
                        for t in range(9):
                            dy, dx = t // 3, t % 3
                            nc.tensor.matmul(
                                ps2[:], w2sb[:, t * C:(t + 1) * C],
                                y1p[:, 24 * g + dy: 24 * g + dy + 24: 2,
                                    dx: dx + 48: 2],
                                start=(t == 0), stop=(t == 8))
                        ab2 = pa.tile([C, 12 * 24], f32, tag="ab2", name="ab2",
                                      bufs=2)
                        nc.scalar.activation(ab2[:], ps2[:], AF.Abs,
                                             scale=0.4)
                        nc.vector.scalar_tensor_tensor(
                            y2p[:, 1 + 12 * g: 13 + 12 * g, 1:25], ps2[:], 0.6,
                            ab2[:], op0=ALU.mult, op1=ALU.add)

                    # conv3: 24x24 -> 12x12 (no activation), then PE
                    # transpose into y3T[(row, col), c] halves
                    ps3 = paps.tile([C, 12, 12], f32, tag="cv", name="ps3")
                    for t in range(9):
                        dy, dx = t // 3, t % 3
                        nc.tensor.matmul(
                            ps3[:], w3sb[:, t * C:(t + 1) * C],
                            y2p[:, dy: dy + 24: 2, dx: dx + 24: 2],
                            start=(t == 0), stop=(t == 8))
                    y3f = pa.tile([C, 144], f16)
                    nc.vector.tensor_copy(y3f[:], ps3[:])
                    for hh, y3t in ((0, y3Ta), (1, y3Tb)):
                        pst = paps.tile([72, C], f16, tag="cv", name="pst")
                        nc.tensor.transpose(
                            pst[:], y3f[:, 72 * hh:72 * (hh + 1)], ident[:])
                        nc.vector.tensor_copy(y3t[:], pst[:])

                    # E^T = gw^T WwT [C, C]
                    eps = paps.tile([C, 512], f32, tag="prj", name="eps",
                                    bufs=3)
                    nc.tensor.matmul(eps[:, 0:C], gwsb[:], WwTsb[:],
                                     start=True, stop=True)
                    ET16 = hp.tile([C, C], f16)
                    nc.vector.tensor_copy(ET16[:], eps[:, 0:C])

                    # gate pipeline: all krons first (kron -> sigmoid ->
                    # fp16 gate-mul per sub), then the phi chain (it feeds
                    # the AllGather = the pass-1 critical path), then theta
                    gtc = pa.tile([C, CH], f16)
                    for o0, w in SUBS0:
                        kps = paps.tile([C, 512], f32, tag="prj",
                                        name="kps", bufs=3)
                        nc.tensor.matmul(kps[:, 0:w], y3Ta[:],
                                         krCa[:, o0:o0 + w],
                                         start=True, stop=False)
                        nc.tensor.matmul(kps[:, 0:w], y3Tb[:],
                                         krCb[:, o0:o0 + w],
                                         start=False, stop=True)
                        nc.scalar.activation(gtc[:, o0:o0 + w],
                                             kps[:, 0:w], AF.Sigmoid)
                        nc.vector.tensor_mul(xgc16[:, o0:o0 + w],
                                             gtc[:, o0:o0 + w],
                                             xchsb[:, o0:o0 + w])
                    # preload the Exp table during the gate pipeline: the read
                    # of gtc pins this after sigmoid0 (it cannot be hoisted
                    # to t=0 where the sigmoid load would evict it again)
                    tld1 = pa.tile([C, 1], f32)
                    nc.scalar.activation(tld1[:], gtc[:, 0:1], AF.Exp)
                    for o0, w in SUBS0:
                        pps = paps.tile([C, 512], f32, tag="prj",
                                        name="pps", bufs=3)
                        nc.tensor.matmul(pps[0:CI, 0:w], pwT16[:],
                                         xgc16[:, o0:o0 + w],
                                         start=True, stop=True)
                        nc.vector.tensor_copy(phiown[:, o0:o0 + w],
                                              pps[0:CI, 0:w])
                    for o0, w in SUBS0:
                        tps = paps.tile([C, 512], f32, tag="prj",
                                        name="tps", bufs=3)
                        nc.tensor.matmul(tps[0:CI, 0:w], twT16[:],
                                         xgc16[:, o0:o0 + w],
                                         start=True, stop=True)
                        # (GPSIMD cannot read PSUM on HW: copies on DVE)
                        nc.vector.tensor_copy(th16[:, o0:o0 + w],
                                              tps[0:CI, 0:w])

                    # own G^T tiles [128, 9*C] (AllGathered later)
                    gps = paps.tile([128, TPC * C], f32, tag="gps",
                                    name="gps", bufs=1)
                    for u in range(TPC):
                        nc.tensor.matmul(gps[:, u * C:(u + 1) * C],
                                         xgc16[:, u * 128:(u + 1) * 128],
                                         ET16[:], start=True, stop=True)
                    nc.vector.tensor_copy(gown[:], gps[:])

                # ====== PASS 1 with seg-interleaved fp8 PASS 2 + ARs ======
                with tc.tile_pool(name="p1ps", bufs=2, space="PSUM") as p1ps, \
                     tc.tile_pool(name="p2ps", bufs=2, space="PSUM") as p2ps, \
                     tc.tile_pool(name="p2", bufs=1) as p2:
                    # AllGathers emitted inside this scope so no pool-close
                    # barrier or clock alignment gates pass-1 on them.
                    # single-mode convention: ONE DRAM hop stands in for
                    # upload+collective; landing DMAs are modeled in full.
                    if single:
                        nc.sync.dma_start(phout[0, :, :], phiown[:])
                    else:
                        nc.sync.dma_start(phin[:], phiown[:])
                        nc.gpsimd.collective_compute(
                            "AllGather", ALU.bypass, replica_groups=groups,
                            ins=[phin.opt()], outs=[phout.opt()])
                    # land slice r=0 first: it unblocks pass-1 tiles 0-8
                    nc.sync.dma_start(phi16[:, 0:CH], phout[0, :, :])
                    nc.sync.dma_start(
                        phi16[:, CH:].rearrange("c (r n) -> c r n",
                                                r=NCORES - 1),
                        phout[1:, :, :].rearrange("r c n -> c r n"))

                    # warm the PE through the AG landing wait with fake
                    # pass-1 tiles read from phiown (already in SBUF)
                    for _ in range(3):
                        wfps = p1ps.tile([128, EIW], f32, tag="fps",
                                         name="fps")
                        for o0 in range(0, EIW, 512):
                            nc.tensor.matmul(wfps[:, o0:o0 + 512],
                                             phiown[:, 0:128],
                                             th16[:, 0:512],
                                             start=True, stop=True)

                    def emit_G_ag():
                        if single:
                            nc.sync.dma_start(gout[0, :, :], gown[:])
                        else:
                            nc.sync.dma_start(gin[:], gown[:])
                            nc.gpsimd.collective_compute(
                                "AllGather", ALU.bypass,
                                replica_groups=groups,
                                ins=[gin.opt()], outs=[gout.opt()])
                        nc.sync.dma_start(
                            G16[:].rearrange("p (r n) -> p r n", r=NCORES),
                            gout[:].rearrange("r p n -> p r n"))

                    def pass1_instr(i):
                        # one exp instruction = up to 1.33 m-tiles; the fp8
                        # cache is contiguous so the exp span can cross
                        # m-tile boundaries; Z is per-m-tile off the cache,
                        # except single-tile-aligned instrs which use the
                        # ACT f32 accumulator directly
                        c0, wd = INSTRS[i]
                        fps = p1ps.tile([128, wd], f32, tag="fps",
                                        name="fps")
                        edges = {0, wd}
                        for b in range(512, wd, 512):
                            edges.add(b)
                        jlo, jhi = c0 // CH, (c0 + wd - 1) // CH
                        for j in range(jlo, jhi + 1):
                            if c0 < j * CH < c0 + wd:
                                edges.add(j * CH - c0)
                        edges = sorted(edges)
                        for a, b in zip(edges[:-1], edges[1:]):
                            j = (c0 + a) // CH
                            ta = c0 + a - j * CH
                            nc.tensor.matmul(fps[:, a:b],
                                             phi16[:, j * 128:(j + 1) * 128],
                                             th16[:, ta:ta + (b - a)],
                                             start=True, stop=True)
                        aligned = (wd == CH and c0 % CH == 0)
                        if aligned:
                            j = c0 // CH
                            k, j0 = _seg_of(j)
                            nc.scalar.activation(
                                s8[:, c0:c0 + wd], fps[:], AF.Exp,
                                bias=nb5[:], scale=1.0,
                                accum_out=zsumk[k][:, j - j0:j - j0 + 1])
                            return
                        nc.scalar.activation(s8[:, c0:c0 + wd], fps[:],
                                             AF.Exp, bias=nb5[:], scale=1.0)
                        # Z for every m-tile completed by this instruction
                        for j in range(jlo, jhi + 1):
                            if (j + 1) * CH <= c0 + wd:
                                k, j0 = _seg_of(j)
                                zcol = zsumk[k][:, j - j0:j - j0 + 1]
                                nc.vector.tensor_scalar(
                                    zdeadV[:], s83[:, j, :], 1.0, 0.0,
                                    op0=ALU.mult, op1=ALU.add,
                                    accum_out=zcol)

                    def allreduce(k):
                        # single-mode convention (as for the AllGathers):
                        # one DRAM hop stands in for upload+collective
                        if single:
                            nc.sync.dma_start(zout[k][:], zsumk[k][:])
                        else:
                            nc.sync.dma_start(zin[k][:], zsumk[k][:])
                            nc.gpsimd.collective_compute(
                                "AllReduce", ALU.add,
                                replica_groups=groups,
                                ins=[zin[k].opt()], outs=[zout[k].opt()])
                        nc.sync.dma_start(zredk[k][:], zout[k][:])

                    def scale_G(k):
                        j0, j1 = SEGS[k]
                        ln = j1 - j0
                        zf = p2.tile([128, 22], f32, tag="zf", name="zf",
                                     bufs=2)
                        # 1/(Z/GSCALE) = GSCALE/Z
                        nc.vector.tensor_scalar(zf[:, 0:ln], zredk[k][:],
                                                1.0 / GSCALE, None,
                                                op0=ALU.mult)
                        rz = p2.tile([128, 22], f32, tag="rz", name="rz",
                                     bufs=2)
                        nc.vector.reciprocal(rz[:, 0:ln], zf[:, 0:ln])
                        rzb = rz[:, 0:ln].unsqueeze(-1).to_broadcast(
                            (128, ln, C))
                        nc.vector.tensor_mul(G3[:, j0:j1, :],
                                             G3[:, j0:j1, :], rzb)
                        nc.vector.tensor_copy(G83[:, j0:j1, :],
                                              G3[:, j0:j1, :])
                        if RESID:
                            # split G into fp8 high + fp8 residual parts
                            rt = p2.tile([128, 22 * C], f16, tag="rt",
                                         name="rt", bufs=2)
                            rt3 = rt[:].rearrange("p (j c) -> p j c", c=C)
                            nc.vector.tensor_sub(rt3[:, 0:ln, :],
                                                 G3[:, j0:j1, :],
                                                 G83[:, j0:j1, :])
                            nc.vector.tensor_copy(R83[:, j0:j1, :],
                                                  rt3[:, 0:ln, :])

                    # pass-2 work units: (k, ci, u); per-segment PSUM
                    # accumulation, DVE adds across segments into outsb
                    units = []
                    for k in range(len(SEGS)):
                        j0, j1 = SEGS[k]
                        for ci in range(len(YSUBS)):
                            for u in range((j1 - j0) // 2):
                                units.append((k, ci, u))
                    emitted = 0
                    cur_ps = {}

                    def emit_unit():
                        nonlocal emitted
                        k, ci, u = units[emitted]
                        j0, j1 = SEGS[k]
                        o0, w = YSUBS[ci]
                        npr = (j1 - j0) // 2
                        jj = j0 + 2 * u
                        if u == 0:
                            cur_ps[ci] = p2ps.tile([C, 512], f32, tag="yps",
                                                   name="yps")
                        yp = cur_ps[ci]
                        nc.tensor.matmul(
                            yp[:, 0:w], G83[:, jj:jj + 2, :],
                            s83[:, jj:jj + 2, o0:o0 + w],
                            start=(u == 0), stop=(not RESID and u == npr - 1),
                            perf_mode=DR, skip_group_check=True)
                        if RESID:
                            nc.tensor.matmul(
                                yp[:, 0:w], R83[:, jj:jj + 2, :],
                                s83[:, jj:jj + 2, o0:o0 + w],
                                start=False, stop=(u == npr - 1),
                                perf_mode=DR, skip_group_check=True)
                        if u == npr - 1:
                            eng = nc.vector
                            osl = outsb[:, o0:o0 + w]
                            if k == 0:
                                eng.tensor_copy(osl, yp[:, 0:w])
                            else:
                                eng.tensor_add(osl, osl, yp[:, 0:w])
                            if k == len(SEGS) - 1:
                                # undo GSCALE pre-scale, add gated residual
                                eng.scalar_tensor_tensor(
                                    osl, osl, 1.0 / GSCALE,
                                    xgc16[:, o0:o0 + w],
                                    op0=ALU.mult, op1=ALU.add)
                                nc.sync.dma_start(out_io[:, o0:o0 + w], osl)
                        emitted += 1

                    # m-tile j's exp completes during exp-instr ei(j)
                    def ei_of(j):
                        end = (j + 1) * CH
                        for i, (c0, wd) in enumerate(INSTRS):
                            if c0 + wd >= end:
                                return i
                        raise ValueError(j)

                    seg_ei = [ei_of(s[1] - 1) for s in SEGS]
                    seg_units = [sum(1 for x in units if x[0] <= k)
                                 for k in range(len(SEGS))]
                    avail = [0]

                    def pump(i):
                        if i == 2:
                            emit_G_ag()
                        for k in range(len(SEGS)):
                            if i == seg_ei[k]:
                                allreduce(k)
                                scale_G(k)
                            if (k < len(SEGS) - 2
                                    and i == seg_ei[k] + MARGINS[k]):
                                # last 2 segs drain after the loop, behind
                                # the PE warm-up (parked units would block
                                # the warm-up and drop the p-state)
                                avail[0] = seg_units[k]
                        budget = BUDGET[0] if i < 30 else BUDGET[1]
                        while emitted < avail[0] and budget > 0:
                            emit_unit()
                            budget -= 1

                    for i in range(len(INSTRS)):
                        pass1_instr(i)
                        pump(i)
                    # keep the PE p-state warm through the final Z-AR wait:
                    # re-run an already-satisfied pair into a scratch bank
                    wps = p2ps.tile([C, 512], f32, tag="yps", name="wps")
                    for _ in range(26):
                        nc.tensor.matmul(wps[:], G83[:, 0:2, :],
                                         s83[:, 0:2, 0:512],
                                         start=True, stop=True, perf_mode=DR,
                                         skip_group_check=True)
                    while emitted < len(units):
                        emit_unit()

    nc.compile()
    return nc


def get_program():
    if "nc" not in _compiled:
        _compiled["nc"] = _build()
    return _compiled["nc"]


def _bilinear_kron():
    """K[(k,j), (R,Cc)] = uv[R,k]*uv[Cc,j] for x8 bilinear upsample 12->96
    (align_corners=False, edge-clamped), split into two 72-row halves."""
    uv = np.zeros((96, 12), np.float64)
    for R in range(96):
        t = (R + 0.5) / 8.0 - 0.5
        k0 = int(np.floor(t))
        fr = t - k0
        for k, wt in ((k0, 1.0 - fr), (k0 + 1, fr)):
            kc = min(max(k, 0), 11)
            uv[R, kc] += wt
    K = np.einsum("Rk,Cj->kjRC", uv, uv).reshape(144, 9216)
    return np.ascontiguousarray(K).astype(np.float16)


def make_in_maps(inputs):
    f16 = np.float16
    x = np.asarray(inputs["x"], np.float32).reshape(C, H, W)
    xflat = np.ascontiguousarray(x.reshape(C, N))
    xpad = np.zeros((C, 98, 98), f16)
    xpad[:, 1:97, 1:97] = x.astype(f16)
    krF = _bilinear_kron()

    def conv_w(w):
        # [o, i, dy, dx] -> [i, (dy dx), o]
        return np.ascontiguousarray(
            np.asarray(w, np.float32).transpose(1, 2, 3, 0).reshape(C, 9 * C)
        ).astype(f16)

    base = {
        "xpad": xpad,
        "w1": conv_w(inputs["d1_w"]),
        "w2": conv_w(inputs["d2_w"]),
        "w3": conv_w(inputs["d3_w"]),
        "twT": np.ascontiguousarray(
            np.asarray(inputs["th_w"], np.float32)[:, :, 0, 0].T).astype(f16),
        "pwT": np.ascontiguousarray(
            np.asarray(inputs["ph_w"], np.float32)[:, :, 0, 0].T).astype(f16),
        "gw": np.ascontiguousarray(
            np.asarray(inputs["g_w"], np.float32)[:, :, 0, 0]),
        "WwT": np.ascontiguousarray(
            np.asarray(inputs["W_w"], np.float32)[:, :, 0, 0].T),
    }
    in_maps = []
    for k in range(NCORES):
        m = dict(base)
        m["xch"] = np.ascontiguousarray(
            xflat[:, k * CH:(k + 1) * CH]).astype(f16)
        m["krC"] = np.ascontiguousarray(krF[:, k * CH:(k + 1) * CH])
        in_maps.append(m)
    return in_maps


def kernel(**inputs):
    from concourse import bass_utils

    nc = get_program()
    in_maps = make_in_maps(inputs)
    res = bass_utils.run_bass_kernel_spmd(nc, in_maps,
                                          core_ids=list(range(NCORES)))
    out = np.concatenate([res.results[k]["out"] for k in range(NCORES)], axis=1)
    return out.reshape(1, C, H, W).astype(np.float32)


# revision 101
# speedup vs baseline: 1.0023x; 1.0023x over previous
"""Trainium2 Bass kernel for AttentiveNonLocalBlock2D (AllGather design).

Sequence-parallel over N=H*W across 8 cores, per the sharding hint's
tensor-parallel scheme: each core computes the gate + projections ONLY for
its own 1152-pixel chunk, then phi [32,1152] and G^T [128,9*64] are
AllGathered (DRAM-staged collectives) to form the full phi [32,9216] /
G [128,72*64] every core needs for its n-slice of the attention.

Per core:
  Phase A (one pool scope, no mid barriers): identity-matmul PE p-state
    warm-up under the input DMAs; 3x stride-2 conv gating unit (fp16 PE,
    lrelu = 0.6x+0.4|x| via ACT Abs + DVE stt); conv3 emits pre-transposed
    y3T halves; bilinear-upsample columns for the OWN chunk only via the
    per-core krC input (y3T^T @ krC) -> sigmoid -> fp16 gate-mul ->
    phi/theta/G^T projections.  The AllGathers + Exp table load launch
    outside the phase-A pools so no close-barrier gates pass-1 on them.
  Pass 1: 55 exp instructions (52x1536 cols = 1.33 m-tiles each, then
    tile-aligned 768/1152/1152 tails; the fp8 cache is contiguous so spans
    may cross m-tile boundaries): PE score matmuls fT = phi_tile^T
    theta_chunk into a 2-buffer PSUM ring, ACT exp(f - 2.5) written
    straight to a float8e5 cache (e5m2: wide range so the softmax
    denominator doesn't lose its tail to subnormal flushing; e4m3 loses
    ~10% of Z's mass).  Z[m] partials via DVE dead-store tensor_scalar
    accumulation over the cache (2x SBUF mode), except the final two
    tile-aligned instrs which use ACT's f32 accumulator (shortest path
    into the last Z-AllReduce); Z is AllReduced in 6 segments.
  Pass 2: per segment G is scaled by GSCALE/Z and split into fp8e4 high +
    residual parts; fp8 DoubleRow matmuls (2 m-tiles/instr, 0.5 cy/col,
    e4 stationary x e5 moving) accumulate into per-segment PSUM banks,
    pace-interleaved between later pass-1 instrs (margins keep not-ready
    units from parking at the PE queue head, which would starve ACT);
    DVE adds across segments, final read-out divides by GSCALE and adds
    the gated residual.  The last two segments drain after the loop behind
    a PE warm-up burst that keeps the p-state up through the final Z
    AllReduce latency.
  Host concatenates the per-core n-chunks.

Single-device build (the TimelineSim timing variant) replaces each
collective with one DRAM-hop DMA (upload straight to the gathered buffer);
landing DMAs are modeled in full.
"""

import sys

if "/opt/trn_rl_repo" not in sys.path:
    sys.path.insert(0, "/opt/trn_rl_repo")

import numpy as np

NCORES = 8
C, CI, H, W = 64, 32, 96, 96
N = H * W            # 9216
CH = N // NCORES     # 1152 pixels per core
MT = N // 128        # 72 m-tiles of 128
TPC = MT // NCORES   # 9 own m-tiles per core
EXP_BIAS = -2.5      # keeps exp(f+bias) <= ~16k < 57344 (e5m2 max) while
                     # minimizing subnormal flushing of tiny softmax terms
GSCALE = 64.0 * float(np.exp(-2.5 + 7.5))
                     # pre-scale so G*GSCALE/Z clears the e4m3 subnormal
                     # floor; tracks EXP_BIAS (Z scales with exp(bias))
SEGS = ((0, 22), (22, 40), (40, 54), (54, 64), (64, 70), (70, 72))
EIW = 1536           # exp-instruction width (cols)
# 52 x 1536-col instrs, then tile-aligned tails (768, 1152, 1152): the last
# two instrs cover exactly tiles 70 / 71 so their Z comes from the ACT f32
# accumulator (saves the DVE round trip on the final Z-AllReduce chain)
INSTRS = tuple([(i * EIW, EIW) for i in range(52)]
               + [(52 * EIW, 768), (70 * CH, CH), (71 * CH, CH)])
MARGINS = (7, 8, 7, 5, 99, 99)  # exp-instrs between AR issue and pass-2
                                # emit; last two segs drain after the loop
BUDGET = (3, 4)      # pass-2 units per exp instr (early, late)
RESID = True         # add an fp8 residual pass for G (extra accuracy)
# n-chunk subtiles for the two PSUM ring halves (bank-boundary aligned)
SUBS0 = ((0, 512), (512, 512), (1024, 128))
SUBS1 = ((0, 384), (384, 512), (896, 256))
YSUBS = ((0, 512), (512, 512), (1024, 128))  # pass-2 output subtiles

_compiled = {}


def _zmode(j):
    """Z accumulation engine per tile: DVE only (the dead-store
    tensor_scalar opcode does not exist on GPSIMD, and ACT's accumulator
    cannot be used because exp instructions span m-tile boundaries)."""
    return "dve"


def _seg_of(j):
    for k, (j0, j1) in enumerate(SEGS):
        if j0 <= j < j1:
            return k, j0
    raise ValueError(j)


def _build(single=False):
    import concourse.bacc as bacc
    import concourse.bass as bass
    import concourse.mybir as mybir
    import concourse.tile as tile
    from concourse import masks

    f16 = mybir.dt.float16
    f32 = mybir.dt.float32
    f8 = mybir.dt.float8e4
    f8w = mybir.dt.float8e5   # exp cache: wide range so tiny softmax terms
                              # aren't flushed (Z would lose ~10% of its mass)
    DR = mybir.MatmulPerfMode.DoubleRow
    AF = mybir.ActivationFunctionType
    ALU = mybir.AluOpType

    nc = bacc.Bacc("TRN2", target_bir_lowering=False, debug=False,
                   num_devices=1 if single else NCORES)

    xpad_io = nc.dram_tensor("xpad", [C, 98, 98], f16, kind="ExternalInput")
    w1_io = nc.dram_tensor("w1", [C, 9 * C], f16, kind="ExternalInput")
    w2_io = nc.dram_tensor("w2", [C, 9 * C], f16, kind="ExternalInput")
    w3_io = nc.dram_tensor("w3", [C, 9 * C], f16, kind="ExternalInput")
    twT_io = nc.dram_tensor("twT", [C, CI], f16, kind="ExternalInput")
    pwT_io = nc.dram_tensor("pwT", [C, CI], f16, kind="ExternalInput")
    gw_io = nc.dram_tensor("gw", [CI, C], f32, kind="ExternalInput")
    WwT_io = nc.dram_tensor("WwT", [CI, C], f32, kind="ExternalInput")
    xch_io = nc.dram_tensor("xch", [C, CH], f16, kind="ExternalInput")
    krC_io = nc.dram_tensor("krC", [144, CH], f16, kind="ExternalInput")
    out_io = nc.dram_tensor("out", [C, CH], f32, kind="ExternalOutput")

    groups = [list(range(NCORES))]

    with tile.TileContext(nc) as tc:
        with tc.tile_pool(name="persist", bufs=1) as pp, \
             tc.tile_pool(name="dram", bufs=1, space="DRAM") as dp:
            # per-segment Z tiles so the AR DMA reads never alias later writes
            zsumk = [pp.tile([128, j1 - j0], f32, name=f"zsum{k}")
                     for k, (j0, j1) in enumerate(SEGS)]
            zredk = [pp.tile([128, j1 - j0], f32, name=f"zred{k}")
                     for k, (j0, j1) in enumerate(SEGS)]
            nb5 = pp.tile([128, 1], f32)
            nc.gpsimd.memset(nb5[:], EXP_BIAS)
            zin = [dp.tile([128, j1 - j0], f32, name=f"zin{k}")
                   for k, (j0, j1) in enumerate(SEGS)]
            zout = [dp.tile([128, j1 - j0], f32, addr_space="Shared",
                            name=f"zout{k}")
                    for k, (j0, j1) in enumerate(SEGS)]
            phin = dp.tile([CI, CH], f16, name="phin")
            phout = dp.tile([NCORES, CI, CH], f16, addr_space="Shared",
                            name="phout")
            gin = dp.tile([128, TPC * C], f16, name="gin")
            gout = dp.tile([NCORES, 128, TPC * C], f16, addr_space="Shared",
                           name="gout")

            with tc.tile_pool(name="hand", bufs=1) as hp:
                phi16 = hp.tile([CI, N], f16)
                th16 = hp.tile([CI, CH], f16)
                G16 = hp.tile([128, MT * C], f16)
                G3 = G16[:].rearrange("p (j c) -> p j c", c=C)
                G8 = hp.tile([128, MT * C], f8)
                G83 = G8[:].rearrange("p (j c) -> p j c", c=C)
                R8 = hp.tile([128, MT * C], f8)
                R83 = R8[:].rearrange("p (j c) -> p j c", c=C)
                xgc16 = hp.tile([C, CH], f16)
                outsb = hp.tile([C, CH], f32)
                # (exp-table load is implicit before the first pass-1 exp;
                # it hides behind the phi AllGather landing wait)
                zdeadV = hp.tile([128, CH], f8w)  # dead stores for Z accum
                zdeadP = hp.tile([128, CH], f8w)  # (same dtype as the cache)
                phiown = hp.tile([CI, CH], f16)
                gown = hp.tile([128, TPC * C], f16)
                s8 = hp.tile([128, MT * CH], f8w)
                s83 = s8[:].rearrange("p (j n) -> p j n", n=CH)

                # ==================== PHASE A ====================
                # single merged pool scope: no mid-phase close barrier
                # between the convs and the gate/projection pipeline
                with tc.tile_pool(name="pa", bufs=1) as pa, \
                     tc.tile_pool(name="paps", bufs=2, space="PSUM") as paps:
                    y3Ta = pa.tile([72, C], f16)
                    y3Tb = pa.tile([72, C], f16)
                    # preload the Sigmoid table while input DMAs fly
                    tld0 = pa.tile([128, 1], f32)
                    nc.scalar.activation(tld0[:], nb5[:], AF.Sigmoid)
                    # ramp the PE p-state during the input-DMA wait so conv1
                    # runs at full speed from its first matmul (identity
                    # needs no DMA)
                    ident = pa.tile([C, C], f16)
                    masks.make_identity(nc, ident[:])
                    wmps = paps.tile([C, C], f32, tag="warm", name="wmps",
                                     bufs=1)
                    for _ in range(140):
                        nc.tensor.matmul(wmps[:], ident[:], ident[:],
                                         start=True, stop=True,
                                         skip_group_check=True)

                    # conv-critical DMAs first: HWDGE is one serial queue,
                    # and conv1 must run gapless to keep the PE p-state up
                    xpad = pa.tile([C, 98, 98], f16)
                    w1sb = pa.tile([C, 9 * C], f16)
                    nc.sync.dma_start(xpad[:, 0:18, :], xpad_io[:, 0:18, :])
                    nc.sync.dma_start(w1sb[:], w1_io[:])
                    nc.sync.dma_start(xpad[:, 18:50, :], xpad_io[:, 18:50, :])
                    nc.sync.dma_start(xpad[:, 50:98, :], xpad_io[:, 50:98, :])
                    w2sb = pa.tile([C, 9 * C], f16)
                    nc.sync.dma_start(w2sb[:], w2_io[:])
                    w3sb = pa.tile([C, 9 * C], f16)
                    nc.sync.dma_start(w3sb[:], w3_io[:])
                    twT16 = pa.tile([C, CI], f16)
                    nc.sync.dma_start(twT16[:], twT_io[:])
                    pwT16 = pa.tile([C, CI], f16)
                    nc.sync.dma_start(pwT16[:], pwT_io[:])
                    gwsb = pa.tile([CI, C], f32)
                    nc.sync.dma_start(gwsb[:], gw_io[:])
                    WwTsb = pa.tile([CI, C], f32)
                    nc.sync.dma_start(WwTsb[:], WwT_io[:])
                    krCa = pa.tile([72, CH], f16)
                    nc.sync.dma_start(krCa[:], krC_io[0:72, :])
                    krCb = pa.tile([72, CH], f16)
                    nc.sync.dma_start(krCb[:], krC_io[72:144, :])
                    xchsb = pa.tile([C, CH], f16)
                    nc.sync.dma_start(xchsb[:], xch_io[:])

                    # conv1: 96x96 -> 48x48, stride 2, pad 1, lrelu(0.2)
                    y1p = pa.tile([C, 50, 50], f16)
                    nc.gpsimd.memset(y1p[:], 0.0)
                    for g in range(6):
                        ps1 = paps.tile([C, 8, 48], f32, tag="cv", name="ps1")
                        for t in range(9):
                            dy, dx = t // 3, t % 3
                            nc.tensor.matmul(
                                ps1[:], w1sb[:, t * C:(t + 1) * C],
                                xpad[:, 16 * g + dy: 16 * g + dy + 16: 2,
                                     dx: dx + 96: 2],
                                start=(t == 0), stop=(t == 8))
                        # lrelu(x) = 0.6*x + 0.4*|x| (only one PSUM input
                        # allowed per DVE op; ACT is idle during the convs)
                        ab1 = pa.tile([C, 8 * 48], f32, tag="ab1", name="ab1",
                                      bufs=2)
                        nc.scalar.activation(ab1[:], ps1[:], AF.Abs,
                                             scale=0.4)
                        nc.vector.scalar_tensor_tensor(
                            y1p[:, 1 + 8 * g: 9 + 8 * g, 1:49], ps1[:], 0.6,
                            ab1[:], op0=ALU.mult, op1=ALU.add)

                    # conv2: 48x48 -> 24x24
                    y2p = pa.tile([C, 26, 26], f16)
                    nc.gpsimd.memset(y2p[:], 0.0)
                    for g in range(2):
                        ps2 = paps.tile([C, 12, 24], f32, tag="cv", name="ps2")
                        for t in range(9):
                            dy, dx = t // 3, t % 3
                            nc.tensor.matmul(
                                ps2[:], w2sb[:, t * C:(t + 1) * C],
                                y1p[:, 24 * g + dy: 24 * g + dy + 24: 2,
                                    dx: dx + 48: 2],
                                start=(t == 0), stop=(t == 8))
                        ab2 = pa.tile([C, 12 * 24], f32, tag="ab2", name="ab2",
                                      bufs=2)
                        nc.scalar.activation(ab2[:], ps2[:], AF.Abs,
                                             scale=0.4)
                        nc.vector.scalar_tensor_tensor(
                            y2p[:, 1 + 12 * g: 13 + 12 * g, 1:25], ps2[:], 0.6,
                            ab2[:], op0=ALU.mult, op1=ALU.add)

                    # conv3: 24x24 -> 12x12 (no activation), then PE
                    # transpose into y3T[(row, col), c] halves
                    ps3 = paps.tile([C, 12, 12], f32, tag="cv", name="ps3")
                    for t in range(9):
                        dy, dx = t // 3, t % 3
                        nc.tensor.matmul(
                            ps3[:], w3sb[:, t * C:(t + 1) * C],
                            y2p[:, dy: dy + 24: 2, dx: dx + 24: 2],
                            start=(t == 0), stop=(t == 8))
                    y3f = pa.tile([C, 144], f16)
                    nc.vector.tensor_copy(y3f[:], ps3[:])
                    for hh, y3t in ((0, y3Ta), (1, y3Tb)):
                        pst = paps.tile([72, C], f16, tag="cv", name="pst")
                        nc.tensor.transpose(
                            pst[:], y3f[:, 72 * hh:72 * (hh + 1)], ident[:])
                        nc.vector.tensor_copy(y3t[:], pst[:])

                    # E^T = gw^T WwT [C, C]
                    eps = paps.tile([C, 512], f32, tag="prj", name="eps",
                                    bufs=3)
                    nc.tensor.matmul(eps[:, 0:C], gwsb[:], WwTsb[:],
                                     start=True, stop=True)
                    ET16 = hp.tile([C, C], f16)
                    nc.vector.tensor_copy(ET16[:], eps[:, 0:C])

                    # gate pipeline: all krons first (kron -> sigmoid ->
                    # fp16 gate-mul per sub), then the phi chain (it feeds
                    # the AllGather = the pass-1 critical path), then theta
                    gtc = pa.tile([C, CH], f16)
                    for o0, w in SUBS0:
                        kps = paps.tile([C, 512], f32, tag="prj",
                                        name="kps", bufs=3)
                        nc.tensor.matmul(kps[:, 0:w], y3Ta[:],
                                         krCa[:, o0:o0 + w],
                                         start=True, stop=False)
                        nc.tensor.matmul(kps[:, 0:w], y3Tb[:],
                                         krCb[:, o0:o0 + w],
                                         start=False, stop=True)
                        nc.scalar.activation(gtc[:, o0:o0 + w],
                                             kps[:, 0:w], AF.Sigmoid)
                        nc.vector.tensor_mul(xgc16[:, o0:o0 + w],
                                             gtc[:, o0:o0 + w],
                                             xchsb[:, o0:o0 + w])
                    # preload the Exp table during the gate pipeline: the read
                    # of gtc pins this after sigmoid0 (it cannot be hoisted
                    # to t=0 where the sigmoid load would evict it again)
                    tld1 = pa.tile([C, 1], f32)
                    nc.scalar.activation(tld1[:], gtc[:, 0:1], AF.Exp)
                    for o0, w in SUBS0:
                        pps = paps.tile([C, 512], f32, tag="prj",
                                        name="pps", bufs=3)
                        nc.tensor.matmul(pps[0:CI, 0:w], pwT16[:],
                                         xgc16[:, o0:o0 + w],
                                         start=True, stop=True)
                        nc.vector.tensor_copy(phiown[:, o0:o0 + w],
                                              pps[0:CI, 0:w])
                    for o0, w in SUBS0:
                        tps = paps.tile([C, 512], f32, tag="prj",
                                        name="tps", bufs=3)
                        nc.tensor.matmul(tps[0:CI, 0:w], twT16[:],
                                         xgc16[:, o0:o0 + w],
                                         start=True, stop=True)
                        # (GPSIMD cannot read PSUM on HW: copies on DVE)
                        nc.vector.tensor_copy(th16[:, o0:o0 + w],
                                              tps[0:CI, 0:w])

                    # own G^T tiles [128, 9*C] (AllGathered later)
                    gps = paps.tile([128, TPC * C], f32, tag="gps",
                                    name="gps", bufs=1)
                    for u in range(TPC):
                        nc.tensor.matmul(gps[:, u * C:(u + 1) * C],
                                         xgc16[:, u * 128:(u + 1) * 128],
                                         ET16[:], start=True, stop=True)
                    nc.vector.tensor_copy(gown[:], gps[:])

                # ====== PASS 1 with seg-interleaved fp8 PASS 2 + ARs ======
                with tc.tile_pool(name="p1ps", bufs=2, space="PSUM") as p1ps, \
                     tc.tile_pool(name="p2ps", bufs=2, space="PSUM") as p2ps, \
                     tc.tile_pool(name="p2", bufs=1) as p2:
                    # AllGathers emitted inside this scope so no pool-close
                    # barrier or clock alignment gates pass-1 on them.
                    # single-mode convention: ONE DRAM hop stands in for
                    # upload+collective; landing DMAs are modeled in full.
                    if single:
                        nc.sync.dma_start(phout[0, :, :], phiown[:])
                    else:
                        nc.sync.dma_start(phin[:], phiown[:])
                        nc.gpsimd.collective_compute(
                            "AllGather", ALU.bypass, replica_groups=groups,
                            ins=[phin.opt()], outs=[phout.opt()])
                    # land slice r=0 first: it unblocks pass-1 tiles 0-8
                    nc.sync.dma_start(phi16[:, 0:CH], phout[0, :, :])
                    nc.sync.dma_start(
                        phi16[:, CH:].rearrange("c (r n) -> c r n",
                                                r=NCORES - 1),
                        phout[1:, :, :].rearrange("r c n -> c r n"))

                    # warm the PE through the AG landing wait with fake
                    # pass-1 tiles read from phiown (already in SBUF)
                    for _ in range(3):
                        wfps = p1ps.tile([128, EIW], f32, tag="fps",
                                         name="fps")
                        for o0 in range(0, EIW, 512):
                            nc.tensor.matmul(wfps[:, o0:o0 + 512],
                                             phiown[:, 0:128],
                                             th16[:, 0:512],
                                             start=True, stop=True)

                    def emit_G_ag():
                        if single:
                            nc.sync.dma_start(gout[0, :, :], gown[:])
                        else:
                            nc.sync.dma_start(gin[:], gown[:])
                            nc.gpsimd.collective_compute(
                                "AllGather", ALU.bypass,
                                replica_groups=groups,
                                ins=[gin.opt()], outs=[gout.opt()])
                        nc.sync.dma_start(
                            G16[:].rearrange("p (r n) -> p r n", r=NCORES),
                            gout[:].rearrange("r p n -> p r n"))

                    def pass1_instr(i):
                        # one exp instruction = up to 1.33 m-tiles; the fp8
                        # cache is contiguous so the exp span can cross
                        # m-tile boundaries; Z is per-m-tile off the cache,
                        # except single-tile-aligned instrs which use the
                        # ACT f32 accumulator directly
                        c0, wd = INSTRS[i]
                        fps = p1ps.tile([128, wd], f32, tag="fps",
                                        name="fps")
                        edges = {0, wd}
                        for b in range(512, wd, 512):
                            edges.add(b)
                        jlo, jhi = c0 // CH, (c0 + wd - 1) // CH
                        for j in range(jlo, jhi + 1):
                            if c0 < j * CH < c0 + wd:
                                edges.add(j * CH - c0)
                        edges = sorted(edges)
                        for a, b in zip(edges[:-1], edges[1:]):
                            j = (c0 + a) // CH
                            ta = c0 + a - j * CH
                            nc.tensor.matmul(fps[:, a:b],
                                             phi16[:, j * 128:(j + 1) * 128],
                                             th16[:, ta:ta + (b - a)],
                                             start=True, stop=True)
                        aligned = (wd == CH and c0 % CH == 0)
                        if aligned:
                            j = c0 // CH
                            k, j0 = _seg_of(j)
                            nc.scalar.activation(
                                s8[:, c0:c0 + wd], fps[:], AF.Exp,
                                bias=nb5[:], scale=1.0,
                                accum_out=zsumk[k][:, j - j0:j - j0 + 1])
                            return
                        nc.scalar.activation(s8[:, c0:c0 + wd], fps[:],
                                             AF.Exp, bias=nb5[:], scale=1.0)
                        # Z for every m-tile completed by this instruction
                        for j in range(jlo, jhi + 1):
                            if (j + 1) * CH <= c0 + wd:
                                k, j0 = _seg_of(j)
                                zcol = zsumk[k][:, j - j0:j - j0 + 1]
                                nc.vector.tensor_scalar(
                                    zdeadV[:], s83[:, j, :], 1.0, 0.0,
                                    op0=ALU.mult, op1=ALU.add,
                                    accum_out=zcol)

                    def allreduce(k):
                        # single-mode convention (as for the AllGathers):
                        # one DRAM hop stands in for upload+collective
                        if single:
                            nc.sync.dma_start(zout[k][:], zsumk[k][:])
                        else:
                            nc.sync.dma_start(zin[k][:], zsumk[k][:])
                            nc.gpsimd.collective_compute(
                                "AllReduce", ALU.add,
                                replica_groups=groups,
                                ins=[zin[k].opt()], outs=[zout[k].opt()])
                        nc.sync.dma_start(zredk[k][:], zout[k][:])

                    def scale_G(k):
                        j0, j1 = SEGS[k]
                        ln = j1 - j0
                        zf = p2.tile([128, 22], f32, tag="zf", name="zf",
                                     bufs=2)
                        # 1/(Z/GSCALE) = GSCALE/Z
                        nc.vector.tensor_scalar(zf[:, 0:ln], zredk[k][:],
                                                1.0 / GSCALE, None,
                                                op0=ALU.mult)
                        rz = p2.tile([128, 22], f32, tag="rz", name="rz",
                                     bufs=2)
                        nc.vector.reciprocal(rz[:, 0:ln], zf[:, 0:ln])
                        rzb = rz[:, 0:ln].unsqueeze(-1).to_broadcast(
                            (128, ln, C))
                        nc.vector.tensor_mul(G3[:, j0:j1, :],
                                             G3[:, j0:j1, :], rzb)
                        nc.vector.tensor_copy(G83[:, j0:j1, :],
                                              G3[:, j0:j1, :])
                        if RESID:
                            # split G into fp8 high + fp8 residual parts
                            rt = p2.tile([128, 22 * C], f16, tag="rt",
                                         name="rt", bufs=2)
                            rt3 = rt[:].rearrange("p (j c) -> p j c", c=C)
                            nc.vector.tensor_sub(rt3[:, 0:ln, :],
                                                 G3[:, j0:j1, :],
                                                 G83[:, j0:j1, :])
                            nc.vector.tensor_copy(R83[:, j0:j1, :],
                                                  rt3[:, 0:ln, :])

                    # pass-2 work units: (k, ci, u); per-segment PSUM
                    # accumulation, DVE adds across segments into outsb
                    units = []
                    for k in range(len(SEGS)):
                        j0, j1 = SEGS[k]
                        for ci in range(len(YSUBS)):
                            for u in range((j1 - j0) // 2):
                                units.append((k, ci, u))
                    emitted = 0
                    cur_ps = {}

                    def emit_unit():
                        nonlocal emitted
                        k, ci, u = units[emitted]
                        j0, j1 = SEGS[k]
                        o0, w = YSUBS[ci]
                        npr = (j1 - j0) // 2
                        jj = j0 + 2 * u
                        if u == 0:
                            cur_ps[ci] = p2ps.tile([C, 512], f32, tag="yps",
                                                   name="yps")
                        yp = cur_ps[ci]
                        nc.tensor.matmul(
                            yp[:, 0:w], G83[:, jj:jj + 2, :],
                            s83[:, jj:jj + 2, o0:o0 + w],
                            start=(u == 0), stop=(not RESID and u == npr - 1),
                            perf_mode=DR, skip_group_check=True)
                        if RESID:
                            nc.tensor.matmul(
                                yp[:, 0:w], R83[:, jj:jj + 2, :],
                                s83[:, jj:jj + 2, o0:o0 + w],
                                start=False, stop=(u == npr - 1),
                                perf_mode=DR, skip_group_check=True)
                        if u == npr - 1:
                            eng = nc.vector
                            osl = outsb[:, o0:o0 + w]
                            if k == 0:
                                eng.tensor_copy(osl, yp[:, 0:w])
                            else:
                                eng.tensor_add(osl, osl, yp[:, 0:w])
                            if k == len(SEGS) - 1:
                                # undo GSCALE pre-scale, add gated residual
                                eng.scalar_tensor_tensor(
                                    osl, osl, 1.0 / GSCALE,
                                    xgc16[:, o0:o0 + w],
                                    op0=ALU.mult, op1=ALU.add)
                                nc.sync.dma_start(out_io[:, o0:o0 + w], osl)
                        emitted += 1

                    # m-tile j's exp completes during exp-instr ei(j)
                    def ei_of(j):
                        end = (j + 1) * CH
                        for i, (c0, wd) in enumerate(INSTRS):
                            if c0 + wd >= end:
                                return i
                        raise ValueError(j)

                    seg_ei = [ei_of(s[1] - 1) for s in SEGS]
                    seg_units = [sum(1 for x in units if x[0] <= k)
                                 for k in range(len(SEGS))]
                    avail = [0]

                    def pump(i):
                        if i == 2:
                            emit_G_ag()
                        for k in range(len(SEGS)):
                            if i == seg_ei[k]:
                                allreduce(k)
                                scale_G(k)
                            if (k < len(SEGS) - 2
                                    and i == seg_ei[k] + MARGINS[k]):
                                # last 2 segs drain after the loop, behind
                                # the PE warm-up (parked units would block
                                # the warm-up and drop the p-state)
                                avail[0] = seg_units[k]
                        budget = (BUDGET[0] if i < 30 else
                                  2 if i >= 52 else BUDGET[1])
                        while emitted < avail[0] and budget > 0:
                            emit_unit()
                            budget -= 1

                    for i in range(len(INSTRS)):
                        pass1_instr(i)
                        pump(i)
                    # keep the PE p-state warm through the final Z-AR wait:
                    # re-run an already-satisfied pair into a scratch bank
                    wps = p2ps.tile([C, 512], f32, tag="yps", name="wps")
                    for _ in range(22):
                        nc.tensor.matmul(wps[:], G83[:, 0:2, :],
                                         s83[:, 0:2, 0:512],
                                         start=True, stop=True, perf_mode=DR,
                                         skip_group_check=True)
                    while emitted < len(units):
                        emit_unit()

    nc.compile()
    return nc


def get_program():
    if "nc" not in _compiled:
        _compiled["nc"] = _build()
    return _compiled["nc"]


def _bilinear_kron():
    """K[(k,j), (R,Cc)] = uv[R,k]*uv[Cc,j] for x8 bilinear upsample 12->96
    (align_corners=False, edge-clamped), split into two 72-row halves."""
    uv = np.zeros((96, 12), np.float64)
    for R in range(96):
        t = (R + 0.5) / 8.0 - 0.5
        k0 = int(np.floor(t))
        fr = t - k0
        for k, wt in ((k0, 1.0 - fr), (k0 + 1, fr)):
            kc = min(max(k, 0), 11)
            uv[R, kc] += wt
    K = np.einsum("Rk,Cj->kjRC", uv, uv).reshape(144, 9216)
    return np.ascontiguousarray(K).astype(np.float16)


def make_in_maps(inputs):
    f16 = np.float16
    x = np.asarray(inputs["x"], np.float32).reshape(C, H, W)
    xflat = np.ascontiguousarray(x.reshape(C, N))
    xpad = np.zeros((C, 98, 98), f16)
    xpad[:, 1:97, 1:97] = x.astype(f16)
    krF = _bilinear_kron()

    def conv_w(w):
        # [o, i, dy, dx] -> [i, (dy dx), o]
        return np.ascontiguousarray(
            np.asarray(w, np.float32).transpose(1, 2, 3, 0).reshape(C, 9 * C)
        ).astype(f16)

    base = {
        "xpad": xpad,
        "w1": conv_w(inputs["d1_w"]),
        "w2": conv_w(inputs["d2_w"]),
        "w3": conv_w(inputs["d3_w"]),
        "twT": np.ascontiguousarray(
            np.asarray(inputs["th_w"], np.float32)[:, :, 0, 0].T).astype(f16),
        "pwT": np.ascontiguousarray(
            np.asarray(inputs["ph_w"], np.float32)[:, :, 0, 0].T).astype(f16),
        "gw": np.ascontiguousarray(
            np.asarray(inputs["g_w"], np.float32)[:, :, 0, 0]),
        "WwT": np.ascontiguousarray(
            np.asarray(inputs["W_w"], np.float32)[:, :, 0, 0].T),
    }
    in_maps = []
    for k in range(NCORES):
        m = dict(base)
        m["xch"] = np.ascontiguousarray(
            xflat[:, k * CH:(k + 1) * CH]).astype(f16)
        m["krC"] = np.ascontiguousarray(krF[:, k * CH:(k + 1) * CH])
        in_maps.append(m)
    return in_maps


def kernel(**inputs):
    from concourse import bass_utils

    nc = get_program()
    in_maps = make_in_maps(inputs)
    res = bass_utils.run_bass_kernel_spmd(nc, in_maps,
                                          core_ids=list(range(NCORES)))
    out = np.concatenate([res.results[k]["out"] for k in range(NCORES)], axis=1)
    return out.reshape(1, C, H, W).astype(np.float32)


# revision 104
# speedup vs baseline: 1.0111x; 1.0088x over previous
"""Trainium2 Bass kernel for AttentiveNonLocalBlock2D (AllGather design).

Sequence-parallel over N=H*W across 8 cores, per the sharding hint's
tensor-parallel scheme: each core computes the gate + projections ONLY for
its own 1152-pixel chunk, then phi [32,1152] and G^T [128,9*64] are
AllGathered (DRAM-staged collectives) to form the full phi [32,9216] /
G [128,72*64] every core needs for its n-slice of the attention.

Per core:
  Phase A (one pool scope, no mid barriers): identity-matmul PE p-state
    warm-up under the input DMAs; 3x stride-2 conv gating unit (fp16 PE,
    lrelu = 0.6x+0.4|x| via ACT Abs + DVE stt); conv3 emits pre-transposed
    y3T halves; bilinear-upsample columns for the OWN chunk only via the
    per-core krC input (y3T^T @ krC) -> sigmoid -> fp16 gate-mul ->
    phi/theta/G^T projections.  The AllGathers + Exp table load launch
    outside the phase-A pools so no close-barrier gates pass-1 on them.
  Pass 1: 55 exp instructions (52x1536 cols = 1.33 m-tiles each, then
    tile-aligned 768/1152/1152 tails; the fp8 cache is contiguous so spans
    may cross m-tile boundaries): PE score matmuls fT = phi_tile^T
    theta_chunk into a 2-buffer PSUM ring, ACT exp(f - 2.5) written
    straight to a float8e5 cache (e5m2: wide range so the softmax
    denominator doesn't lose its tail to subnormal flushing; e4m3 loses
    ~10% of Z's mass).  Z[m] partials via DVE dead-store tensor_scalar
    accumulation over the cache (2x SBUF mode), except the final two
    tile-aligned instrs which use ACT's f32 accumulator (shortest path
    into the last Z-AllReduce); Z is AllReduced in 6 segments.
  Pass 2: per segment G is scaled by GSCALE/Z and split into fp8e4 high +
    residual parts; fp8 DoubleRow matmuls (2 m-tiles/instr, 0.5 cy/col,
    e4 stationary x e5 moving) accumulate into per-segment PSUM banks,
    pace-interleaved between later pass-1 instrs (margins keep not-ready
    units from parking at the PE queue head, which would starve ACT);
    DVE adds across segments, final read-out divides by GSCALE and adds
    the gated residual.  The last two segments drain after the loop behind
    a PE warm-up burst that keeps the p-state up through the final Z
    AllReduce latency.
  Host concatenates the per-core n-chunks.

Single-device build (the TimelineSim timing variant) replaces each
collective with one DRAM-hop DMA (upload straight to the gathered buffer);
landing DMAs are modeled in full.
"""

import sys

if "/opt/trn_rl_repo" not in sys.path:
    sys.path.insert(0, "/opt/trn_rl_repo")

import numpy as np

NCORES = 8
C, CI, H, W = 64, 32, 96, 96
N = H * W            # 9216
CH = N // NCORES     # 1152 pixels per core
MT = N // 128        # 72 m-tiles of 128
TPC = MT // NCORES   # 9 own m-tiles per core
EXP_BIAS = -2.5      # keeps exp(f+bias) <= ~16k < 57344 (e5m2 max) while
                     # minimizing subnormal flushing of tiny softmax terms
GSCALE = 64.0 * float(np.exp(-2.5 + 7.5))
                     # pre-scale so G*GSCALE/Z clears the e4m3 subnormal
                     # floor; tracks EXP_BIAS (Z scales with exp(bias))
SEGS = ((0, 22), (22, 40), (40, 54), (54, 64), (64, 70), (70, 72))
EIW = 1536           # exp-instruction width (cols)
# 52 x 1536-col instrs, then tile-aligned tails (768, 1152, 1152): the last
# two instrs cover exactly tiles 70 / 71 so their Z comes from the ACT f32
# accumulator (saves the DVE round trip on the final Z-AllReduce chain)
INSTRS = tuple([(i * EIW, EIW) for i in range(52)]
               + [(52 * EIW, 768), (70 * CH, CH), (71 * CH, CH)])
MARGINS = (7, 8, 7, 5, 99, 99)  # exp-instrs between AR issue and pass-2
                                # emit; last two segs drain after the loop
BUDGET = (3, 4)      # pass-2 units per exp instr (early, late)
RESID = True         # add an fp8 residual pass for G (extra accuracy)
# n-chunk subtiles for the two PSUM ring halves (bank-boundary aligned)
SUBS0 = ((0, 512), (512, 512), (1024, 128))
SUBS1 = ((0, 384), (384, 512), (896, 256))
YSUBS = ((0, 512), (512, 512), (1024, 128))  # pass-2 output subtiles

_compiled = {}


def _zmode(j):
    """Z accumulation engine per tile: DVE only (the dead-store
    tensor_scalar opcode does not exist on GPSIMD, and ACT's accumulator
    cannot be used because exp instructions span m-tile boundaries)."""
    return "dve"


def _seg_of(j):
    for k, (j0, j1) in enumerate(SEGS):
        if j0 <= j < j1:
            return k, j0
    raise ValueError(j)


def _build(single=False):
    import concourse.bacc as bacc
    import concourse.bass as bass
    import concourse.mybir as mybir
    import concourse.tile as tile
    from concourse import masks

    f16 = mybir.dt.float16
    f32 = mybir.dt.float32
    f8 = mybir.dt.float8e4
    f8w = mybir.dt.float8e5   # exp cache: wide range so tiny softmax terms
                              # aren't flushed (Z would lose ~10% of its mass)
    DR = mybir.MatmulPerfMode.DoubleRow
    AF = mybir.ActivationFunctionType
    ALU = mybir.AluOpType

    nc = bacc.Bacc("TRN2", target_bir_lowering=False, debug=False,
                   num_devices=1 if single else NCORES)

    xpad_io = nc.dram_tensor("xpad", [C, 98, 98], f16, kind="ExternalInput")
    w1_io = nc.dram_tensor("w1", [C, 9 * C], f16, kind="ExternalInput")
    w2_io = nc.dram_tensor("w2", [C, 9 * C], f16, kind="ExternalInput")
    w3_io = nc.dram_tensor("w3", [C, 9 * C], f16, kind="ExternalInput")
    twT_io = nc.dram_tensor("twT", [C, CI], f16, kind="ExternalInput")
    pwT_io = nc.dram_tensor("pwT", [C, CI], f16, kind="ExternalInput")
    gw_io = nc.dram_tensor("gw", [CI, C], f32, kind="ExternalInput")
    WwT_io = nc.dram_tensor("WwT", [CI, C], f32, kind="ExternalInput")
    xch_io = nc.dram_tensor("xch", [C, CH], f16, kind="ExternalInput")
    krC_io = nc.dram_tensor("krC", [144, CH], f16, kind="ExternalInput")
    out_io = nc.dram_tensor("out", [C, CH], f32, kind="ExternalOutput")

    groups = [list(range(NCORES))]

    with tile.TileContext(nc) as tc:
        with tc.tile_pool(name="persist", bufs=1) as pp, \
             tc.tile_pool(name="dram", bufs=1, space="DRAM") as dp:
            # per-segment Z tiles so the AR DMA reads never alias later writes
            zsumk = [pp.tile([128, j1 - j0], f32, name=f"zsum{k}")
                     for k, (j0, j1) in enumerate(SEGS)]
            zredk = [pp.tile([128, j1 - j0], f32, name=f"zred{k}")
                     for k, (j0, j1) in enumerate(SEGS)]
            nb5 = pp.tile([128, 1], f32)
            nc.gpsimd.memset(nb5[:], EXP_BIAS)
            zin = [dp.tile([128, j1 - j0], f32, name=f"zin{k}")
                   for k, (j0, j1) in enumerate(SEGS)]
            zout = [dp.tile([128, j1 - j0], f32, addr_space="Shared",
                            name=f"zout{k}")
                    for k, (j0, j1) in enumerate(SEGS)]
            phin = dp.tile([CI, CH], f16, name="phin")
            phout = dp.tile([NCORES, CI, CH], f16, addr_space="Shared",
                            name="phout")
            gin = dp.tile([128, TPC * C], f16, name="gin")
            gout = dp.tile([NCORES, 128, TPC * C], f16, addr_space="Shared",
                           name="gout")

            with tc.tile_pool(name="hand", bufs=1) as hp:
                phi16 = hp.tile([CI, N], f16)
                th16 = hp.tile([CI, CH], f16)
                G16 = hp.tile([128, MT * C], f16)
                G3 = G16[:].rearrange("p (j c) -> p j c", c=C)
                G8 = hp.tile([128, MT * C], f8)
                G83 = G8[:].rearrange("p (j c) -> p j c", c=C)
                R8 = hp.tile([128, MT * C], f8)
                R83 = R8[:].rearrange("p (j c) -> p j c", c=C)
                xgc16 = hp.tile([C, CH], f16)
                outsb = hp.tile([C, CH], f32)
                ofold = hp.tile([C, CH], f32)  # segs 0-3 sum, pre-folded
                # (exp-table load is implicit before the first pass-1 exp;
                # it hides behind the phi AllGather landing wait)
                zdeadV = hp.tile([128, CH], f8w)  # dead stores for Z accum
                zdeadP = hp.tile([128, CH], f8w)  # (same dtype as the cache)
                phiown = hp.tile([CI, CH], f16)
                gown = hp.tile([128, TPC * C], f16)
                s8 = hp.tile([128, MT * CH], f8w)
                s83 = s8[:].rearrange("p (j n) -> p j n", n=CH)

                # ==================== PHASE A ====================
                # single merged pool scope: no mid-phase close barrier
                # between the convs and the gate/projection pipeline
                with tc.tile_pool(name="pa", bufs=1) as pa, \
                     tc.tile_pool(name="paps", bufs=2, space="PSUM") as paps:
                    y3Ta = pa.tile([72, C], f16)
                    y3Tb = pa.tile([72, C], f16)
                    # preload the Sigmoid table while input DMAs fly
                    tld0 = pa.tile([128, 1], f32)
                    nc.scalar.activation(tld0[:], nb5[:], AF.Sigmoid)
                    # ramp the PE p-state during the input-DMA wait so conv1
                    # runs at full speed from its first matmul (identity
                    # needs no DMA)
                    ident = pa.tile([C, C], f16)
                    masks.make_identity(nc, ident[:])
                    wmps = paps.tile([C, C], f32, tag="warm", name="wmps",
                                     bufs=1)
                    for _ in range(140):
                        nc.tensor.matmul(wmps[:], ident[:], ident[:],
                                         start=True, stop=True,
                                         skip_group_check=True)

                    # conv-critical DMAs first: HWDGE is one serial queue,
                    # and conv1 must run gapless to keep the PE p-state up
                    xpad = pa.tile([C, 98, 98], f16)
                    w1sb = pa.tile([C, 9 * C], f16)
                    nc.sync.dma_start(xpad[:, 0:18, :], xpad_io[:, 0:18, :])
                    nc.sync.dma_start(w1sb[:], w1_io[:])
                    nc.sync.dma_start(xpad[:, 18:50, :], xpad_io[:, 18:50, :])
                    nc.sync.dma_start(xpad[:, 50:98, :], xpad_io[:, 50:98, :])
                    w2sb = pa.tile([C, 9 * C], f16)
                    nc.sync.dma_start(w2sb[:], w2_io[:])
                    w3sb = pa.tile([C, 9 * C], f16)
                    nc.sync.dma_start(w3sb[:], w3_io[:])
                    twT16 = pa.tile([C, CI], f16)
                    nc.sync.dma_start(twT16[:], twT_io[:])
                    pwT16 = pa.tile([C, CI], f16)
                    nc.sync.dma_start(pwT16[:], pwT_io[:])
                    gwsb = pa.tile([CI, C], f32)
                    nc.sync.dma_start(gwsb[:], gw_io[:])
                    WwTsb = pa.tile([CI, C], f32)
                    nc.sync.dma_start(WwTsb[:], WwT_io[:])
                    krCa = pa.tile([72, CH], f16)
                    nc.sync.dma_start(krCa[:], krC_io[0:72, :])
                    krCb = pa.tile([72, CH], f16)
                    nc.sync.dma_start(krCb[:], krC_io[72:144, :])
                    xchsb = pa.tile([C, CH], f16)
                    nc.sync.dma_start(xchsb[:], xch_io[:])

                    # conv1: 96x96 -> 48x48, stride 2, pad 1, lrelu(0.2)
                    y1p = pa.tile([C, 50, 50], f16)
                    nc.gpsimd.memset(y1p[:], 0.0)
                    for g in range(6):
                        ps1 = paps.tile([C, 8, 48], f32, tag="cv", name="ps1")
                        for t in range(9):
                            dy, dx = t // 3, t % 3
                            nc.tensor.matmul(
                                ps1[:], w1sb[:, t * C:(t + 1) * C],
                                xpad[:, 16 * g + dy: 16 * g + dy + 16: 2,
                                     dx: dx + 96: 2],
                                start=(t == 0), stop=(t == 8))
                        # lrelu(x) = 0.6*x + 0.4*|x| (only one PSUM input
                        # allowed per DVE op; ACT is idle during the convs)
                        ab1 = pa.tile([C, 8 * 48], f32, tag="ab1", name="ab1",
                                      bufs=2)
                        nc.scalar.activation(ab1[:], ps1[:], AF.Abs,
                                             scale=0.4)
                        nc.vector.scalar_tensor_tensor(
                            y1p[:, 1 + 8 * g: 9 + 8 * g, 1:49], ps1[:], 0.6,
                            ab1[:], op0=ALU.mult, op1=ALU.add)

                    # conv2: 48x48 -> 24x24
                    y2p = pa.tile([C, 26, 26], f16)
                    nc.gpsimd.memset(y2p[:], 0.0)
                    for g in range(2):
                        ps2 = paps.tile([C, 12, 24], f32, tag="cv", name="ps2")
                        for t in range(9):
                            dy, dx = t // 3, t % 3
                            nc.tensor.matmul(
                                ps2[:], w2sb[:, t * C:(t + 1) * C],
                                y1p[:, 24 * g + dy: 24 * g + dy + 24: 2,
                                    dx: dx + 48: 2],
                                start=(t == 0), stop=(t == 8))
                        ab2 = pa.tile([C, 12 * 24], f32, tag="ab2", name="ab2",
                                      bufs=2)
                        nc.scalar.activation(ab2[:], ps2[:], AF.Abs,
                                             scale=0.4)
                        nc.vector.scalar_tensor_tensor(
                            y2p[:, 1 + 12 * g: 13 + 12 * g, 1:25], ps2[:], 0.6,
                            ab2[:], op0=ALU.mult, op1=ALU.add)

                    # conv3: 24x24 -> 12x12 (no activation), then PE
                    # transpose into y3T[(row, col), c] halves
                    ps3 = paps.tile([C, 12, 12], f32, tag="cv", name="ps3")
                    for t in range(9):
                        dy, dx = t // 3, t % 3
                        nc.tensor.matmul(
                            ps3[:], w3sb[:, t * C:(t + 1) * C],
                            y2p[:, dy: dy + 24: 2, dx: dx + 24: 2],
                            start=(t == 0), stop=(t == 8))
                    y3f = pa.tile([C, 144], f16)
                    nc.vector.tensor_copy(y3f[:], ps3[:])
                    for hh, y3t in ((0, y3Ta), (1, y3Tb)):
                        pst = paps.tile([72, C], f16, tag="cv", name="pst")
                        nc.tensor.transpose(
                            pst[:], y3f[:, 72 * hh:72 * (hh + 1)], ident[:])
                        nc.vector.tensor_copy(y3t[:], pst[:])

                    # E^T = gw^T WwT [C, C]
                    eps = paps.tile([C, 512], f32, tag="prj", name="eps",
                                    bufs=3)
                    nc.tensor.matmul(eps[:, 0:C], gwsb[:], WwTsb[:],
                                     start=True, stop=True)
                    ET16 = hp.tile([C, C], f16)
                    nc.vector.tensor_copy(ET16[:], eps[:, 0:C])

                    # gate pipeline: all krons first (kron -> sigmoid ->
                    # fp16 gate-mul per sub), then the phi chain (it feeds
                    # the AllGather = the pass-1 critical path), then theta
                    gtc = pa.tile([C, CH], f16)
                    for o0, w in SUBS0:
                        kps = paps.tile([C, 512], f32, tag="prj",
                                        name="kps", bufs=3)
                        nc.tensor.matmul(kps[:, 0:w], y3Ta[:],
                                         krCa[:, o0:o0 + w],
                                         start=True, stop=False)
                        nc.tensor.matmul(kps[:, 0:w], y3Tb[:],
                                         krCb[:, o0:o0 + w],
                                         start=False, stop=True)
                        nc.scalar.activation(gtc[:, o0:o0 + w],
                                             kps[:, 0:w], AF.Sigmoid)
                        nc.vector.tensor_mul(xgc16[:, o0:o0 + w],
                                             gtc[:, o0:o0 + w],
                                             xchsb[:, o0:o0 + w])
                    # preload the Exp table during the gate pipeline: the read
                    # of gtc pins this after sigmoid0 (it cannot be hoisted
                    # to t=0 where the sigmoid load would evict it again)
                    tld1 = pa.tile([C, 1], f32)
                    nc.scalar.activation(tld1[:], gtc[:, 0:1], AF.Exp)
                    for o0, w in SUBS0:
                        pps = paps.tile([C, 512], f32, tag="prj",
                                        name="pps", bufs=3)
                        nc.tensor.matmul(pps[0:CI, 0:w], pwT16[:],
                                         xgc16[:, o0:o0 + w],
                                         start=True, stop=True)
                        nc.vector.tensor_copy(phiown[:, o0:o0 + w],
                                              pps[0:CI, 0:w])
                    for o0, w in SUBS0:
                        tps = paps.tile([C, 512], f32, tag="prj",
                                        name="tps", bufs=3)
                        nc.tensor.matmul(tps[0:CI, 0:w], twT16[:],
                                         xgc16[:, o0:o0 + w],
                                         start=True, stop=True)
                        # (GPSIMD cannot read PSUM on HW: copies on DVE)
                        nc.vector.tensor_copy(th16[:, o0:o0 + w],
                                              tps[0:CI, 0:w])

                    # own G^T tiles [128, 9*C] (AllGathered later)
                    gps = paps.tile([128, TPC * C], f32, tag="gps",
                                    name="gps", bufs=1)
                    for u in range(TPC):
                        nc.tensor.matmul(gps[:, u * C:(u + 1) * C],
                                         xgc16[:, u * 128:(u + 1) * 128],
                                         ET16[:], start=True, stop=True)
                    nc.vector.tensor_copy(gown[:], gps[:])

                # ====== PASS 1 with seg-interleaved fp8 PASS 2 + ARs ======
                with tc.tile_pool(name="p1ps", bufs=2, space="PSUM") as p1ps, \
                     tc.tile_pool(name="p2ps", bufs=2, space="PSUM") as p2ps, \
                     tc.tile_pool(name="p2", bufs=1) as p2:
                    # AllGathers emitted inside this scope so no pool-close
                    # barrier or clock alignment gates pass-1 on them.
                    # single-mode convention: ONE DRAM hop stands in for
                    # upload+collective; landing DMAs are modeled in full.
                    if single:
                        nc.sync.dma_start(phout[0, :, :], phiown[:])
                    else:
                        nc.sync.dma_start(phin[:], phiown[:])
                        nc.gpsimd.collective_compute(
                            "AllGather", ALU.bypass, replica_groups=groups,
                            ins=[phin.opt()], outs=[phout.opt()])
                    # land slice r=0 first: it unblocks pass-1 tiles 0-8
                    nc.sync.dma_start(phi16[:, 0:CH], phout[0, :, :])
                    nc.sync.dma_start(
                        phi16[:, CH:].rearrange("c (r n) -> c r n",
                                                r=NCORES - 1),
                        phout[1:, :, :].rearrange("r c n -> c r n"))

                    # warm the PE through the AG landing wait with fake
                    # pass-1 tiles read from phiown (already in SBUF)
                    for _ in range(3):
                        wfps = p1ps.tile([128, EIW], f32, tag="fps",
                                         name="fps")
                        for o0 in range(0, EIW, 512):
                            nc.tensor.matmul(wfps[:, o0:o0 + 512],
                                             phiown[:, 0:128],
                                             th16[:, 0:512],
                                             start=True, stop=True)

                    def emit_G_ag():
                        if single:
                            nc.sync.dma_start(gout[0, :, :], gown[:])
                        else:
                            nc.sync.dma_start(gin[:], gown[:])
                            nc.gpsimd.collective_compute(
                                "AllGather", ALU.bypass,
                                replica_groups=groups,
                                ins=[gin.opt()], outs=[gout.opt()])
                        nc.sync.dma_start(
                            G16[:].rearrange("p (r n) -> p r n", r=NCORES),
                            gout[:].rearrange("r p n -> p r n"))

                    def pass1_instr(i):
                        # one exp instruction = up to 1.33 m-tiles; the fp8
                        # cache is contiguous so the exp span can cross
                        # m-tile boundaries; Z is per-m-tile off the cache,
                        # except single-tile-aligned instrs which use the
                        # ACT f32 accumulator directly
                        c0, wd = INSTRS[i]
                        fps = p1ps.tile([128, wd], f32, tag="fps",
                                        name="fps")
                        edges = {0, wd}
                        for b in range(512, wd, 512):
                            edges.add(b)
                        jlo, jhi = c0 // CH, (c0 + wd - 1) // CH
                        for j in range(jlo, jhi + 1):
                            if c0 < j * CH < c0 + wd:
                                edges.add(j * CH - c0)
                        edges = sorted(edges)
                        for a, b in zip(edges[:-1], edges[1:]):
                            j = (c0 + a) // CH
                            ta = c0 + a - j * CH
                            nc.tensor.matmul(fps[:, a:b],
                                             phi16[:, j * 128:(j + 1) * 128],
                                             th16[:, ta:ta + (b - a)],
                                             start=True, stop=True)
                        aligned = (wd == CH and c0 % CH == 0)
                        if aligned:
                            j = c0 // CH
                            k, j0 = _seg_of(j)
                            nc.scalar.activation(
                                s8[:, c0:c0 + wd], fps[:], AF.Exp,
                                bias=nb5[:], scale=1.0,
                                accum_out=zsumk[k][:, j - j0:j - j0 + 1])
                            return
                        nc.scalar.activation(s8[:, c0:c0 + wd], fps[:],
                                             AF.Exp, bias=nb5[:], scale=1.0)
                        # Z for every m-tile completed by this instruction
                        for j in range(jlo, jhi + 1):
                            if (j + 1) * CH <= c0 + wd:
                                k, j0 = _seg_of(j)
                                zcol = zsumk[k][:, j - j0:j - j0 + 1]
                                nc.vector.tensor_scalar(
                                    zdeadV[:], s83[:, j, :], 1.0, 0.0,
                                    op0=ALU.mult, op1=ALU.add,
                                    accum_out=zcol)

                    def allreduce(k):
                        # single-mode convention (as for the AllGathers):
                        # one DRAM hop stands in for upload+collective
                        if single:
                            nc.sync.dma_start(zout[k][:], zsumk[k][:])
                        else:
                            nc.sync.dma_start(zin[k][:], zsumk[k][:])
                            nc.gpsimd.collective_compute(
                                "AllReduce", ALU.add,
                                replica_groups=groups,
                                ins=[zin[k].opt()], outs=[zout[k].opt()])
                        nc.sync.dma_start(zredk[k][:], zout[k][:])

                    def scale_G(k):
                        j0, j1 = SEGS[k]
                        ln = j1 - j0
                        zf = p2.tile([128, 22], f32, tag="zf", name="zf",
                                     bufs=2)
                        # 1/(Z/GSCALE) = GSCALE/Z
                        nc.vector.tensor_scalar(zf[:, 0:ln], zredk[k][:],
                                                1.0 / GSCALE, None,
                                                op0=ALU.mult)
                        rz = p2.tile([128, 22], f32, tag="rz", name="rz",
                                     bufs=2)
                        nc.vector.reciprocal(rz[:, 0:ln], zf[:, 0:ln])
                        rzb = rz[:, 0:ln].unsqueeze(-1).to_broadcast(
                            (128, ln, C))
                        nc.vector.tensor_mul(G3[:, j0:j1, :],
                                             G3[:, j0:j1, :], rzb)
                        nc.vector.tensor_copy(G83[:, j0:j1, :],
                                              G3[:, j0:j1, :])
                        if RESID:
                            # split G into fp8 high + fp8 residual parts
                            rt = p2.tile([128, 22 * C], f16, tag="rt",
                                         name="rt", bufs=2)
                            rt3 = rt[:].rearrange("p (j c) -> p j c", c=C)
                            nc.vector.tensor_sub(rt3[:, 0:ln, :],
                                                 G3[:, j0:j1, :],
                                                 G83[:, j0:j1, :])
                            nc.vector.tensor_copy(R83[:, j0:j1, :],
                                                  rt3[:, 0:ln, :])

                    # pass-2 work units: (k, ci, u); per-segment PSUM
                    # accumulation, DVE adds across segments into outsb
                    units = []
                    for k in range(len(SEGS)):
                        j0, j1 = SEGS[k]
                        for ci in range(len(YSUBS)):
                            for u in range((j1 - j0) // 2):
                                units.append((k, ci, u))
                    emitted = 0
                    cur_ps = {}

                    def emit_unit():
                        nonlocal emitted
                        k, ci, u = units[emitted]
                        j0, j1 = SEGS[k]
                        o0, w = YSUBS[ci]
                        npr = (j1 - j0) // 2
                        jj = j0 + 2 * u
                        if u == 0:
                            cur_ps[ci] = p2ps.tile([C, 512], f32, tag="yps",
                                                   name="yps")
                        yp = cur_ps[ci]
                        nc.tensor.matmul(
                            yp[:, 0:w], G83[:, jj:jj + 2, :],
                            s83[:, jj:jj + 2, o0:o0 + w],
                            start=(u == 0), stop=(not RESID and u == npr - 1),
                            perf_mode=DR, skip_group_check=True)
                        if RESID:
                            nc.tensor.matmul(
                                yp[:, 0:w], R83[:, jj:jj + 2, :],
                                s83[:, jj:jj + 2, o0:o0 + w],
                                start=False, stop=(u == npr - 1),
                                perf_mode=DR, skip_group_check=True)
                        if u == npr - 1:
                            osl = outsb[:, o0:o0 + w]
                            if k == 0:
                                nc.vector.tensor_copy(osl, yp[:, 0:w])
                            else:
                                nc.vector.tensor_add(osl, osl, yp[:, 0:w])
                            if k == 3:
                                # pre-fold GSCALE + gated residual for segs
                                # 0-3 NOW (hidden in the loop) so the final
                                # drain read-out is a single stt per sub
                                nc.vector.scalar_tensor_tensor(
                                    ofold[:, o0:o0 + w], osl, 1.0 / GSCALE,
                                    xgc16[:, o0:o0 + w],
                                    op0=ALU.mult, op1=ALU.add)
                        emitted += 1

                    # m-tile j's exp completes during exp-instr ei(j)
                    def ei_of(j):
                        end = (j + 1) * CH
                        for i, (c0, wd) in enumerate(INSTRS):
                            if c0 + wd >= end:
                                return i
                        raise ValueError(j)

                    seg_ei = [ei_of(s[1] - 1) for s in SEGS]
                    seg_units = [sum(1 for x in units if x[0] <= k)
                                 for k in range(len(SEGS))]
                    avail = [0]

                    def pump(i):
                        if i == 2:
                            emit_G_ag()
                        for k in range(len(SEGS)):
                            if i == seg_ei[k]:
                                allreduce(k)
                                scale_G(k)
                            if (k < len(SEGS) - 2
                                    and i == seg_ei[k] + MARGINS[k]):
                                # last 2 segs drain after the loop, behind
                                # the PE warm-up (parked units would block
                                # the warm-up and drop the p-state)
                                avail[0] = seg_units[k]
                        budget = (BUDGET[0] if i < 30 else
                                  2 if i >= 52 else BUDGET[1])
                        while emitted < avail[0] and budget > 0:
                            emit_unit()
                            budget -= 1

                    for i in range(len(INSTRS)):
                        pass1_instr(i)
                        pump(i)
                    # keep the PE p-state warm through the final Z-AR wait:
                    # re-run an already-satisfied pair into a scratch bank
                    wps = p2ps.tile([C, 512], f32, tag="yps", name="wps")
                    for _ in range(22):
                        nc.tensor.matmul(wps[:], G83[:, 0:2, :],
                                         s83[:, 0:2, 0:512],
                                         start=True, stop=True, perf_mode=DR,
                                         skip_group_check=True)
                    # segs <=3 stragglers, then segs 4+5 merged per output
                    # sub: one PSUM accumulation spanning both, and the
                    # final read-out is a single stt (yp/GSCALE + ofold)
                    while emitted < seg_units[3]:
                        emit_unit()
                    prs = [(k, u) for k in (4, 5)
                           for u in range((SEGS[k][1] - SEGS[k][0]) // 2)]
                    for ci, (o0, w) in enumerate(YSUBS):
                        yp = p2ps.tile([C, 512], f32, tag="yps", name="yps")
                        for pi, (k, u) in enumerate(prs):
                            jj = SEGS[k][0] + 2 * u
                            first = pi == 0
                            last = pi == len(prs) - 1
                            nc.tensor.matmul(
                                yp[:, 0:w], G83[:, jj:jj + 2, :],
                                s83[:, jj:jj + 2, o0:o0 + w],
                                start=first, stop=(not RESID and last),
                                perf_mode=DR, skip_group_check=True)
                            if RESID:
                                nc.tensor.matmul(
                                    yp[:, 0:w], R83[:, jj:jj + 2, :],
                                    s83[:, jj:jj + 2, o0:o0 + w],
                                    start=False, stop=last,
                                    perf_mode=DR, skip_group_check=True)
                        osl = outsb[:, o0:o0 + w]
                        nc.vector.scalar_tensor_tensor(
                            osl, yp[:, 0:w], 1.0 / GSCALE,
                            ofold[:, o0:o0 + w], op0=ALU.mult, op1=ALU.add)
                        nc.sync.dma_start(out_io[:, o0:o0 + w], osl)

    nc.compile()
    return nc


def get_program():
    if "nc" not in _compiled:
        _compiled["nc"] = _build()
    return _compiled["nc"]


def _bilinear_kron():
    """K[(k,j), (R,Cc)] = uv[R,k]*uv[Cc,j] for x8 bilinear upsample 12->96
    (align_corners=False, edge-clamped), split into two 72-row halves."""
    uv = np.zeros((96, 12), np.float64)
    for R in range(96):
        t = (R + 0.5) / 8.0 - 0.5
        k0 = int(np.floor(t))
        fr = t - k0
        for k, wt in ((k0, 1.0 - fr), (k0 + 1, fr)):
            kc = min(max(k, 0), 11)
            uv[R, kc] += wt
    K = np.einsum("Rk,Cj->kjRC", uv, uv).reshape(144, 9216)
    return np.ascontiguousarray(K).astype(np.float16)


def make_in_maps(inputs):
    f16 = np.float16
    x = np.asarray(inputs["x"], np.float32).reshape(C, H, W)
    xflat = np.ascontiguousarray(x.reshape(C, N))
    xpad = np.zeros((C, 98, 98), f16)
    xpad[:, 1:97, 1:97] = x.astype(f16)
    krF = _bilinear_kron()

    def conv_w(w):
        # [o, i, dy, dx] -> [i, (dy dx), o]
        return np.ascontiguousarray(
            np.asarray(w, np.float32).transpose(1, 2, 3, 0).reshape(C, 9 * C)
        ).astype(f16)

    base = {
        "xpad": xpad,
        "w1": conv_w(inputs["d1_w"]),
        "w2": conv_w(inputs["d2_w"]),
        "w3": conv_w(inputs["d3_w"]),
        "twT": np.ascontiguousarray(
            np.asarray(inputs["th_w"], np.float32)[:, :, 0, 0].T).astype(f16),
        "pwT": np.ascontiguousarray(
            np.asarray(inputs["ph_w"], np.float32)[:, :, 0, 0].T).astype(f16),
        "gw": np.ascontiguousarray(
            np.asarray(inputs["g_w"], np.float32)[:, :, 0, 0]),
        "WwT": np.ascontiguousarray(
            np.asarray(inputs["W_w"], np.float32)[:, :, 0, 0].T),
    }
    in_maps = []
    for k in range(NCORES):
        m = dict(base)
        m["xch"] = np.ascontiguousarray(
            xflat[:, k * CH:(k + 1) * CH]).astype(f16)
        m["krC"] = np.ascontiguousarray(krF[:, k * CH:(k + 1) * CH])
        in_maps.append(m)
    return in_maps


def kernel(**inputs):
    from concourse import bass_utils

    nc = get_program()
    in_maps = make_in_maps(inputs)
    res = bass_utils.run_bass_kernel_spmd(nc, in_maps,
                                          core_ids=list(range(NCORES)))
    out = np.concatenate([res.results[k]["out"] for k in range(NCORES)], axis=1)
    return out.reshape(1, C, H, W).astype(np.float32)


# revision 105
# speedup vs baseline: 1.0119x; 1.0007x over previous
"""Trainium2 Bass kernel for AttentiveNonLocalBlock2D (AllGather design).

Sequence-parallel over N=H*W across 8 cores, per the sharding hint's
tensor-parallel scheme: each core computes the gate + projections ONLY for
its own 1152-pixel chunk, then phi [32,1152] and G^T [128,9*64] are
AllGathered (DRAM-staged collectives) to form the full phi [32,9216] /
G [128,72*64] every core needs for its n-slice of the attention.

Per core:
  Phase A (one pool scope, no mid barriers): identity-matmul PE p-state
    warm-up under the input DMAs; 3x stride-2 conv gating unit (fp16 PE,
    lrelu = 0.6x+0.4|x| via ACT Abs + DVE stt); conv3 emits pre-transposed
    y3T halves; bilinear-upsample columns for the OWN chunk only via the
    per-core krC input (y3T^T @ krC) -> sigmoid -> fp16 gate-mul ->
    phi/theta/G^T projections.  The AllGathers + Exp table load launch
    outside the phase-A pools so no close-barrier gates pass-1 on them.
  Pass 1: 55 exp instructions (52x1536 cols = 1.33 m-tiles each, then
    tile-aligned 768/1152/1152 tails; the fp8 cache is contiguous so spans
    may cross m-tile boundaries): PE score matmuls fT = phi_tile^T
    theta_chunk into a 2-buffer PSUM ring, ACT exp(f - 2.5) written
    straight to a float8e5 cache (e5m2: wide range so the softmax
    denominator doesn't lose its tail to subnormal flushing; e4m3 loses
    ~10% of Z's mass).  Z[m] partials via DVE dead-store tensor_scalar
    accumulation over the cache (2x SBUF mode), except the final two
    tile-aligned instrs which use ACT's f32 accumulator (shortest path
    into the last Z-AllReduce); Z is AllReduced in 6 segments.
  Pass 2: per segment G is scaled by GSCALE/Z and split into fp8e4 high +
    residual parts; fp8 DoubleRow matmuls (2 m-tiles/instr, 0.5 cy/col,
    e4 stationary x e5 moving) accumulate into per-segment PSUM banks,
    pace-interleaved between later pass-1 instrs (margins keep not-ready
    units from parking at the PE queue head, which would starve ACT);
    DVE adds across segments, final read-out divides by GSCALE and adds
    the gated residual.  The last two segments drain after the loop behind
    a PE warm-up burst that keeps the p-state up through the final Z
    AllReduce latency.
  Host concatenates the per-core n-chunks.

Single-device build (the TimelineSim timing variant) replaces each
collective with one DRAM-hop DMA (upload straight to the gathered buffer);
landing DMAs are modeled in full.
"""

import sys

if "/opt/trn_rl_repo" not in sys.path:
    sys.path.insert(0, "/opt/trn_rl_repo")

import numpy as np

NCORES = 8
C, CI, H, W = 64, 32, 96, 96
N = H * W            # 9216
CH = N // NCORES     # 1152 pixels per core
MT = N // 128        # 72 m-tiles of 128
TPC = MT // NCORES   # 9 own m-tiles per core
EXP_BIAS = -2.5      # keeps exp(f+bias) <= ~16k < 57344 (e5m2 max) while
                     # minimizing subnormal flushing of tiny softmax terms
GSCALE = 64.0 * float(np.exp(-2.5 + 7.5))
                     # pre-scale so G*GSCALE/Z clears the e4m3 subnormal
                     # floor; tracks EXP_BIAS (Z scales with exp(bias))
SEGS = ((0, 22), (22, 40), (40, 54), (54, 64), (64, 70), (70, 72))
EIW = 1536           # exp-instruction width (cols)
# 52 x 1536-col instrs, then tile-aligned tails (768, 1152, 1152): the last
# two instrs cover exactly tiles 70 / 71 so their Z comes from the ACT f32
# accumulator (saves the DVE round trip on the final Z-AllReduce chain)
INSTRS = tuple([(i * EIW, EIW) for i in range(52)]
               + [(52 * EIW, 768), (70 * CH, CH), (71 * CH, CH)])
MARGINS = (7, 8, 7, 5, 99, 99)  # exp-instrs between AR issue and pass-2
                                # emit; last two segs drain after the loop
BUDGET = (3, 4)      # pass-2 units per exp instr (early, late)
RESID = True         # add an fp8 residual pass for G (extra accuracy)
# n-chunk subtiles for the two PSUM ring halves (bank-boundary aligned)
SUBS0 = ((0, 512), (512, 512), (1024, 128))
SUBS1 = ((0, 384), (384, 512), (896, 256))
YSUBS = ((0, 512), (512, 512), (1024, 128))  # pass-2 output subtiles

_compiled = {}


def _zmode(j):
    """Z accumulation engine per tile: DVE only (the dead-store
    tensor_scalar opcode does not exist on GPSIMD, and ACT's accumulator
    cannot be used because exp instructions span m-tile boundaries)."""
    return "dve"


def _seg_of(j):
    for k, (j0, j1) in enumerate(SEGS):
        if j0 <= j < j1:
            return k, j0
    raise ValueError(j)


def _build(single=False):
    import concourse.bacc as bacc
    import concourse.bass as bass
    import concourse.mybir as mybir
    import concourse.tile as tile
    from concourse import masks

    f16 = mybir.dt.float16
    f32 = mybir.dt.float32
    f8 = mybir.dt.float8e4
    f8w = mybir.dt.float8e5   # exp cache: wide range so tiny softmax terms
                              # aren't flushed (Z would lose ~10% of its mass)
    DR = mybir.MatmulPerfMode.DoubleRow
    AF = mybir.ActivationFunctionType
    ALU = mybir.AluOpType

    nc = bacc.Bacc("TRN2", target_bir_lowering=False, debug=False,
                   num_devices=1 if single else NCORES)

    xpad_io = nc.dram_tensor("xpad", [C, 98, 98], f16, kind="ExternalInput")
    w1_io = nc.dram_tensor("w1", [C, 9 * C], f16, kind="ExternalInput")
    w2_io = nc.dram_tensor("w2", [C, 9 * C], f16, kind="ExternalInput")
    w3_io = nc.dram_tensor("w3", [C, 9 * C], f16, kind="ExternalInput")
    twT_io = nc.dram_tensor("twT", [C, CI], f16, kind="ExternalInput")
    pwT_io = nc.dram_tensor("pwT", [C, CI], f16, kind="ExternalInput")
    gw_io = nc.dram_tensor("gw", [CI, C], f32, kind="ExternalInput")
    WwT_io = nc.dram_tensor("WwT", [CI, C], f32, kind="ExternalInput")
    xch_io = nc.dram_tensor("xch", [C, CH], f16, kind="ExternalInput")
    krC_io = nc.dram_tensor("krC", [144, CH], f16, kind="ExternalInput")
    out_io = nc.dram_tensor("out", [C, CH], f32, kind="ExternalOutput")

    groups = [list(range(NCORES))]

    with tile.TileContext(nc) as tc:
        with tc.tile_pool(name="persist", bufs=1) as pp, \
             tc.tile_pool(name="dram", bufs=1, space="DRAM") as dp:
            # per-segment Z tiles so the AR DMA reads never alias later writes
            zsumk = [pp.tile([128, j1 - j0], f32, name=f"zsum{k}")
                     for k, (j0, j1) in enumerate(SEGS)]
            zredk = [pp.tile([128, j1 - j0], f32, name=f"zred{k}")
                     for k, (j0, j1) in enumerate(SEGS)]
            nb5 = pp.tile([128, 1], f32)
            nc.gpsimd.memset(nb5[:], EXP_BIAS)
            zin = [dp.tile([128, j1 - j0], f32, name=f"zin{k}")
                   for k, (j0, j1) in enumerate(SEGS)]
            zout = [dp.tile([128, j1 - j0], f32, addr_space="Shared",
                            name=f"zout{k}")
                    for k, (j0, j1) in enumerate(SEGS)]
            phin = dp.tile([CI, CH], f16, name="phin")
            phout = dp.tile([NCORES, CI, CH], f16, addr_space="Shared",
                            name="phout")
            gin = dp.tile([128, TPC * C], f16, name="gin")
            gout = dp.tile([NCORES, 128, TPC * C], f16, addr_space="Shared",
                           name="gout")

            with tc.tile_pool(name="hand", bufs=1) as hp:
                phi16 = hp.tile([CI, N], f16)
                th16 = hp.tile([CI, CH], f16)
                G16 = hp.tile([128, MT * C], f16)
                G3 = G16[:].rearrange("p (j c) -> p j c", c=C)
                G8 = hp.tile([128, MT * C], f8)
                G83 = G8[:].rearrange("p (j c) -> p j c", c=C)
                R8 = hp.tile([128, MT * C], f8)
                R83 = R8[:].rearrange("p (j c) -> p j c", c=C)
                xgc16 = hp.tile([C, CH], f16)
                outsb = hp.tile([C, CH], f32)
                ofold = hp.tile([C, CH], f32)  # segs 0-3 sum, pre-folded
                # (exp-table load is implicit before the first pass-1 exp;
                # it hides behind the phi AllGather landing wait)
                zdeadV = hp.tile([128, CH], f8w)  # dead stores for Z accum
                zdeadP = hp.tile([128, CH], f8w)  # (same dtype as the cache)
                phiown = hp.tile([CI, CH], f16)
                gown = hp.tile([128, TPC * C], f16)
                s8 = hp.tile([128, MT * CH], f8w)
                s83 = s8[:].rearrange("p (j n) -> p j n", n=CH)

                # ==================== PHASE A ====================
                # single merged pool scope: no mid-phase close barrier
                # between the convs and the gate/projection pipeline
                with tc.tile_pool(name="pa", bufs=1) as pa, \
                     tc.tile_pool(name="paps", bufs=2, space="PSUM") as paps:
                    y3Ta = pa.tile([72, C], f16)
                    y3Tb = pa.tile([72, C], f16)
                    # preload the Sigmoid table while input DMAs fly
                    tld0 = pa.tile([128, 1], f32)
                    nc.scalar.activation(tld0[:], nb5[:], AF.Sigmoid)
                    # ramp the PE p-state during the input-DMA wait so conv1
                    # runs at full speed from its first matmul (identity
                    # needs no DMA)
                    ident = pa.tile([C, C], f16)
                    masks.make_identity(nc, ident[:])
                    wmps = paps.tile([C, C], f32, tag="warm", name="wmps",
                                     bufs=1)
                    for _ in range(140):
                        nc.tensor.matmul(wmps[:], ident[:], ident[:],
                                         start=True, stop=True,
                                         skip_group_check=True)

                    # conv-critical DMAs first: HWDGE is one serial queue,
                    # and conv1 must run gapless to keep the PE p-state up
                    xpad = pa.tile([C, 98, 98], f16)
                    w1sb = pa.tile([C, 9 * C], f16)
                    nc.sync.dma_start(xpad[:, 0:18, :], xpad_io[:, 0:18, :])
                    nc.sync.dma_start(w1sb[:], w1_io[:])
                    nc.sync.dma_start(xpad[:, 18:50, :], xpad_io[:, 18:50, :])
                    nc.sync.dma_start(xpad[:, 50:98, :], xpad_io[:, 50:98, :])
                    w2sb = pa.tile([C, 9 * C], f16)
                    nc.sync.dma_start(w2sb[:], w2_io[:])
                    w3sb = pa.tile([C, 9 * C], f16)
                    nc.sync.dma_start(w3sb[:], w3_io[:])
                    twT16 = pa.tile([C, CI], f16)
                    nc.sync.dma_start(twT16[:], twT_io[:])
                    pwT16 = pa.tile([C, CI], f16)
                    nc.sync.dma_start(pwT16[:], pwT_io[:])
                    gwsb = pa.tile([CI, C], f32)
                    nc.sync.dma_start(gwsb[:], gw_io[:])
                    WwTsb = pa.tile([CI, C], f32)
                    nc.sync.dma_start(WwTsb[:], WwT_io[:])
                    krCa = pa.tile([72, CH], f16)
                    nc.sync.dma_start(krCa[:], krC_io[0:72, :])
                    krCb = pa.tile([72, CH], f16)
                    nc.sync.dma_start(krCb[:], krC_io[72:144, :])
                    xchsb = pa.tile([C, CH], f16)
                    nc.sync.dma_start(xchsb[:], xch_io[:])

                    # conv1: 96x96 -> 48x48, stride 2, pad 1, lrelu(0.2)
                    y1p = pa.tile([C, 50, 50], f16)
                    nc.gpsimd.memset(y1p[:], 0.0)
                    for g in range(6):
                        ps1 = paps.tile([C, 8, 48], f32, tag="cv", name="ps1")
                        for t in range(9):
                            dy, dx = t // 3, t % 3
                            nc.tensor.matmul(
                                ps1[:], w1sb[:, t * C:(t + 1) * C],
                                xpad[:, 16 * g + dy: 16 * g + dy + 16: 2,
                                     dx: dx + 96: 2],
                                start=(t == 0), stop=(t == 8))
                        # lrelu(x) = 0.6*x + 0.4*|x| (only one PSUM input
                        # allowed per DVE op; ACT is idle during the convs)
                        ab1 = pa.tile([C, 8 * 48], f32, tag="ab1", name="ab1",
                                      bufs=2)
                        nc.scalar.activation(ab1[:], ps1[:], AF.Abs,
                                             scale=0.4)
                        nc.vector.scalar_tensor_tensor(
                            y1p[:, 1 + 8 * g: 9 + 8 * g, 1:49], ps1[:], 0.6,
                            ab1[:], op0=ALU.mult, op1=ALU.add)

                    # conv2: 48x48 -> 24x24
                    y2p = pa.tile([C, 26, 26], f16)
                    nc.gpsimd.memset(y2p[:], 0.0)
                    for g in range(2):
                        ps2 = paps.tile([C, 12, 24], f32, tag="cv", name="ps2")
                        for t in range(9):
                            dy, dx = t // 3, t % 3
                            nc.tensor.matmul(
                                ps2[:], w2sb[:, t * C:(t + 1) * C],
                                y1p[:, 24 * g + dy: 24 * g + dy + 24: 2,
                                    dx: dx + 48: 2],
                                start=(t == 0), stop=(t == 8))
                        ab2 = pa.tile([C, 12 * 24], f32, tag="ab2", name="ab2",
                                      bufs=2)
                        nc.scalar.activation(ab2[:], ps2[:], AF.Abs,
                                             scale=0.4)
                        nc.vector.scalar_tensor_tensor(
                            y2p[:, 1 + 12 * g: 13 + 12 * g, 1:25], ps2[:], 0.6,
                            ab2[:], op0=ALU.mult, op1=ALU.add)

                    # conv3: 24x24 -> 12x12 (no activation), then PE
                    # transpose into y3T[(row, col), c] halves
                    ps3 = paps.tile([C, 12, 12], f32, tag="cv", name="ps3")
                    for t in range(9):
                        dy, dx = t // 3, t % 3
                        nc.tensor.matmul(
                            ps3[:], w3sb[:, t * C:(t + 1) * C],
                            y2p[:, dy: dy + 24: 2, dx: dx + 24: 2],
                            start=(t == 0), stop=(t == 8))
                    y3f = pa.tile([C, 144], f16)
                    nc.vector.tensor_copy(y3f[:], ps3[:])
                    for hh, y3t in ((0, y3Ta), (1, y3Tb)):
                        pst = paps.tile([72, C], f16, tag="cv", name="pst")
                        nc.tensor.transpose(
                            pst[:], y3f[:, 72 * hh:72 * (hh + 1)], ident[:])
                        nc.vector.tensor_copy(y3t[:], pst[:])

                    # E^T = gw^T WwT [C, C]
                    eps = paps.tile([C, 512], f32, tag="prj", name="eps",
                                    bufs=3)
                    nc.tensor.matmul(eps[:, 0:C], gwsb[:], WwTsb[:],
                                     start=True, stop=True)
                    ET16 = hp.tile([C, C], f16)
                    nc.vector.tensor_copy(ET16[:], eps[:, 0:C])

                    # gate pipeline: all krons first (kron -> sigmoid ->
                    # fp16 gate-mul per sub), then the phi chain (it feeds
                    # the AllGather = the pass-1 critical path), then theta
                    gtc = pa.tile([C, CH], f16)
                    for o0, w in SUBS0:
                        kps = paps.tile([C, 512], f32, tag="prj",
                                        name="kps", bufs=3)
                        nc.tensor.matmul(kps[:, 0:w], y3Ta[:],
                                         krCa[:, o0:o0 + w],
                                         start=True, stop=False)
                        nc.tensor.matmul(kps[:, 0:w], y3Tb[:],
                                         krCb[:, o0:o0 + w],
                                         start=False, stop=True)
                        nc.scalar.activation(gtc[:, o0:o0 + w],
                                             kps[:, 0:w], AF.Sigmoid)
                        nc.vector.tensor_mul(xgc16[:, o0:o0 + w],
                                             gtc[:, o0:o0 + w],
                                             xchsb[:, o0:o0 + w])
                    # preload the Exp table during the gate pipeline: the read
                    # of gtc pins this after sigmoid0 (it cannot be hoisted
                    # to t=0 where the sigmoid load would evict it again)
                    tld1 = pa.tile([C, 1], f32)
                    nc.scalar.activation(tld1[:], gtc[:, 0:1], AF.Exp)
                    for o0, w in SUBS0:
                        pps = paps.tile([C, 512], f32, tag="prj",
                                        name="pps", bufs=3)
                        nc.tensor.matmul(pps[0:CI, 0:w], pwT16[:],
                                         xgc16[:, o0:o0 + w],
                                         start=True, stop=True)
                        nc.vector.tensor_copy(phiown[:, o0:o0 + w],
                                              pps[0:CI, 0:w])
                    for o0, w in SUBS0:
                        tps = paps.tile([C, 512], f32, tag="prj",
                                        name="tps", bufs=3)
                        nc.tensor.matmul(tps[0:CI, 0:w], twT16[:],
                                         xgc16[:, o0:o0 + w],
                                         start=True, stop=True)
                        # (GPSIMD cannot read PSUM on HW: copies on DVE)
                        nc.vector.tensor_copy(th16[:, o0:o0 + w],
                                              tps[0:CI, 0:w])

                    # own G^T tiles [128, 9*C] (AllGathered later)
                    gps = paps.tile([128, TPC * C], f32, tag="gps",
                                    name="gps", bufs=1)
                    for u in range(TPC):
                        nc.tensor.matmul(gps[:, u * C:(u + 1) * C],
                                         xgc16[:, u * 128:(u + 1) * 128],
                                         ET16[:], start=True, stop=True)
                    nc.vector.tensor_copy(gown[:], gps[:])

                # ====== PASS 1 with seg-interleaved fp8 PASS 2 + ARs ======
                with tc.tile_pool(name="p1ps", bufs=2, space="PSUM") as p1ps, \
                     tc.tile_pool(name="p2ps", bufs=2, space="PSUM") as p2ps, \
                     tc.tile_pool(name="p2", bufs=1) as p2:
                    # AllGathers emitted inside this scope so no pool-close
                    # barrier or clock alignment gates pass-1 on them.
                    # single-mode convention: ONE DRAM hop stands in for
                    # upload+collective; landing DMAs are modeled in full.
                    if single:
                        nc.sync.dma_start(phout[0, :, :], phiown[:])
                    else:
                        nc.sync.dma_start(phin[:], phiown[:])
                        nc.gpsimd.collective_compute(
                            "AllGather", ALU.bypass, replica_groups=groups,
                            ins=[phin.opt()], outs=[phout.opt()])
                    # land slice r=0 first: it unblocks pass-1 tiles 0-8
                    nc.sync.dma_start(phi16[:, 0:CH], phout[0, :, :])
                    nc.sync.dma_start(
                        phi16[:, CH:].rearrange("c (r n) -> c r n",
                                                r=NCORES - 1),
                        phout[1:, :, :].rearrange("r c n -> c r n"))

                    # warm the PE through the AG landing wait with fake
                    # pass-1 tiles read from phiown (already in SBUF)
                    for _ in range(3):
                        wfps = p1ps.tile([128, EIW], f32, tag="fps",
                                         name="fps")
                        for o0 in range(0, EIW, 512):
                            nc.tensor.matmul(wfps[:, o0:o0 + 512],
                                             phiown[:, 0:128],
                                             th16[:, 0:512],
                                             start=True, stop=True)

                    def emit_G_ag():
                        if single:
                            nc.sync.dma_start(gout[0, :, :], gown[:])
                        else:
                            nc.sync.dma_start(gin[:], gown[:])
                            nc.gpsimd.collective_compute(
                                "AllGather", ALU.bypass,
                                replica_groups=groups,
                                ins=[gin.opt()], outs=[gout.opt()])
                        nc.sync.dma_start(
                            G16[:].rearrange("p (r n) -> p r n", r=NCORES),
                            gout[:].rearrange("r p n -> p r n"))

                    def pass1_instr(i):
                        # one exp instruction = up to 1.33 m-tiles; the fp8
                        # cache is contiguous so the exp span can cross
                        # m-tile boundaries; Z is per-m-tile off the cache,
                        # except single-tile-aligned instrs which use the
                        # ACT f32 accumulator directly
                        c0, wd = INSTRS[i]
                        fps = p1ps.tile([128, wd], f32, tag="fps",
                                        name="fps")
                        edges = {0, wd}
                        for b in range(512, wd, 512):
                            edges.add(b)
                        jlo, jhi = c0 // CH, (c0 + wd - 1) // CH
                        for j in range(jlo, jhi + 1):
                            if c0 < j * CH < c0 + wd:
                                edges.add(j * CH - c0)
                        edges = sorted(edges)
                        for a, b in zip(edges[:-1], edges[1:]):
                            j = (c0 + a) // CH
                            ta = c0 + a - j * CH
                            nc.tensor.matmul(fps[:, a:b],
                                             phi16[:, j * 128:(j + 1) * 128],
                                             th16[:, ta:ta + (b - a)],
                                             start=True, stop=True)
                        aligned = (wd == CH and c0 % CH == 0)
                        if aligned:
                            j = c0 // CH
                            k, j0 = _seg_of(j)
                            nc.scalar.activation(
                                s8[:, c0:c0 + wd], fps[:], AF.Exp,
                                bias=nb5[:], scale=1.0,
                                accum_out=zsumk[k][:, j - j0:j - j0 + 1])
                            return
                        nc.scalar.activation(s8[:, c0:c0 + wd], fps[:],
                                             AF.Exp, bias=nb5[:], scale=1.0)
                        # Z for every m-tile completed by this instruction
                        for j in range(jlo, jhi + 1):
                            if (j + 1) * CH <= c0 + wd:
                                k, j0 = _seg_of(j)
                                zcol = zsumk[k][:, j - j0:j - j0 + 1]
                                nc.vector.tensor_scalar(
                                    zdeadV[:], s83[:, j, :], 1.0, 0.0,
                                    op0=ALU.mult, op1=ALU.add,
                                    accum_out=zcol)

                    def allreduce(k):
                        # single-mode convention (as for the AllGathers):
                        # one DRAM hop stands in for upload+collective
                        if single:
                            nc.sync.dma_start(zout[k][:], zsumk[k][:])
                        else:
                            nc.sync.dma_start(zin[k][:], zsumk[k][:])
                            nc.gpsimd.collective_compute(
                                "AllReduce", ALU.add,
                                replica_groups=groups,
                                ins=[zin[k].opt()], outs=[zout[k].opt()])
                        nc.sync.dma_start(zredk[k][:], zout[k][:])

                    def scale_G(k):
                        j0, j1 = SEGS[k]
                        ln = j1 - j0
                        zf = p2.tile([128, 22], f32, tag="zf", name="zf",
                                     bufs=2)
                        # 1/(Z/GSCALE) = GSCALE/Z
                        nc.vector.tensor_scalar(zf[:, 0:ln], zredk[k][:],
                                                1.0 / GSCALE, None,
                                                op0=ALU.mult)
                        rz = p2.tile([128, 22], f32, tag="rz", name="rz",
                                     bufs=2)
                        nc.vector.reciprocal(rz[:, 0:ln], zf[:, 0:ln])
                        rzb = rz[:, 0:ln].unsqueeze(-1).to_broadcast(
                            (128, ln, C))
                        nc.vector.tensor_mul(G3[:, j0:j1, :],
                                             G3[:, j0:j1, :], rzb)
                        nc.vector.tensor_copy(G83[:, j0:j1, :],
                                              G3[:, j0:j1, :])
                        if RESID:
                            # split G into fp8 high + fp8 residual parts
                            rt = p2.tile([128, 22 * C], f16, tag="rt",
                                         name="rt", bufs=2)
                            rt3 = rt[:].rearrange("p (j c) -> p j c", c=C)
                            nc.vector.tensor_sub(rt3[:, 0:ln, :],
                                                 G3[:, j0:j1, :],
                                                 G83[:, j0:j1, :])
                            nc.vector.tensor_copy(R83[:, j0:j1, :],
                                                  rt3[:, 0:ln, :])

                    # pass-2 work units: (k, ci, u); per-segment PSUM
                    # accumulation, DVE adds across segments into outsb
                    units = []
                    for k in range(len(SEGS)):
                        j0, j1 = SEGS[k]
                        for ci in range(len(YSUBS)):
                            for u in range((j1 - j0) // 2):
                                units.append((k, ci, u))
                    emitted = 0
                    cur_ps = {}

                    def emit_unit():
                        nonlocal emitted
                        k, ci, u = units[emitted]
                        j0, j1 = SEGS[k]
                        o0, w = YSUBS[ci]
                        npr = (j1 - j0) // 2
                        jj = j0 + 2 * u
                        if u == 0:
                            cur_ps[ci] = p2ps.tile([C, 512], f32, tag="yps",
                                                   name="yps")
                        yp = cur_ps[ci]
                        nc.tensor.matmul(
                            yp[:, 0:w], G83[:, jj:jj + 2, :],
                            s83[:, jj:jj + 2, o0:o0 + w],
                            start=(u == 0), stop=(not RESID and u == npr - 1),
                            perf_mode=DR, skip_group_check=True)
                        if RESID:
                            nc.tensor.matmul(
                                yp[:, 0:w], R83[:, jj:jj + 2, :],
                                s83[:, jj:jj + 2, o0:o0 + w],
                                start=False, stop=(u == npr - 1),
                                perf_mode=DR, skip_group_check=True)
                        if u == npr - 1:
                            osl = outsb[:, o0:o0 + w]
                            if k == 0:
                                nc.vector.tensor_copy(osl, yp[:, 0:w])
                            else:
                                nc.vector.tensor_add(osl, osl, yp[:, 0:w])
                            if k == 3:
                                # pre-fold GSCALE + gated residual for segs
                                # 0-3 NOW (hidden in the loop) so the final
                                # drain read-out is a single stt per sub
                                nc.vector.scalar_tensor_tensor(
                                    ofold[:, o0:o0 + w], osl, 1.0 / GSCALE,
                                    xgc16[:, o0:o0 + w],
                                    op0=ALU.mult, op1=ALU.add)
                        emitted += 1

                    # m-tile j's exp completes during exp-instr ei(j)
                    def ei_of(j):
                        end = (j + 1) * CH
                        for i, (c0, wd) in enumerate(INSTRS):
                            if c0 + wd >= end:
                                return i
                        raise ValueError(j)

                    seg_ei = [ei_of(s[1] - 1) for s in SEGS]
                    seg_units = [sum(1 for x in units if x[0] <= k)
                                 for k in range(len(SEGS))]
                    avail = [0]

                    def pump(i):
                        if i == 2:
                            emit_G_ag()
                        for k in range(len(SEGS)):
                            if i == seg_ei[k]:
                                allreduce(k)
                                scale_G(k)
                            if (k < len(SEGS) - 2
                                    and i == seg_ei[k] + MARGINS[k]):
                                # last 2 segs drain after the loop, behind
                                # the PE warm-up (parked units would block
                                # the warm-up and drop the p-state)
                                avail[0] = seg_units[k]
                        budget = (BUDGET[0] if i < 30 else
                                  2 if i >= 52 else BUDGET[1])
                        while emitted < avail[0] and budget > 0:
                            emit_unit()
                            budget -= 1

                    for i in range(len(INSTRS)):
                        pass1_instr(i)
                        pump(i)
                    # keep the PE p-state warm through the final Z-AR wait:
                    # re-run an already-satisfied pair into a scratch bank
                    wps = p2ps.tile([C, 512], f32, tag="yps", name="wps")
                    for _ in range(16):
                        nc.tensor.matmul(wps[:], G83[:, 0:2, :],
                                         s83[:, 0:2, 0:512],
                                         start=True, stop=True, perf_mode=DR,
                                         skip_group_check=True)
                    # segs <=3 stragglers, then segs 4+5 merged per output
                    # sub: one PSUM accumulation spanning both, and the
                    # final read-out is a single stt (yp/GSCALE + ofold)
                    while emitted < seg_units[3]:
                        emit_unit()
                    prs = [(k, u) for k in (4, 5)
                           for u in range((SEGS[k][1] - SEGS[k][0]) // 2)]
                    for ci, (o0, w) in enumerate(YSUBS):
                        yp = p2ps.tile([C, 512], f32, tag="yps", name="yps")
                        for pi, (k, u) in enumerate(prs):
                            jj = SEGS[k][0] + 2 * u
                            first = pi == 0
                            last = pi == len(prs) - 1
                            nc.tensor.matmul(
                                yp[:, 0:w], G83[:, jj:jj + 2, :],
                                s83[:, jj:jj + 2, o0:o0 + w],
                                start=first, stop=(not RESID and last),
                                perf_mode=DR, skip_group_check=True)
                            if RESID:
                                nc.tensor.matmul(
                                    yp[:, 0:w], R83[:, jj:jj + 2, :],
                                    s83[:, jj:jj + 2, o0:o0 + w],
                                    start=False, stop=last,
                                    perf_mode=DR, skip_group_check=True)
                        osl = outsb[:, o0:o0 + w]
                        nc.vector.scalar_tensor_tensor(
                            osl, yp[:, 0:w], 1.0 / GSCALE,
                            ofold[:, o0:o0 + w], op0=ALU.mult, op1=ALU.add)
                        nc.sync.dma_start(out_io[:, o0:o0 + w], osl)

    nc.compile()
    return nc


def get_program():
    if "nc" not in _compiled:
        _compiled["nc"] = _build()
    return _compiled["nc"]


def _bilinear_kron():
    """K[(k,j), (R,Cc)] = uv[R,k]*uv[Cc,j] for x8 bilinear upsample 12->96
    (align_corners=False, edge-clamped), split into two 72-row halves."""
    uv = np.zeros((96, 12), np.float64)
    for R in range(96):
        t = (R + 0.5) / 8.0 - 0.5
        k0 = int(np.floor(t))
        fr = t - k0
        for k, wt in ((k0, 1.0 - fr), (k0 + 1, fr)):
            kc = min(max(k, 0), 11)
            uv[R, kc] += wt
    K = np.einsum("Rk,Cj->kjRC", uv, uv).reshape(144, 9216)
    return np.ascontiguousarray(K).astype(np.float16)


def make_in_maps(inputs):
    f16 = np.float16
    x = np.asarray(inputs["x"], np.float32).reshape(C, H, W)
    xflat = np.ascontiguousarray(x.reshape(C, N))
    xpad = np.zeros((C, 98, 98), f16)
    xpad[:, 1:97, 1:97] = x.astype(f16)
    krF = _bilinear_kron()

    def conv_w(w):
        # [o, i, dy, dx] -> [i, (dy dx), o]
        return np.ascontiguousarray(
            np.asarray(w, np.float32).transpose(1, 2, 3, 0).reshape(C, 9 * C)
        ).astype(f16)

    base = {
        "xpad": xpad,
        "w1": conv_w(inputs["d1_w"]),
        "w2": conv_w(inputs["d2_w"]),
        "w3": conv_w(inputs["d3_w"]),
        "twT": np.ascontiguousarray(
            np.asarray(inputs["th_w"], np.float32)[:, :, 0, 0].T).astype(f16),
        "pwT": np.ascontiguousarray(
            np.asarray(inputs["ph_w"], np.float32)[:, :, 0, 0].T).astype(f16),
        "gw": np.ascontiguousarray(
            np.asarray(inputs["g_w"], np.float32)[:, :, 0, 0]),
        "WwT": np.ascontiguousarray(
            np.asarray(inputs["W_w"], np.float32)[:, :, 0, 0].T),
    }
    in_maps = []
    for k in range(NCORES):
        m = dict(base)
        m["xch"] = np.ascontiguousarray(
            xflat[:, k * CH:(k + 1) * CH]).astype(f16)
        m["krC"] = np.ascontiguousarray(krF[:, k * CH:(k + 1) * CH])
        in_maps.append(m)
    return in_maps


def kernel(**inputs):
    from concourse import bass_utils

    nc = get_program()
    in_maps = make_in_maps(inputs)
    res = bass_utils.run_bass_kernel_spmd(nc, in_maps,
                                          core_ids=list(range(NCORES)))
    out = np.concatenate([res.results[k]["out"] for k in range(NCORES)], axis=1)
    return out.reshape(1, C, H, W).astype(np.float32)


# revision 106
# speedup vs baseline: 1.0142x; 1.0023x over previous
"""Trainium2 Bass kernel for AttentiveNonLocalBlock2D (AllGather design).

Sequence-parallel over N=H*W across 8 cores, per the sharding hint's
tensor-parallel scheme: each core computes the gate + projections ONLY for
its own 1152-pixel chunk, then phi [32,1152] and G^T [128,9*64] are
AllGathered (DRAM-staged collectives) to form the full phi [32,9216] /
G [128,72*64] every core needs for its n-slice of the attention.

Per core:
  Phase A (one pool scope, no mid barriers): identity-matmul PE p-state
    warm-up under the input DMAs; 3x stride-2 conv gating unit (fp16 PE,
    lrelu = 0.6x+0.4|x| via ACT Abs + DVE stt); conv3 emits pre-transposed
    y3T halves; bilinear-upsample columns for the OWN chunk only via the
    per-core krC input (y3T^T @ krC) -> sigmoid -> fp16 gate-mul ->
    phi/theta/G^T projections.  The AllGathers + Exp table load launch
    outside the phase-A pools so no close-barrier gates pass-1 on them.
  Pass 1: 55 exp instructions (52x1536 cols = 1.33 m-tiles each, then
    tile-aligned 768/1152/1152 tails; the fp8 cache is contiguous so spans
    may cross m-tile boundaries): PE score matmuls fT = phi_tile^T
    theta_chunk into a 2-buffer PSUM ring, ACT exp(f - 2.5) written
    straight to a float8e5 cache (e5m2: wide range so the softmax
    denominator doesn't lose its tail to subnormal flushing; e4m3 loses
    ~10% of Z's mass).  Z[m] partials via DVE dead-store tensor_scalar
    accumulation over the cache (2x SBUF mode), except the final two
    tile-aligned instrs which use ACT's f32 accumulator (shortest path
    into the last Z-AllReduce); Z is AllReduced in 6 segments.
  Pass 2: per segment G is scaled by GSCALE/Z and split into fp8e4 high +
    residual parts; fp8 DoubleRow matmuls (2 m-tiles/instr, 0.5 cy/col,
    e4 stationary x e5 moving) accumulate into per-segment PSUM banks,
    pace-interleaved between later pass-1 instrs (margins keep not-ready
    units from parking at the PE queue head, which would starve ACT);
    DVE adds across segments, final read-out divides by GSCALE and adds
    the gated residual.  The last two segments drain after the loop behind
    a PE warm-up burst that keeps the p-state up through the final Z
    AllReduce latency.
  Host concatenates the per-core n-chunks.

Single-device build (the TimelineSim timing variant) replaces each
collective with one DRAM-hop DMA (upload straight to the gathered buffer);
landing DMAs are modeled in full.
"""

import sys

if "/opt/trn_rl_repo" not in sys.path:
    sys.path.insert(0, "/opt/trn_rl_repo")

import numpy as np

NCORES = 8
C, CI, H, W = 64, 32, 96, 96
N = H * W            # 9216
CH = N // NCORES     # 1152 pixels per core
MT = N // 128        # 72 m-tiles of 128
TPC = MT // NCORES   # 9 own m-tiles per core
EXP_BIAS = -2.5      # keeps exp(f+bias) <= ~16k < 57344 (e5m2 max) while
                     # minimizing subnormal flushing of tiny softmax terms
GSCALE = 64.0 * float(np.exp(-2.5 + 7.5))
                     # pre-scale so G*GSCALE/Z clears the e4m3 subnormal
                     # floor; tracks EXP_BIAS (Z scales with exp(bias))
SEGS = ((0, 22), (22, 40), (40, 54), (54, 64), (64, 70), (70, 72))
EIW = 1536           # exp-instruction width (cols)
# 52 x 1536-col instrs, then tile-aligned tails (768, 1152, 1152): the last
# two instrs cover exactly tiles 70 / 71 so their Z comes from the ACT f32
# accumulator (saves the DVE round trip on the final Z-AllReduce chain)
INSTRS = tuple([(i * EIW, EIW) for i in range(52)]
               + [(52 * EIW, 768), (70 * CH, CH), (71 * CH, CH)])
MARGINS = (7, 8, 7, 4, 99, 99)  # exp-instrs between AR issue and pass-2
                                # emit; last two segs drain after the loop
BUDGET = (3, 4)      # pass-2 units per exp instr (early, late)
RESID = True         # add an fp8 residual pass for G (extra accuracy)
# n-chunk subtiles for the two PSUM ring halves (bank-boundary aligned)
SUBS0 = ((0, 512), (512, 512), (1024, 128))
SUBS1 = ((0, 384), (384, 512), (896, 256))
YSUBS = ((0, 512), (512, 512), (1024, 128))  # pass-2 output subtiles

_compiled = {}


def _zmode(j):
    """Z accumulation engine per tile: DVE only (the dead-store
    tensor_scalar opcode does not exist on GPSIMD, and ACT's accumulator
    cannot be used because exp instructions span m-tile boundaries)."""
    return "dve"


def _seg_of(j):
    for k, (j0, j1) in enumerate(SEGS):
        if j0 <= j < j1:
            return k, j0
    raise ValueError(j)


def _build(single=False):
    import concourse.bacc as bacc
    import concourse.bass as bass
    import concourse.mybir as mybir
    import concourse.tile as tile
    from concourse import masks

    f16 = mybir.dt.float16
    f32 = mybir.dt.float32
    f8 = mybir.dt.float8e4
    f8w = mybir.dt.float8e5   # exp cache: wide range so tiny softmax terms
                              # aren't flushed (Z would lose ~10% of its mass)
    DR = mybir.MatmulPerfMode.DoubleRow
    AF = mybir.ActivationFunctionType
    ALU = mybir.AluOpType

    nc = bacc.Bacc("TRN2", target_bir_lowering=False, debug=False,
                   num_devices=1 if single else NCORES)

    xpad_io = nc.dram_tensor("xpad", [C, 98, 98], f16, kind="ExternalInput")
    w1_io = nc.dram_tensor("w1", [C, 9 * C], f16, kind="ExternalInput")
    w2_io = nc.dram_tensor("w2", [C, 9 * C], f16, kind="ExternalInput")
    w3_io = nc.dram_tensor("w3", [C, 9 * C], f16, kind="ExternalInput")
    twT_io = nc.dram_tensor("twT", [C, CI], f16, kind="ExternalInput")
    pwT_io = nc.dram_tensor("pwT", [C, CI], f16, kind="ExternalInput")
    gw_io = nc.dram_tensor("gw", [CI, C], f32, kind="ExternalInput")
    WwT_io = nc.dram_tensor("WwT", [CI, C], f32, kind="ExternalInput")
    xch_io = nc.dram_tensor("xch", [C, CH], f16, kind="ExternalInput")
    krC_io = nc.dram_tensor("krC", [144, CH], f16, kind="ExternalInput")
    out_io = nc.dram_tensor("out", [C, CH], f32, kind="ExternalOutput")

    groups = [list(range(NCORES))]

    with tile.TileContext(nc) as tc:
        with tc.tile_pool(name="persist", bufs=1) as pp, \
             tc.tile_pool(name="dram", bufs=1, space="DRAM") as dp:
            # per-segment Z tiles so the AR DMA reads never alias later writes
            zsumk = [pp.tile([128, j1 - j0], f32, name=f"zsum{k}")
                     for k, (j0, j1) in enumerate(SEGS)]
            zredk = [pp.tile([128, j1 - j0], f32, name=f"zred{k}")
                     for k, (j0, j1) in enumerate(SEGS)]
            nb5 = pp.tile([128, 1], f32)
            nc.gpsimd.memset(nb5[:], EXP_BIAS)
            zin = [dp.tile([128, j1 - j0], f32, name=f"zin{k}")
                   for k, (j0, j1) in enumerate(SEGS)]
            zout = [dp.tile([128, j1 - j0], f32, addr_space="Shared",
                            name=f"zout{k}")
                    for k, (j0, j1) in enumerate(SEGS)]
            phin = dp.tile([CI, CH], f16, name="phin")
            phout = dp.tile([NCORES, CI, CH], f16, addr_space="Shared",
                            name="phout")
            gin = dp.tile([128, TPC * C], f16, name="gin")
            gout = dp.tile([NCORES, 128, TPC * C], f16, addr_space="Shared",
                           name="gout")

            with tc.tile_pool(name="hand", bufs=1) as hp:
                phi16 = hp.tile([CI, N], f16)
                th16 = hp.tile([CI, CH], f16)
                G16 = hp.tile([128, MT * C], f16)
                G3 = G16[:].rearrange("p (j c) -> p j c", c=C)
                G8 = hp.tile([128, MT * C], f8)
                G83 = G8[:].rearrange("p (j c) -> p j c", c=C)
                R8 = hp.tile([128, MT * C], f8)
                R83 = R8[:].rearrange("p (j c) -> p j c", c=C)
                xgc16 = hp.tile([C, CH], f16)
                outsb = hp.tile([C, CH], f32)
                ofold = hp.tile([C, CH], f32)  # segs 0-3 sum, pre-folded
                # (exp-table load is implicit before the first pass-1 exp;
                # it hides behind the phi AllGather landing wait)
                zdeadV = hp.tile([128, CH], f8w)  # dead stores for Z accum
                zdeadP = hp.tile([128, CH], f8w)  # (same dtype as the cache)
                phiown = hp.tile([CI, CH], f16)
                gown = hp.tile([128, TPC * C], f16)
                s8 = hp.tile([128, MT * CH], f8w)
                s83 = s8[:].rearrange("p (j n) -> p j n", n=CH)

                # ==================== PHASE A ====================
                # single merged pool scope: no mid-phase close barrier
                # between the convs and the gate/projection pipeline
                with tc.tile_pool(name="pa", bufs=1) as pa, \
                     tc.tile_pool(name="paps", bufs=2, space="PSUM") as paps:
                    y3Ta = pa.tile([72, C], f16)
                    y3Tb = pa.tile([72, C], f16)
                    # preload the Sigmoid table while input DMAs fly
                    tld0 = pa.tile([128, 1], f32)
                    nc.scalar.activation(tld0[:], nb5[:], AF.Sigmoid)
                    # ramp the PE p-state during the input-DMA wait so conv1
                    # runs at full speed from its first matmul (identity
                    # needs no DMA)
                    ident = pa.tile([C, C], f16)
                    masks.make_identity(nc, ident[:])
                    wmps = paps.tile([C, C], f32, tag="warm", name="wmps",
                                     bufs=1)
                    for _ in range(140):
                        nc.tensor.matmul(wmps[:], ident[:], ident[:],
                                         start=True, stop=True,
                                         skip_group_check=True)

                    # conv-critical DMAs first: HWDGE is one serial queue,
                    # and conv1 must run gapless to keep the PE p-state up
                    xpad = pa.tile([C, 98, 98], f16)
                    w1sb = pa.tile([C, 9 * C], f16)
                    nc.sync.dma_start(xpad[:, 0:18, :], xpad_io[:, 0:18, :])
                    nc.sync.dma_start(w1sb[:], w1_io[:])
                    nc.sync.dma_start(xpad[:, 18:50, :], xpad_io[:, 18:50, :])
                    nc.sync.dma_start(xpad[:, 50:98, :], xpad_io[:, 50:98, :])
                    w2sb = pa.tile([C, 9 * C], f16)
                    nc.sync.dma_start(w2sb[:], w2_io[:])
                    w3sb = pa.tile([C, 9 * C], f16)
                    nc.sync.dma_start(w3sb[:], w3_io[:])
                    twT16 = pa.tile([C, CI], f16)
                    nc.sync.dma_start(twT16[:], twT_io[:])
                    pwT16 = pa.tile([C, CI], f16)
                    nc.sync.dma_start(pwT16[:], pwT_io[:])
                    gwsb = pa.tile([CI, C], f32)
                    nc.sync.dma_start(gwsb[:], gw_io[:])
                    WwTsb = pa.tile([CI, C], f32)
                    nc.sync.dma_start(WwTsb[:], WwT_io[:])
                    krCa = pa.tile([72, CH], f16)
                    nc.sync.dma_start(krCa[:], krC_io[0:72, :])
                    krCb = pa.tile([72, CH], f16)
                    nc.sync.dma_start(krCb[:], krC_io[72:144, :])
                    xchsb = pa.tile([C, CH], f16)
                    nc.sync.dma_start(xchsb[:], xch_io[:])

                    # conv1: 96x96 -> 48x48, stride 2, pad 1, lrelu(0.2)
                    y1p = pa.tile([C, 50, 50], f16)
                    nc.gpsimd.memset(y1p[:], 0.0)
                    for g in range(6):
                        ps1 = paps.tile([C, 8, 48], f32, tag="cv", name="ps1")
                        for t in range(9):
                            dy, dx = t // 3, t % 3
                            nc.tensor.matmul(
                                ps1[:], w1sb[:, t * C:(t + 1) * C],
                                xpad[:, 16 * g + dy: 16 * g + dy + 16: 2,
                                     dx: dx + 96: 2],
                                start=(t == 0), stop=(t == 8))
                        # lrelu(x) = 0.6*x + 0.4*|x| (only one PSUM input
                        # allowed per DVE op; ACT is idle during the convs)
                        ab1 = pa.tile([C, 8 * 48], f32, tag="ab1", name="ab1",
                                      bufs=2)
                        nc.scalar.activation(ab1[:], ps1[:], AF.Abs,
                                             scale=0.4)
                        nc.vector.scalar_tensor_tensor(
                            y1p[:, 1 + 8 * g: 9 + 8 * g, 1:49], ps1[:], 0.6,
                            ab1[:], op0=ALU.mult, op1=ALU.add)

                    # conv2: 48x48 -> 24x24
                    y2p = pa.tile([C, 26, 26], f16)
                    nc.gpsimd.memset(y2p[:], 0.0)
                    for g in range(2):
                        ps2 = paps.tile([C, 12, 24], f32, tag="cv", name="ps2")
                        for t in range(9):
                            dy, dx = t // 3, t % 3
                            nc.tensor.matmul(
                                ps2[:], w2sb[:, t * C:(t + 1) * C],
                                y1p[:, 24 * g + dy: 24 * g + dy + 24: 2,
                                    dx: dx + 48: 2],
                                start=(t == 0), stop=(t == 8))
                        ab2 = pa.tile([C, 12 * 24], f32, tag="ab2", name="ab2",
                                      bufs=2)
                        nc.scalar.activation(ab2[:], ps2[:], AF.Abs,
                                             scale=0.4)
                        nc.vector.scalar_tensor_tensor(
                            y2p[:, 1 + 12 * g: 13 + 12 * g, 1:25], ps2[:], 0.6,
                            ab2[:], op0=ALU.mult, op1=ALU.add)

                    # conv3: 24x24 -> 12x12 (no activation), then PE
                    # transpose into y3T[(row, col), c] halves
                    ps3 = paps.tile([C, 12, 12], f32, tag="cv", name="ps3")
                    for t in range(9):
                        dy, dx = t // 3, t % 3
                        nc.tensor.matmul(
                            ps3[:], w3sb[:, t * C:(t + 1) * C],
                            y2p[:, dy: dy + 24: 2, dx: dx + 24: 2],
                            start=(t == 0), stop=(t == 8))
                    y3f = pa.tile([C, 144], f16)
                    nc.vector.tensor_copy(y3f[:], ps3[:])
                    for hh, y3t in ((0, y3Ta), (1, y3Tb)):
                        pst = paps.tile([72, C], f16, tag="cv", name="pst")
                        nc.tensor.transpose(
                            pst[:], y3f[:, 72 * hh:72 * (hh + 1)], ident[:])
                        nc.vector.tensor_copy(y3t[:], pst[:])

                    # E^T = gw^T WwT [C, C]
                    eps = paps.tile([C, 512], f32, tag="prj", name="eps",
                                    bufs=3)
                    nc.tensor.matmul(eps[:, 0:C], gwsb[:], WwTsb[:],
                                     start=True, stop=True)
                    ET16 = hp.tile([C, C], f16)
                    nc.vector.tensor_copy(ET16[:], eps[:, 0:C])

                    # gate pipeline: all krons first (kron -> sigmoid ->
                    # fp16 gate-mul per sub), then the phi chain (it feeds
                    # the AllGather = the pass-1 critical path), then theta
                    gtc = pa.tile([C, CH], f16)
                    for o0, w in SUBS0:
                        kps = paps.tile([C, 512], f32, tag="prj",
                                        name="kps", bufs=3)
                        nc.tensor.matmul(kps[:, 0:w], y3Ta[:],
                                         krCa[:, o0:o0 + w],
                                         start=True, stop=False)
                        nc.tensor.matmul(kps[:, 0:w], y3Tb[:],
                                         krCb[:, o0:o0 + w],
                                         start=False, stop=True)
                        nc.scalar.activation(gtc[:, o0:o0 + w],
                                             kps[:, 0:w], AF.Sigmoid)
                        nc.vector.tensor_mul(xgc16[:, o0:o0 + w],
                                             gtc[:, o0:o0 + w],
                                             xchsb[:, o0:o0 + w])
                    # preload the Exp table during the gate pipeline: the read
                    # of gtc pins this after sigmoid0 (it cannot be hoisted
                    # to t=0 where the sigmoid load would evict it again)
                    tld1 = pa.tile([C, 1], f32)
                    nc.scalar.activation(tld1[:], gtc[:, 0:1], AF.Exp)
                    for o0, w in SUBS0:
                        pps = paps.tile([C, 512], f32, tag="prj",
                                        name="pps", bufs=3)
                        nc.tensor.matmul(pps[0:CI, 0:w], pwT16[:],
                                         xgc16[:, o0:o0 + w],
                                         start=True, stop=True)
                        nc.vector.tensor_copy(phiown[:, o0:o0 + w],
                                              pps[0:CI, 0:w])
                    for o0, w in SUBS0:
                        tps = paps.tile([C, 512], f32, tag="prj",
                                        name="tps", bufs=3)
                        nc.tensor.matmul(tps[0:CI, 0:w], twT16[:],
                                         xgc16[:, o0:o0 + w],
                                         start=True, stop=True)
                        # (GPSIMD cannot read PSUM on HW: copies on DVE)
                        nc.vector.tensor_copy(th16[:, o0:o0 + w],
                                              tps[0:CI, 0:w])

                    # own G^T tiles [128, 9*C] (AllGathered later)
                    gps = paps.tile([128, TPC * C], f32, tag="gps",
                                    name="gps", bufs=1)
                    for u in range(TPC):
                        nc.tensor.matmul(gps[:, u * C:(u + 1) * C],
                                         xgc16[:, u * 128:(u + 1) * 128],
                                         ET16[:], start=True, stop=True)
                    nc.vector.tensor_copy(gown[:], gps[:])

                # ====== PASS 1 with seg-interleaved fp8 PASS 2 + ARs ======
                with tc.tile_pool(name="p1ps", bufs=2, space="PSUM") as p1ps, \
                     tc.tile_pool(name="p2ps", bufs=2, space="PSUM") as p2ps, \
                     tc.tile_pool(name="p2", bufs=1) as p2:
                    # AllGathers emitted inside this scope so no pool-close
                    # barrier or clock alignment gates pass-1 on them.
                    # single-mode convention: ONE DRAM hop stands in for
                    # upload+collective; landing DMAs are modeled in full.
                    if single:
                        nc.sync.dma_start(phout[0, :, :], phiown[:])
                    else:
                        nc.sync.dma_start(phin[:], phiown[:])
                        nc.gpsimd.collective_compute(
                            "AllGather", ALU.bypass, replica_groups=groups,
                            ins=[phin.opt()], outs=[phout.opt()])
                    # land slice r=0 first: it unblocks pass-1 tiles 0-8
                    nc.sync.dma_start(phi16[:, 0:CH], phout[0, :, :])
                    nc.sync.dma_start(
                        phi16[:, CH:].rearrange("c (r n) -> c r n",
                                                r=NCORES - 1),
                        phout[1:, :, :].rearrange("r c n -> c r n"))

                    # warm the PE through the AG landing wait with fake
                    # pass-1 tiles read from phiown (already in SBUF)
                    for _ in range(3):
                        wfps = p1ps.tile([128, EIW], f32, tag="fps",
                                         name="fps")
                        for o0 in range(0, EIW, 512):
                            nc.tensor.matmul(wfps[:, o0:o0 + 512],
                                             phiown[:, 0:128],
                                             th16[:, 0:512],
                                             start=True, stop=True)

                    def emit_G_ag():
                        if single:
                            nc.sync.dma_start(gout[0, :, :], gown[:])
                        else:
                            nc.sync.dma_start(gin[:], gown[:])
                            nc.gpsimd.collective_compute(
                                "AllGather", ALU.bypass,
                                replica_groups=groups,
                                ins=[gin.opt()], outs=[gout.opt()])
                        nc.sync.dma_start(
                            G16[:].rearrange("p (r n) -> p r n", r=NCORES),
                            gout[:].rearrange("r p n -> p r n"))

                    def pass1_instr(i):
                        # one exp instruction = up to 1.33 m-tiles; the fp8
                        # cache is contiguous so the exp span can cross
                        # m-tile boundaries; Z is per-m-tile off the cache,
                        # except single-tile-aligned instrs which use the
                        # ACT f32 accumulator directly
                        c0, wd = INSTRS[i]
                        fps = p1ps.tile([128, wd], f32, tag="fps",
                                        name="fps")
                        edges = {0, wd}
                        for b in range(512, wd, 512):
                            edges.add(b)
                        jlo, jhi = c0 // CH, (c0 + wd - 1) // CH
                        for j in range(jlo, jhi + 1):
                            if c0 < j * CH < c0 + wd:
                                edges.add(j * CH - c0)
                        edges = sorted(edges)
                        for a, b in zip(edges[:-1], edges[1:]):
                            j = (c0 + a) // CH
                            ta = c0 + a - j * CH
                            nc.tensor.matmul(fps[:, a:b],
                                             phi16[:, j * 128:(j + 1) * 128],
                                             th16[:, ta:ta + (b - a)],
                                             start=True, stop=True)
                        aligned = (wd == CH and c0 % CH == 0)
                        if aligned:
                            j = c0 // CH
                            k, j0 = _seg_of(j)
                            nc.scalar.activation(
                                s8[:, c0:c0 + wd], fps[:], AF.Exp,
                                bias=nb5[:], scale=1.0,
                                accum_out=zsumk[k][:, j - j0:j - j0 + 1])
                            return
                        nc.scalar.activation(s8[:, c0:c0 + wd], fps[:],
                                             AF.Exp, bias=nb5[:], scale=1.0)
                        # Z for every m-tile completed by this instruction
                        for j in range(jlo, jhi + 1):
                            if (j + 1) * CH <= c0 + wd:
                                k, j0 = _seg_of(j)
                                zcol = zsumk[k][:, j - j0:j - j0 + 1]
                                nc.vector.tensor_scalar(
                                    zdeadV[:], s83[:, j, :], 1.0, 0.0,
                                    op0=ALU.mult, op1=ALU.add,
                                    accum_out=zcol)

                    def allreduce(k):
                        # single-mode convention (as for the AllGathers):
                        # one DRAM hop stands in for upload+collective
                        if single:
                            nc.sync.dma_start(zout[k][:], zsumk[k][:])
                        else:
                            nc.sync.dma_start(zin[k][:], zsumk[k][:])
                            nc.gpsimd.collective_compute(
                                "AllReduce", ALU.add,
                                replica_groups=groups,
                                ins=[zin[k].opt()], outs=[zout[k].opt()])
                        nc.sync.dma_start(zredk[k][:], zout[k][:])

                    def scale_G(k):
                        j0, j1 = SEGS[k]
                        ln = j1 - j0
                        zf = p2.tile([128, 22], f32, tag="zf", name="zf",
                                     bufs=2)
                        # 1/(Z/GSCALE) = GSCALE/Z
                        nc.vector.tensor_scalar(zf[:, 0:ln], zredk[k][:],
                                                1.0 / GSCALE, None,
                                                op0=ALU.mult)
                        rz = p2.tile([128, 22], f32, tag="rz", name="rz",
                                     bufs=2)
                        nc.vector.reciprocal(rz[:, 0:ln], zf[:, 0:ln])
                        rzb = rz[:, 0:ln].unsqueeze(-1).to_broadcast(
                            (128, ln, C))
                        nc.vector.tensor_mul(G3[:, j0:j1, :],
                                             G3[:, j0:j1, :], rzb)
                        nc.vector.tensor_copy(G83[:, j0:j1, :],
                                              G3[:, j0:j1, :])
                        if RESID:
                            # split G into fp8 high + fp8 residual parts
                            rt = p2.tile([128, 22 * C], f16, tag="rt",
                                         name="rt", bufs=2)
                            rt3 = rt[:].rearrange("p (j c) -> p j c", c=C)
                            nc.vector.tensor_sub(rt3[:, 0:ln, :],
                                                 G3[:, j0:j1, :],
                                                 G83[:, j0:j1, :])
                            nc.vector.tensor_copy(R83[:, j0:j1, :],
                                                  rt3[:, 0:ln, :])

                    # pass-2 work units: (k, ci, u); per-segment PSUM
                    # accumulation, DVE adds across segments into outsb
                    units = []
                    for k in range(len(SEGS)):
                        j0, j1 = SEGS[k]
                        for ci in range(len(YSUBS)):
                            for u in range((j1 - j0) // 2):
                                units.append((k, ci, u))
                    emitted = 0
                    cur_ps = {}

                    def emit_unit():
                        nonlocal emitted
                        k, ci, u = units[emitted]
                        j0, j1 = SEGS[k]
                        o0, w = YSUBS[ci]
                        npr = (j1 - j0) // 2
                        jj = j0 + 2 * u
                        if u == 0:
                            cur_ps[ci] = p2ps.tile([C, 512], f32, tag="yps",
                                                   name="yps")
                        yp = cur_ps[ci]
                        nc.tensor.matmul(
                            yp[:, 0:w], G83[:, jj:jj + 2, :],
                            s83[:, jj:jj + 2, o0:o0 + w],
                            start=(u == 0), stop=(not RESID and u == npr - 1),
                            perf_mode=DR, skip_group_check=True)
                        if RESID:
                            nc.tensor.matmul(
                                yp[:, 0:w], R83[:, jj:jj + 2, :],
                                s83[:, jj:jj + 2, o0:o0 + w],
                                start=False, stop=(u == npr - 1),
                                perf_mode=DR, skip_group_check=True)
                        if u == npr - 1:
                            osl = outsb[:, o0:o0 + w]
                            if k == 0:
                                nc.vector.tensor_copy(osl, yp[:, 0:w])
                            else:
                                nc.vector.tensor_add(osl, osl, yp[:, 0:w])
                            if k == 3:
                                # pre-fold GSCALE + gated residual for segs
                                # 0-3 NOW (hidden in the loop) so the final
                                # drain read-out is a single stt per sub
                                nc.vector.scalar_tensor_tensor(
                                    ofold[:, o0:o0 + w], osl, 1.0 / GSCALE,
                                    xgc16[:, o0:o0 + w],
                                    op0=ALU.mult, op1=ALU.add)
                        emitted += 1

                    # m-tile j's exp completes during exp-instr ei(j)
                    def ei_of(j):
                        end = (j + 1) * CH
                        for i, (c0, wd) in enumerate(INSTRS):
                            if c0 + wd >= end:
                                return i
                        raise ValueError(j)

                    seg_ei = [ei_of(s[1] - 1) for s in SEGS]
                    seg_units = [sum(1 for x in units if x[0] <= k)
                                 for k in range(len(SEGS))]
                    avail = [0]

                    def pump(i):
                        if i == 2:
                            emit_G_ag()
                        for k in range(len(SEGS)):
                            if i == seg_ei[k]:
                                allreduce(k)
                                scale_G(k)
                            if (k < len(SEGS) - 2
                                    and i == seg_ei[k] + MARGINS[k]):
                                # last 2 segs drain after the loop, behind
                                # the PE warm-up (parked units would block
                                # the warm-up and drop the p-state)
                                avail[0] = seg_units[k]
                        budget = (BUDGET[0] if i < 30 else
                                  3 if i >= 52 else BUDGET[1])
                        while emitted < avail[0] and budget > 0:
                            emit_unit()
                            budget -= 1

                    for i in range(len(INSTRS)):
                        pass1_instr(i)
                        pump(i)
                    # keep the PE p-state warm through the final Z-AR wait:
                    # re-run an already-satisfied pair into a scratch bank
                    wps = p2ps.tile([C, 512], f32, tag="yps", name="wps")
                    for _ in range(16):
                        nc.tensor.matmul(wps[:], G83[:, 0:2, :],
                                         s83[:, 0:2, 0:512],
                                         start=True, stop=True, perf_mode=DR,
                                         skip_group_check=True)
                    # segs <=3 stragglers, then segs 4+5 merged per output
                    # sub: one PSUM accumulation spanning both, and the
                    # final read-out is a single stt (yp/GSCALE + ofold)
                    while emitted < seg_units[3]:
                        emit_unit()
                    prs = [(k, u) for k in (4, 5)
                           for u in range((SEGS[k][1] - SEGS[k][0]) // 2)]
                    for ci, (o0, w) in enumerate(YSUBS):
                        yp = p2ps.tile([C, 512], f32, tag="yps", name="yps")
                        for pi, (k, u) in enumerate(prs):
                            jj = SEGS[k][0] + 2 * u
                            first = pi == 0
                            last = pi == len(prs) - 1
                            nc.tensor.matmul(
                                yp[:, 0:w], G83[:, jj:jj + 2, :],
                                s83[:, jj:jj + 2, o0:o0 + w],
                                start=first, stop=(not RESID and last),
                                perf_mode=DR, skip_group_check=True)
                            if RESID:
                                nc.tensor.matmul(
                                    yp[:, 0:w], R83[:, jj:jj + 2, :],
                                    s83[:, jj:jj + 2, o0:o0 + w],
                                    start=False, stop=last,
                                    perf_mode=DR, skip_group_check=True)
                        osl = outsb[:, o0:o0 + w]
                        nc.vector.scalar_tensor_tensor(
                            osl, yp[:, 0:w], 1.0 / GSCALE,
                            ofold[:, o0:o0 + w], op0=ALU.mult, op1=ALU.add)
                        nc.sync.dma_start(out_io[:, o0:o0 + w], osl)

    nc.compile()
    return nc


def get_program():
    if "nc" not in _compiled:
        _compiled["nc"] = _build()
    return _compiled["nc"]


def _bilinear_kron():
    """K[(k,j), (R,Cc)] = uv[R,k]*uv[Cc,j] for x8 bilinear upsample 12->96
    (align_corners=False, edge-clamped), split into two 72-row halves."""
    uv = np.zeros((96, 12), np.float64)
    for R in range(96):
        t = (R + 0.5) / 8.0 - 0.5
        k0 = int(np.floor(t))
        fr = t - k0
        for k, wt in ((k0, 1.0 - fr), (k0 + 1, fr)):
            kc = min(max(k, 0), 11)
            uv[R, kc] += wt
    K = np.einsum("Rk,Cj->kjRC", uv, uv).reshape(144, 9216)
    return np.ascontiguousarray(K).astype(np.float16)


def make_in_maps(inputs):
    f16 = np.float16
    x = np.asarray(inputs["x"], np.float32).reshape(C, H, W)
    xflat = np.ascontiguousarray(x.reshape(C, N))
    xpad = np.zeros((C, 98, 98), f16)
    xpad[:, 1:97, 1:97] = x.astype(f16)
    krF = _bilinear_kron()

    def conv_w(w):
        # [o, i, dy, dx] -> [i, (dy dx), o]
        return np.ascontiguousarray(
            np.asarray(w, np.float32).transpose(1, 2, 3, 0).reshape(C, 9 * C)
        ).astype(f16)

    base = {
        "xpad": xpad,
        "w1": conv_w(inputs["d1_w"]),
        "w2": conv_w(inputs["d2_w"]),
        "w3": conv_w(inputs["d3_w"]),
        "twT": np.ascontiguousarray(
            np.asarray(inputs["th_w"], np.float32)[:, :, 0, 0].T).astype(f16),
        "pwT": np.ascontiguousarray(
            np.asarray(inputs["ph_w"], np.float32)[:, :, 0, 0].T).astype(f16),
        "gw": np.ascontiguousarray(
            np.asarray(inputs["g_w"], np.float32)[:, :, 0, 0]),
        "WwT": np.ascontiguousarray(
            np.asarray(inputs["W_w"], np.float32)[:, :, 0, 0].T),
    }
    in_maps = []
    for k in range(NCORES):
        m = dict(base)
        m["xch"] = np.ascontiguousarray(
            xflat[:, k * CH:(k + 1) * CH]).astype(f16)
        m["krC"] = np.ascontiguousarray(krF[:, k * CH:(k + 1) * CH])
        in_maps.append(m)
    return in_maps


def kernel(**inputs):
    from concourse import bass_utils

    nc = get_program()
    in_maps = make_in_maps(inputs)
    res = bass_utils.run_bass_kernel_spmd(nc, in_maps,
                                          core_ids=list(range(NCORES)))
    out = np.concatenate([res.results[k]["out"] for k in range(NCORES)], axis=1)
    return out.reshape(1, C, H, W).astype(np.float32)


# revision 108
# speedup vs baseline: 1.0236x; 1.0093x over previous
"""Trainium2 Bass kernel for AttentiveNonLocalBlock2D (AllGather design).

Sequence-parallel over N=H*W across 8 cores, per the sharding hint's
tensor-parallel scheme: each core computes the gate + projections ONLY for
its own 1152-pixel chunk, then phi [32,1152] and G^T [128,9*64] are
AllGathered (DRAM-staged collectives) to form the full phi [32,9216] /
G [128,72*64] every core needs for its n-slice of the attention.

Per core:
  Phase A (one pool scope, no mid barriers): identity-matmul PE p-state
    warm-up under the input DMAs; 3x stride-2 conv gating unit (fp16 PE,
    lrelu = 0.6x+0.4|x| via ACT Abs + DVE stt); conv3 emits pre-transposed
    y3T halves; bilinear-upsample columns for the OWN chunk only via the
    per-core krC input (y3T^T @ krC) -> sigmoid -> fp16 gate-mul ->
    phi/theta/G^T projections.  The AllGathers + Exp table load launch
    outside the phase-A pools so no close-barrier gates pass-1 on them.
  Pass 1: 55 exp instructions (52x1536 cols = 1.33 m-tiles each, then
    tile-aligned 768/1152/1152 tails; the fp8 cache is contiguous so spans
    may cross m-tile boundaries): PE score matmuls fT = phi_tile^T
    theta_chunk into a 2-buffer PSUM ring, ACT exp(f - 2.5) written
    straight to a float8e5 cache (e5m2: wide range so the softmax
    denominator doesn't lose its tail to subnormal flushing; e4m3 loses
    ~10% of Z's mass).  Z[m] partials via DVE dead-store tensor_scalar
    accumulation over the cache (2x SBUF mode), except the final two
    tile-aligned instrs which use ACT's f32 accumulator (shortest path
    into the last Z-AllReduce); Z is AllReduced in 6 segments.
  Pass 2: per segment G is scaled by GSCALE/Z and split into fp8e4 high +
    residual parts; fp8 DoubleRow matmuls (2 m-tiles/instr, 0.5 cy/col,
    e4 stationary x e5 moving) accumulate into per-segment PSUM banks,
    pace-interleaved between later pass-1 instrs (margins keep not-ready
    units from parking at the PE queue head, which would starve ACT);
    DVE adds across segments, final read-out divides by GSCALE and adds
    the gated residual.  The last two segments drain after the loop behind
    a PE warm-up burst that keeps the p-state up through the final Z
    AllReduce latency.
  Host concatenates the per-core n-chunks.

Single-device build (the TimelineSim timing variant) replaces each
collective with one DRAM-hop DMA (upload straight to the gathered buffer);
landing DMAs are modeled in full.
"""

import sys

if "/opt/trn_rl_repo" not in sys.path:
    sys.path.insert(0, "/opt/trn_rl_repo")

import numpy as np

NCORES = 8
C, CI, H, W = 64, 32, 96, 96
N = H * W            # 9216
CH = N // NCORES     # 1152 pixels per core
MT = N // 128        # 72 m-tiles of 128
TPC = MT // NCORES   # 9 own m-tiles per core
EXP_BIAS = -2.5      # keeps exp(f+bias) <= ~16k < 57344 (e5m2 max) while
                     # minimizing subnormal flushing of tiny softmax terms
GSCALE = 64.0 * float(np.exp(-2.5 + 7.5))
                     # pre-scale so G*GSCALE/Z clears the e4m3 subnormal
                     # floor; tracks EXP_BIAS (Z scales with exp(bias))
SEGS = ((0, 22), (22, 40), (40, 54), (54, 64), (64, 70), (70, 72))
EIW = 1536           # exp-instruction width (cols)
# 52 x 1536-col instrs, then tile-aligned tails (768, 1152, 1152): the last
# two instrs cover exactly tiles 70 / 71 so their Z comes from the ACT f32
# accumulator (saves the DVE round trip on the final Z-AllReduce chain)
INSTRS = tuple([(i * EIW, EIW) for i in range(52)]
               + [(52 * EIW, 768), (70 * CH, CH), (71 * CH, CH)])
MARGINS = (7, 8, 7, 4, 99, 99)  # exp-instrs between AR issue and pass-2
                                # emit; last two segs drain after the loop
BUDGET = (3, 4)      # pass-2 units per exp instr (early, late)
RESID = True         # add an fp8 residual pass for G (extra accuracy)
# n-chunk subtiles for the two PSUM ring halves (bank-boundary aligned)
SUBS0 = ((0, 512), (512, 512), (1024, 128))
SUBS1 = ((0, 384), (384, 512), (896, 256))
YSUBS = ((0, 512), (512, 512), (1024, 128))  # pass-2 output subtiles

_compiled = {}


def _zmode(j):
    """Z accumulation engine per tile: DVE only (the dead-store
    tensor_scalar opcode does not exist on GPSIMD, and ACT's accumulator
    cannot be used because exp instructions span m-tile boundaries)."""
    return "dve"


def _seg_of(j):
    for k, (j0, j1) in enumerate(SEGS):
        if j0 <= j < j1:
            return k, j0
    raise ValueError(j)


def _build(single=False):
    import concourse.bacc as bacc
    import concourse.bass as bass
    import concourse.mybir as mybir
    import concourse.tile as tile
    from concourse import masks

    f16 = mybir.dt.float16
    f32 = mybir.dt.float32
    f8 = mybir.dt.float8e4
    f8w = mybir.dt.float8e5   # exp cache: wide range so tiny softmax terms
                              # aren't flushed (Z would lose ~10% of its mass)
    DR = mybir.MatmulPerfMode.DoubleRow
    AF = mybir.ActivationFunctionType
    ALU = mybir.AluOpType

    nc = bacc.Bacc("TRN2", target_bir_lowering=False, debug=False,
                   num_devices=1 if single else NCORES)

    xpad_io = nc.dram_tensor("xpad", [C, 98, 98], f16, kind="ExternalInput")
    w1_io = nc.dram_tensor("w1", [C, 9 * C], f16, kind="ExternalInput")
    w2_io = nc.dram_tensor("w2", [C, 9 * C], f16, kind="ExternalInput")
    w3_io = nc.dram_tensor("w3", [C, 9 * C], f16, kind="ExternalInput")
    twT_io = nc.dram_tensor("twT", [C, CI], f16, kind="ExternalInput")
    pwT_io = nc.dram_tensor("pwT", [C, CI], f16, kind="ExternalInput")
    gw_io = nc.dram_tensor("gw", [CI, C], f32, kind="ExternalInput")
    WwT_io = nc.dram_tensor("WwT", [CI, C], f32, kind="ExternalInput")
    xch_io = nc.dram_tensor("xch", [C, CH], f16, kind="ExternalInput")
    krC_io = nc.dram_tensor("krC", [144, CH], f16, kind="ExternalInput")
    out_io = nc.dram_tensor("out", [C, CH], f32, kind="ExternalOutput")

    groups = [list(range(NCORES))]

    with tile.TileContext(nc) as tc:
        with tc.tile_pool(name="persist", bufs=1) as pp, \
             tc.tile_pool(name="dram", bufs=1, space="DRAM") as dp:
            # per-segment Z tiles so the AR DMA reads never alias later writes
            zsumk = [pp.tile([128, j1 - j0], f32, name=f"zsum{k}")
                     for k, (j0, j1) in enumerate(SEGS)]
            zredk = [pp.tile([128, j1 - j0], f32, name=f"zred{k}")
                     for k, (j0, j1) in enumerate(SEGS)]
            nb5 = pp.tile([128, 1], f32)
            nc.gpsimd.memset(nb5[:], EXP_BIAS)
            zin = [dp.tile([128, j1 - j0], f32, name=f"zin{k}")
                   for k, (j0, j1) in enumerate(SEGS)]
            zout = [dp.tile([128, j1 - j0], f32, addr_space="Shared",
                            name=f"zout{k}")
                    for k, (j0, j1) in enumerate(SEGS)]
            phin = dp.tile([CI, CH], f16, name="phin")
            phout = dp.tile([NCORES, CI, CH], f16, addr_space="Shared",
                            name="phout")
            gin = dp.tile([128, TPC * C], f16, name="gin")
            gout = dp.tile([NCORES, 128, TPC * C], f16, addr_space="Shared",
                           name="gout")

            with tc.tile_pool(name="hand", bufs=1) as hp:
                phi16 = hp.tile([CI, N], f16)
                th16 = hp.tile([CI, CH], f16)
                G16 = hp.tile([128, MT * C], f16)
                G3 = G16[:].rearrange("p (j c) -> p j c", c=C)
                G8 = hp.tile([128, MT * C], f8)
                G83 = G8[:].rearrange("p (j c) -> p j c", c=C)
                R8 = hp.tile([128, MT * C], f8)
                R83 = R8[:].rearrange("p (j c) -> p j c", c=C)
                xgc16 = hp.tile([C, CH], f16)
                outsb = hp.tile([C, CH], f32)
                ofold = hp.tile([C, CH], f32)  # segs 0-3 sum, pre-folded
                # (exp-table load is implicit before the first pass-1 exp;
                # it hides behind the phi AllGather landing wait)
                zdeadV = hp.tile([128, CH], f8w)  # dead stores for Z accum
                zdeadP = hp.tile([128, CH], f8w)  # (same dtype as the cache)
                phiown = hp.tile([CI, CH], f16)
                gown = hp.tile([128, TPC * C], f16)
                s8 = hp.tile([128, MT * CH], f8w)
                s83 = s8[:].rearrange("p (j n) -> p j n", n=CH)

                # ==================== PHASE A ====================
                # single merged pool scope: no mid-phase close barrier
                # between the convs and the gate/projection pipeline
                with tc.tile_pool(name="pa", bufs=1) as pa, \
                     tc.tile_pool(name="paps", bufs=2, space="PSUM") as paps:
                    y3Ta = pa.tile([72, C], f16)
                    y3Tb = pa.tile([72, C], f16)
                    # preload the Sigmoid table while input DMAs fly
                    tld0 = pa.tile([128, 1], f32)
                    nc.scalar.activation(tld0[:], nb5[:], AF.Sigmoid)
                    # ramp the PE p-state during the input-DMA wait so conv1
                    # runs at full speed from its first matmul (identity
                    # needs no DMA)
                    ident = pa.tile([C, C], f16)
                    masks.make_identity(nc, ident[:])
                    wmps = paps.tile([C, C], f32, tag="warm", name="wmps",
                                     bufs=1)
                    for _ in range(140):
                        nc.tensor.matmul(wmps[:], ident[:], ident[:],
                                         start=True, stop=True,
                                         skip_group_check=True)

                    # conv-critical DMAs first: HWDGE is one serial queue,
                    # and conv1 must run gapless to keep the PE p-state up
                    xpad = pa.tile([C, 98, 98], f16)
                    w1sb = pa.tile([C, 9 * C], f16)
                    nc.sync.dma_start(xpad[:, 0:18, :], xpad_io[:, 0:18, :])
                    nc.sync.dma_start(w1sb[:], w1_io[:])
                    nc.sync.dma_start(xpad[:, 18:50, :], xpad_io[:, 18:50, :])
                    nc.sync.dma_start(xpad[:, 50:98, :], xpad_io[:, 50:98, :])
                    w2sb = pa.tile([C, 9 * C], f16)
                    nc.sync.dma_start(w2sb[:], w2_io[:])
                    w3sb = pa.tile([C, 9 * C], f16)
                    nc.sync.dma_start(w3sb[:], w3_io[:])
                    twT16 = pa.tile([C, CI], f16)
                    nc.sync.dma_start(twT16[:], twT_io[:])
                    pwT16 = pa.tile([C, CI], f16)
                    nc.sync.dma_start(pwT16[:], pwT_io[:])
                    gwsb = pa.tile([CI, C], f32)
                    nc.sync.dma_start(gwsb[:], gw_io[:])
                    WwTsb = pa.tile([CI, C], f32)
                    nc.sync.dma_start(WwTsb[:], WwT_io[:])
                    krCa = pa.tile([72, CH], f16)
                    nc.sync.dma_start(krCa[:], krC_io[0:72, :])
                    krCb = pa.tile([72, CH], f16)
                    nc.sync.dma_start(krCb[:], krC_io[72:144, :])
                    xchsb = pa.tile([C, CH], f16)
                    nc.sync.dma_start(xchsb[:], xch_io[:])

                    # conv1: 96x96 -> 48x48, stride 2, pad 1, lrelu(0.2)
                    y1p = pa.tile([C, 50, 50], f16)
                    nc.gpsimd.memset(y1p[:], 0.0)
                    for g in range(6):
                        ps1 = paps.tile([C, 8, 48], f32, tag="cv", name="ps1")
                        for t in range(9):
                            dy, dx = t // 3, t % 3
                            nc.tensor.matmul(
                                ps1[:], w1sb[:, t * C:(t + 1) * C],
                                xpad[:, 16 * g + dy: 16 * g + dy + 16: 2,
                                     dx: dx + 96: 2],
                                start=(t == 0), stop=(t == 8))
                        # lrelu(x) = 0.6*x + 0.4*|x| (only one PSUM input
                        # allowed per DVE op; ACT is idle during the convs)
                        ab1 = pa.tile([C, 8 * 48], f32, tag="ab1", name="ab1",
                                      bufs=2)
                        nc.scalar.activation(ab1[:], ps1[:], AF.Abs,
                                             scale=0.4)
                        nc.vector.scalar_tensor_tensor(
                            y1p[:, 1 + 8 * g: 9 + 8 * g, 1:49], ps1[:], 0.6,
                            ab1[:], op0=ALU.mult, op1=ALU.add)

                    # conv2: 48x48 -> 24x24
                    y2p = pa.tile([C, 26, 26], f16)
                    nc.gpsimd.memset(y2p[:], 0.0)
                    for g in range(2):
                        ps2 = paps.tile([C, 12, 24], f32, tag="cv", name="ps2")
                        for t in range(9):
                            dy, dx = t // 3, t % 3
                            nc.tensor.matmul(
                                ps2[:], w2sb[:, t * C:(t + 1) * C],
                                y1p[:, 24 * g + dy: 24 * g + dy + 24: 2,
                                    dx: dx + 48: 2],
                                start=(t == 0), stop=(t == 8))
                        ab2 = pa.tile([C, 12 * 24], f32, tag="ab2", name="ab2",
                                      bufs=2)
                        nc.scalar.activation(ab2[:], ps2[:], AF.Abs,
                                             scale=0.4)
                        nc.vector.scalar_tensor_tensor(
                            y2p[:, 1 + 12 * g: 13 + 12 * g, 1:25], ps2[:], 0.6,
                            ab2[:], op0=ALU.mult, op1=ALU.add)

                    # conv3: 24x24 -> 12x12 (no activation), then PE
                    # transpose into y3T[(row, col), c] halves
                    ps3 = paps.tile([C, 12, 12], f32, tag="cv", name="ps3")
                    for t in range(9):
                        dy, dx = t // 3, t % 3
                        nc.tensor.matmul(
                            ps3[:], w3sb[:, t * C:(t + 1) * C],
                            y2p[:, dy: dy + 24: 2, dx: dx + 24: 2],
                            start=(t == 0), stop=(t == 8))
                    y3f = pa.tile([C, 144], f16)
                    nc.vector.tensor_copy(y3f[:], ps3[:])
                    for hh, y3t in ((0, y3Ta), (1, y3Tb)):
                        pst = paps.tile([72, C], f16, tag="cv", name="pst")
                        nc.tensor.transpose(
                            pst[:], y3f[:, 72 * hh:72 * (hh + 1)], ident[:])
                        nc.vector.tensor_copy(y3t[:], pst[:])

                    # E^T = gw^T WwT [C, C]
                    eps = paps.tile([C, 512], f32, tag="prj", name="eps",
                                    bufs=3)
                    nc.tensor.matmul(eps[:, 0:C], gwsb[:], WwTsb[:],
                                     start=True, stop=True)
                    ET16 = hp.tile([C, C], f16)
                    nc.vector.tensor_copy(ET16[:], eps[:, 0:C])

                    # gate pipeline: all krons first (kron -> sigmoid ->
                    # fp16 gate-mul per sub), then the phi chain (it feeds
                    # the AllGather = the pass-1 critical path), then theta
                    gtc = pa.tile([C, CH], f16)
                    for o0, w in SUBS0:
                        kps = paps.tile([C, 512], f32, tag="prj",
                                        name="kps", bufs=3)
                        nc.tensor.matmul(kps[:, 0:w], y3Ta[:],
                                         krCa[:, o0:o0 + w],
                                         start=True, stop=False)
                        nc.tensor.matmul(kps[:, 0:w], y3Tb[:],
                                         krCb[:, o0:o0 + w],
                                         start=False, stop=True)
                        nc.scalar.activation(gtc[:, o0:o0 + w],
                                             kps[:, 0:w], AF.Sigmoid)
                        nc.vector.tensor_mul(xgc16[:, o0:o0 + w],
                                             gtc[:, o0:o0 + w],
                                             xchsb[:, o0:o0 + w])
                    # preload the Exp table during the gate pipeline: the read
                    # of gtc pins this after sigmoid0 (it cannot be hoisted
                    # to t=0 where the sigmoid load would evict it again)
                    tld1 = pa.tile([C, 1], f32)
                    nc.scalar.activation(tld1[:], gtc[:, 0:1], AF.Exp)
                    for o0, w in SUBS0:
                        pps = paps.tile([C, 512], f32, tag="prj",
                                        name="pps", bufs=3)
                        nc.tensor.matmul(pps[0:CI, 0:w], pwT16[:],
                                         xgc16[:, o0:o0 + w],
                                         start=True, stop=True)
                        nc.vector.tensor_copy(phiown[:, o0:o0 + w],
                                              pps[0:CI, 0:w])
                    for o0, w in SUBS0:
                        tps = paps.tile([C, 512], f32, tag="prj",
                                        name="tps", bufs=3)
                        nc.tensor.matmul(tps[0:CI, 0:w], twT16[:],
                                         xgc16[:, o0:o0 + w],
                                         start=True, stop=True)
                        # (GPSIMD cannot read PSUM on HW: copies on DVE)
                        nc.vector.tensor_copy(th16[:, o0:o0 + w],
                                              tps[0:CI, 0:w])

                    # own G^T tiles [128, 9*C] (AllGathered later)
                    gps = paps.tile([128, TPC * C], f32, tag="gps",
                                    name="gps", bufs=1)
                    for u in range(TPC):
                        nc.tensor.matmul(gps[:, u * C:(u + 1) * C],
                                         xgc16[:, u * 128:(u + 1) * 128],
                                         ET16[:], start=True, stop=True)
                    nc.vector.tensor_copy(gown[:], gps[:])

                # ====== PASS 1 with seg-interleaved fp8 PASS 2 + ARs ======
                with tc.tile_pool(name="p1ps", bufs=2, space="PSUM") as p1ps, \
                     tc.tile_pool(name="p2ps", bufs=2, space="PSUM") as p2ps, \
                     tc.tile_pool(name="p2", bufs=1) as p2:
                    # AllGathers emitted inside this scope so no pool-close
                    # barrier or clock alignment gates pass-1 on them.
                    # single-mode convention: ONE DRAM hop stands in for
                    # upload+collective; landing DMAs are modeled in full.
                    # per-sub uploads pipeline behind the phiown copies so
                    # the first landed piece arrives ~3us earlier (Shared
                    # phout allows only one writer, so single mode lands
                    # the r=0 pieces straight from phin - same hop count)
                    for o0, w in SUBS0:
                        nc.sync.dma_start(phin[:, o0:o0 + w],
                                          phiown[:, o0:o0 + w])
                    if not single:
                        nc.gpsimd.collective_compute(
                            "AllGather", ALU.bypass, replica_groups=groups,
                            ins=[phin.opt()], outs=[phout.opt()])
                    r0src = phin if single else phout[0]
                    # land r=0 in 2 pieces: cols 0-512 unblock pass-1
                    # instrs 0-2, the rest covers tiles 4-8
                    nc.sync.dma_start(phi16[:, 0:512], r0src[:, 0:512])
                    nc.sync.dma_start(phi16[:, 512:CH], r0src[:, 512:CH])
                    nc.sync.dma_start(
                        phi16[:, CH:].rearrange("c (r n) -> c r n",
                                                r=NCORES - 1),
                        phout[1:, :, :].rearrange("r c n -> c r n"))

                    # warm the PE through the AG landing wait with fake
                    # pass-1 tiles read from phiown (already in SBUF)
                    for _ in range(3):
                        wfps = p1ps.tile([128, EIW], f32, tag="fps",
                                         name="fps")
                        for o0 in range(0, EIW, 512):
                            nc.tensor.matmul(wfps[:, o0:o0 + 512],
                                             phiown[:, 0:128],
                                             th16[:, 0:512],
                                             start=True, stop=True)

                    def emit_G_ag():
                        if single:
                            nc.sync.dma_start(gout[0, :, :], gown[:])
                        else:
                            nc.sync.dma_start(gin[:], gown[:])
                            nc.gpsimd.collective_compute(
                                "AllGather", ALU.bypass,
                                replica_groups=groups,
                                ins=[gin.opt()], outs=[gout.opt()])
                        nc.sync.dma_start(
                            G16[:].rearrange("p (r n) -> p r n", r=NCORES),
                            gout[:].rearrange("r p n -> p r n"))

                    def pass1_instr(i):
                        # one exp instruction = up to 1.33 m-tiles; the fp8
                        # cache is contiguous so the exp span can cross
                        # m-tile boundaries; Z is per-m-tile off the cache,
                        # except single-tile-aligned instrs which use the
                        # ACT f32 accumulator directly
                        c0, wd = INSTRS[i]
                        fps = p1ps.tile([128, wd], f32, tag="fps",
                                        name="fps")
                        edges = {0, wd}
                        for b in range(512, wd, 512):
                            edges.add(b)
                        jlo, jhi = c0 // CH, (c0 + wd - 1) // CH
                        for j in range(jlo, jhi + 1):
                            if c0 < j * CH < c0 + wd:
                                edges.add(j * CH - c0)
                        edges = sorted(edges)
                        for a, b in zip(edges[:-1], edges[1:]):
                            j = (c0 + a) // CH
                            ta = c0 + a - j * CH
                            nc.tensor.matmul(fps[:, a:b],
                                             phi16[:, j * 128:(j + 1) * 128],
                                             th16[:, ta:ta + (b - a)],
                                             start=True, stop=True)
                        aligned = (wd == CH and c0 % CH == 0)
                        if aligned:
                            j = c0 // CH
                            k, j0 = _seg_of(j)
                            nc.scalar.activation(
                                s8[:, c0:c0 + wd], fps[:], AF.Exp,
                                bias=nb5[:], scale=1.0,
                                accum_out=zsumk[k][:, j - j0:j - j0 + 1])
                            return
                        nc.scalar.activation(s8[:, c0:c0 + wd], fps[:],
                                             AF.Exp, bias=nb5[:], scale=1.0)
                        # Z for every m-tile completed by this instruction
                        for j in range(jlo, jhi + 1):
                            if (j + 1) * CH <= c0 + wd:
                                k, j0 = _seg_of(j)
                                zcol = zsumk[k][:, j - j0:j - j0 + 1]
                                nc.vector.tensor_scalar(
                                    zdeadV[:], s83[:, j, :], 1.0, 0.0,
                                    op0=ALU.mult, op1=ALU.add,
                                    accum_out=zcol)

                    def allreduce(k):
                        # single-mode convention (as for the AllGathers):
                        # one DRAM hop stands in for upload+collective
                        if single:
                            nc.sync.dma_start(zout[k][:], zsumk[k][:])
                        else:
                            nc.sync.dma_start(zin[k][:], zsumk[k][:])
                            nc.gpsimd.collective_compute(
                                "AllReduce", ALU.add,
                                replica_groups=groups,
                                ins=[zin[k].opt()], outs=[zout[k].opt()])
                        nc.sync.dma_start(zredk[k][:], zout[k][:])

                    def scale_G(k):
                        j0, j1 = SEGS[k]
                        ln = j1 - j0
                        zf = p2.tile([128, 22], f32, tag="zf", name="zf",
                                     bufs=2)
                        # 1/(Z/GSCALE) = GSCALE/Z
                        nc.vector.tensor_scalar(zf[:, 0:ln], zredk[k][:],
                                                1.0 / GSCALE, None,
                                                op0=ALU.mult)
                        rz = p2.tile([128, 22], f32, tag="rz", name="rz",
                                     bufs=2)
                        nc.vector.reciprocal(rz[:, 0:ln], zf[:, 0:ln])
                        rzb = rz[:, 0:ln].unsqueeze(-1).to_broadcast(
                            (128, ln, C))
                        nc.vector.tensor_mul(G3[:, j0:j1, :],
                                             G3[:, j0:j1, :], rzb)
                        nc.vector.tensor_copy(G83[:, j0:j1, :],
                                              G3[:, j0:j1, :])
                        if RESID:
                            # split G into fp8 high + fp8 residual parts
                            rt = p2.tile([128, 22 * C], f16, tag="rt",
                                         name="rt", bufs=2)
                            rt3 = rt[:].rearrange("p (j c) -> p j c", c=C)
                            nc.vector.tensor_sub(rt3[:, 0:ln, :],
                                                 G3[:, j0:j1, :],
                                                 G83[:, j0:j1, :])
                            nc.vector.tensor_copy(R83[:, j0:j1, :],
                                                  rt3[:, 0:ln, :])

                    # pass-2 work units: (k, ci, u); per-segment PSUM
                    # accumulation, DVE adds across segments into outsb
                    units = []
                    for k in range(len(SEGS)):
                        j0, j1 = SEGS[k]
                        for ci in range(len(YSUBS)):
                            for u in range((j1 - j0) // 2):
                                units.append((k, ci, u))
                    emitted = 0
                    cur_ps = {}

                    def emit_unit():
                        nonlocal emitted
                        k, ci, u = units[emitted]
                        j0, j1 = SEGS[k]
                        o0, w = YSUBS[ci]
                        npr = (j1 - j0) // 2
                        jj = j0 + 2 * u
                        if u == 0:
                            cur_ps[ci] = p2ps.tile([C, 512], f32, tag="yps",
                                                   name="yps")
                        yp = cur_ps[ci]
                        nc.tensor.matmul(
                            yp[:, 0:w], G83[:, jj:jj + 2, :],
                            s83[:, jj:jj + 2, o0:o0 + w],
                            start=(u == 0), stop=(not RESID and u == npr - 1),
                            perf_mode=DR, skip_group_check=True)
                        if RESID:
                            nc.tensor.matmul(
                                yp[:, 0:w], R83[:, jj:jj + 2, :],
                                s83[:, jj:jj + 2, o0:o0 + w],
                                start=False, stop=(u == npr - 1),
                                perf_mode=DR, skip_group_check=True)
                        if u == npr - 1:
                            osl = outsb[:, o0:o0 + w]
                            if k == 0:
                                nc.vector.tensor_copy(osl, yp[:, 0:w])
                            else:
                                nc.vector.tensor_add(osl, osl, yp[:, 0:w])
                            if k == 3:
                                # pre-fold GSCALE + gated residual for segs
                                # 0-3 NOW (hidden in the loop) so the final
                                # drain read-out is a single stt per sub
                                nc.vector.scalar_tensor_tensor(
                                    ofold[:, o0:o0 + w], osl, 1.0 / GSCALE,
                                    xgc16[:, o0:o0 + w],
                                    op0=ALU.mult, op1=ALU.add)
                        emitted += 1

                    # m-tile j's exp completes during exp-instr ei(j)
                    def ei_of(j):
                        end = (j + 1) * CH
                        for i, (c0, wd) in enumerate(INSTRS):
                            if c0 + wd >= end:
                                return i
                        raise ValueError(j)

                    seg_ei = [ei_of(s[1] - 1) for s in SEGS]
                    seg_units = [sum(1 for x in units if x[0] <= k)
                                 for k in range(len(SEGS))]
                    avail = [0]

                    def pump(i):
                        if i == 2:
                            emit_G_ag()
                        for k in range(len(SEGS)):
                            if i == seg_ei[k]:
                                allreduce(k)
                                scale_G(k)
                            if (k < len(SEGS) - 2
                                    and i == seg_ei[k] + MARGINS[k]):
                                # last 2 segs drain after the loop, behind
                                # the PE warm-up (parked units would block
                                # the warm-up and drop the p-state)
                                avail[0] = seg_units[k]
                        budget = (BUDGET[0] if i < 30 else
                                  3 if i >= 52 else BUDGET[1])
                        while emitted < avail[0] and budget > 0:
                            emit_unit()
                            budget -= 1

                    for i in range(len(INSTRS)):
                        pass1_instr(i)
                        pump(i)
                    # keep the PE p-state warm through the final Z-AR wait:
                    # re-run an already-satisfied pair into a scratch bank
                    wps = p2ps.tile([C, 512], f32, tag="yps", name="wps")
                    for _ in range(16):
                        nc.tensor.matmul(wps[:], G83[:, 0:2, :],
                                         s83[:, 0:2, 0:512],
                                         start=True, stop=True, perf_mode=DR,
                                         skip_group_check=True)
                    # segs <=3 stragglers, then segs 4+5 merged per output
                    # sub: one PSUM accumulation spanning both, and the
                    # final read-out is a single stt (yp/GSCALE + ofold)
                    while emitted < seg_units[3]:
                        emit_unit()
                    prs = [(k, u) for k in (4, 5)
                           for u in range((SEGS[k][1] - SEGS[k][0]) // 2)]
                    for ci, (o0, w) in enumerate(YSUBS):
                        yp = p2ps.tile([C, 512], f32, tag="yps", name="yps")
                        for pi, (k, u) in enumerate(prs):
                            jj = SEGS[k][0] + 2 * u
                            first = pi == 0
                            last = pi == len(prs) - 1
                            nc.tensor.matmul(
                                yp[:, 0:w], G83[:, jj:jj + 2, :],
                                s83[:, jj:jj + 2, o0:o0 + w],
                                start=first, stop=(not RESID and last),
                                perf_mode=DR, skip_group_check=True)
                            if RESID:
                                nc.tensor.matmul(
                                    yp[:, 0:w], R83[:, jj:jj + 2, :],
                                    s83[:, jj:jj + 2, o0:o0 + w],
                                    start=False, stop=last,
                                    perf_mode=DR, skip_group_check=True)
                        osl = outsb[:, o0:o0 + w]
                        nc.vector.scalar_tensor_tensor(
                            osl, yp[:, 0:w], 1.0 / GSCALE,
                            ofold[:, o0:o0 + w], op0=ALU.mult, op1=ALU.add)
                        nc.sync.dma_start(out_io[:, o0:o0 + w], osl)

    nc.compile()
    return nc


def get_program():
    if "nc" not in _compiled:
        _compiled["nc"] = _build()
    return _compiled["nc"]


def _bilinear_kron():
    """K[(k,j), (R,Cc)] = uv[R,k]*uv[Cc,j] for x8 bilinear upsample 12->96
    (align_corners=False, edge-clamped), split into two 72-row halves."""
    uv = np.zeros((96, 12), np.float64)
    for R in range(96):
        t = (R + 0.5) / 8.0 - 0.5
        k0 = int(np.floor(t))
        fr = t - k0
        for k, wt in ((k0, 1.0 - fr), (k0 + 1, fr)):
            kc = min(max(k, 0), 11)
            uv[R, kc] += wt
    K = np.einsum("Rk,Cj->kjRC", uv, uv).reshape(144, 9216)
    return np.ascontiguousarray(K).astype(np.float16)


def make_in_maps(inputs):
    f16 = np.float16
    x = np.asarray(inputs["x"], np.float32).reshape(C, H, W)
    xflat = np.ascontiguousarray(x.reshape(C, N))
    xpad = np.zeros((C, 98, 98), f16)
    xpad[:, 1:97, 1:97] = x.astype(f16)
    krF = _bilinear_kron()

    def conv_w(w):
        # [o, i, dy, dx] -> [i, (dy dx), o]
        return np.ascontiguousarray(
            np.asarray(w, np.float32).transpose(1, 2, 3, 0).reshape(C, 9 * C)
        ).astype(f16)

    base = {
        "xpad": xpad,
        "w1": conv_w(inputs["d1_w"]),
        "w2": conv_w(inputs["d2_w"]),
        "w3": conv_w(inputs["d3_w"]),
        "twT": np.ascontiguousarray(
            np.asarray(inputs["th_w"], np.float32)[:, :, 0, 0].T).astype(f16),
        "pwT": np.ascontiguousarray(
            np.asarray(inputs["ph_w"], np.float32)[:, :, 0, 0].T).astype(f16),
        "gw": np.ascontiguousarray(
            np.asarray(inputs["g_w"], np.float32)[:, :, 0, 0]),
        "WwT": np.ascontiguousarray(
            np.asarray(inputs["W_w"], np.float32)[:, :, 0, 0].T),
    }
    in_maps = []
    for k in range(NCORES):
        m = dict(base)
        m["xch"] = np.ascontiguousarray(
            xflat[:, k * CH:(k + 1) * CH]).astype(f16)
        m["krC"] = np.ascontiguousarray(krF[:, k * CH:(k + 1) * CH])
        in_maps.append(m)
    return in_maps


def kernel(**inputs):
    from concourse import bass_utils

    nc = get_program()
    in_maps = make_in_maps(inputs)
    res = bass_utils.run_bass_kernel_spmd(nc, in_maps,
                                          core_ids=list(range(NCORES)))
    out = np.concatenate([res.results[k]["out"] for k in range(NCORES)], axis=1)
    return out.reshape(1, C, H, W).astype(np.float32)


# revision 113
# speedup vs baseline: 1.0247x; 1.0010x over previous
"""Trainium2 Bass kernel for AttentiveNonLocalBlock2D (AllGather design).

Sequence-parallel over N=H*W across 8 cores, per the sharding hint's
tensor-parallel scheme: each core computes the gate + projections ONLY for
its own 1152-pixel chunk, then phi [32,1152] and G^T [128,9*64] are
AllGathered (DRAM-staged collectives) to form the full phi [32,9216] /
G [128,72*64] every core needs for its n-slice of the attention.

Per core:
  Phase A (one pool scope, no mid barriers): identity-matmul PE p-state
    warm-up under the input DMAs; 3x stride-2 conv gating unit (fp16 PE,
    lrelu = 0.6x+0.4|x| via ACT Abs + DVE stt); conv3 emits pre-transposed
    y3T halves; bilinear-upsample columns for the OWN chunk only via the
    per-core krC input (y3T^T @ krC) -> sigmoid -> fp16 gate-mul ->
    phi/theta/G^T projections.  The AllGathers + Exp table load launch
    outside the phase-A pools so no close-barrier gates pass-1 on them.
  Pass 1: 55 exp instructions (52x1536 cols = 1.33 m-tiles each, then
    tile-aligned 768/1152/1152 tails; the fp8 cache is contiguous so spans
    may cross m-tile boundaries): PE score matmuls fT = phi_tile^T
    theta_chunk into a 2-buffer PSUM ring, ACT exp(f - 2.5) written
    straight to a float8e5 cache (e5m2: wide range so the softmax
    denominator doesn't lose its tail to subnormal flushing; e4m3 loses
    ~10% of Z's mass).  Z[m] partials via DVE dead-store tensor_scalar
    accumulation over the cache (2x SBUF mode), except the final two
    tile-aligned instrs which use ACT's f32 accumulator (shortest path
    into the last Z-AllReduce); Z is AllReduced in 6 segments.
  Pass 2: per segment G is scaled by GSCALE/Z and split into fp8e4 high +
    residual parts; fp8 DoubleRow matmuls (2 m-tiles/instr, 0.5 cy/col,
    e4 stationary x e5 moving) accumulate into per-segment PSUM banks,
    pace-interleaved between later pass-1 instrs (margins keep not-ready
    units from parking at the PE queue head, which would starve ACT);
    DVE adds across segments, final read-out divides by GSCALE and adds
    the gated residual.  The last two segments drain after the loop behind
    a PE warm-up burst that keeps the p-state up through the final Z
    AllReduce latency.
  Host concatenates the per-core n-chunks.

Single-device build (the TimelineSim timing variant) replaces each
collective with one DRAM-hop DMA (upload straight to the gathered buffer);
landing DMAs are modeled in full.
"""

import sys

if "/opt/trn_rl_repo" not in sys.path:
    sys.path.insert(0, "/opt/trn_rl_repo")

import numpy as np

NCORES = 8
C, CI, H, W = 64, 32, 96, 96
N = H * W            # 9216
CH = N // NCORES     # 1152 pixels per core
MT = N // 128        # 72 m-tiles of 128
TPC = MT // NCORES   # 9 own m-tiles per core
EXP_BIAS = -2.5      # keeps exp(f+bias) <= ~16k < 57344 (e5m2 max) while
                     # minimizing subnormal flushing of tiny softmax terms
GSCALE = 64.0 * float(np.exp(-2.5 + 7.5))
                     # pre-scale so G*GSCALE/Z clears the e4m3 subnormal
                     # floor; tracks EXP_BIAS (Z scales with exp(bias))
SEGS = ((0, 22), (22, 40), (40, 54), (54, 64), (64, 70), (70, 72))
EIW = 1536           # exp-instruction width (cols)
# 52 x 1536-col instrs, then tile-aligned tails (768, 1152, 1152): the last
# two instrs cover exactly tiles 70 / 71 so their Z comes from the ACT f32
# accumulator (saves the DVE round trip on the final Z-AllReduce chain)
INSTRS = tuple([(i * EIW, EIW) for i in range(52)]
               + [(52 * EIW, 768), (70 * CH, CH), (71 * CH, CH)])
MARGINS = (7, 8, 7, 4, 99, 99)  # exp-instrs between AR issue and pass-2
                                # emit; last two segs drain after the loop
BUDGET = (3, 4)      # pass-2 units per exp instr (early, late)
RESID = True         # add an fp8 residual pass for G (extra accuracy)
# n-chunk subtiles for the two PSUM ring halves (bank-boundary aligned)
SUBS0 = ((0, 512), (512, 512), (1024, 128))
SUBS1 = ((0, 384), (384, 512), (896, 256))
YSUBS = ((0, 512), (512, 512), (1024, 128))  # pass-2 output subtiles
GSUBS = ((0, 384), (384, 512), (896, 256))   # gate-pipeline subtiles: the
                                             # first 384 cols feed the phi
                                             # chain for exp-instr 0

_compiled = {}


def _zmode(j):
    """Z accumulation engine per tile: DVE only (the dead-store
    tensor_scalar opcode does not exist on GPSIMD, and ACT's accumulator
    cannot be used because exp instructions span m-tile boundaries)."""
    return "dve"


def _seg_of(j):
    for k, (j0, j1) in enumerate(SEGS):
        if j0 <= j < j1:
            return k, j0
    raise ValueError(j)


def _build(single=False):
    import concourse.bacc as bacc
    import concourse.bass as bass
    import concourse.mybir as mybir
    import concourse.tile as tile
    from concourse import masks

    f16 = mybir.dt.float16
    f32 = mybir.dt.float32
    f8 = mybir.dt.float8e4
    f8w = mybir.dt.float8e5   # exp cache: wide range so tiny softmax terms
                              # aren't flushed (Z would lose ~10% of its mass)
    DR = mybir.MatmulPerfMode.DoubleRow
    AF = mybir.ActivationFunctionType
    ALU = mybir.AluOpType

    nc = bacc.Bacc("TRN2", target_bir_lowering=False, debug=False,
                   num_devices=1 if single else NCORES)

    xpad_io = nc.dram_tensor("xpad", [C, 98, 98], f16, kind="ExternalInput")
    w1_io = nc.dram_tensor("w1", [C, 9 * C], f16, kind="ExternalInput")
    w2_io = nc.dram_tensor("w2", [C, 9 * C], f16, kind="ExternalInput")
    w3_io = nc.dram_tensor("w3", [C, 9 * C], f16, kind="ExternalInput")
    twT_io = nc.dram_tensor("twT", [C, CI], f16, kind="ExternalInput")
    pwT_io = nc.dram_tensor("pwT", [C, CI], f16, kind="ExternalInput")
    gw_io = nc.dram_tensor("gw", [CI, C], f32, kind="ExternalInput")
    WwT_io = nc.dram_tensor("WwT", [CI, C], f32, kind="ExternalInput")
    xch_io = nc.dram_tensor("xch", [C, CH], f16, kind="ExternalInput")
    krC_io = nc.dram_tensor("krC", [144, CH], f16, kind="ExternalInput")
    out_io = nc.dram_tensor("out", [C, CH], f32, kind="ExternalOutput")

    groups = [list(range(NCORES))]

    with tile.TileContext(nc) as tc:
        with tc.tile_pool(name="persist", bufs=1) as pp, \
             tc.tile_pool(name="dram", bufs=1, space="DRAM") as dp:
            # per-segment Z tiles so the AR DMA reads never alias later writes
            zsumk = [pp.tile([128, j1 - j0], f32, name=f"zsum{k}")
                     for k, (j0, j1) in enumerate(SEGS)]
            zredk = [pp.tile([128, j1 - j0], f32, name=f"zred{k}")
                     for k, (j0, j1) in enumerate(SEGS)]
            nb5 = pp.tile([128, 1], f32)
            nc.gpsimd.memset(nb5[:], EXP_BIAS)
            zin = [dp.tile([128, j1 - j0], f32, name=f"zin{k}")
                   for k, (j0, j1) in enumerate(SEGS)]
            zout = [dp.tile([128, j1 - j0], f32, addr_space="Shared",
                            name=f"zout{k}")
                    for k, (j0, j1) in enumerate(SEGS)]
            phin = dp.tile([CI, CH], f16, name="phin")
            phout = dp.tile([NCORES, CI, CH], f16, addr_space="Shared",
                            name="phout")
            gin = dp.tile([128, TPC * C], f16, name="gin")
            gout = dp.tile([NCORES, 128, TPC * C], f16, addr_space="Shared",
                           name="gout")

            with tc.tile_pool(name="hand", bufs=1) as hp:
                phi16 = hp.tile([CI, N], f16)
                th16 = hp.tile([CI, CH], f16)
                G16 = hp.tile([128, MT * C], f16)
                G3 = G16[:].rearrange("p (j c) -> p j c", c=C)
                G8 = hp.tile([128, MT * C], f8)
                G83 = G8[:].rearrange("p (j c) -> p j c", c=C)
                R8 = hp.tile([128, MT * C], f8)
                R83 = R8[:].rearrange("p (j c) -> p j c", c=C)
                xgc16 = hp.tile([C, CH], f16)
                outsb = hp.tile([C, CH], f32)
                ofold = hp.tile([C, CH], f32)  # segs 0-3 sum, pre-folded
                # (exp-table load is implicit before the first pass-1 exp;
                # it hides behind the phi AllGather landing wait)
                zdeadV = hp.tile([128, CH], f8w)  # dead stores for Z accum
                zdeadP = hp.tile([128, CH], f8w)  # (same dtype as the cache)
                phiown = hp.tile([CI, CH], f16)
                gown = hp.tile([128, TPC * C], f16)
                s8 = hp.tile([128, MT * CH], f8w)
                s83 = s8[:].rearrange("p (j n) -> p j n", n=CH)

                # ==================== PHASE A ====================
                # single merged pool scope: no mid-phase close barrier
                # between the convs and the gate/projection pipeline
                with tc.tile_pool(name="pa", bufs=1) as pa, \
                     tc.tile_pool(name="paps", bufs=2, space="PSUM") as paps:
                    y3Ta = pa.tile([72, C], f16)
                    y3Tb = pa.tile([72, C], f16)
                    # preload the Sigmoid table while input DMAs fly
                    tld0 = pa.tile([128, 1], f32)
                    nc.scalar.activation(tld0[:], nb5[:], AF.Sigmoid)
                    # ramp the PE p-state during the input-DMA wait so conv1
                    # runs at full speed from its first matmul (identity
                    # needs no DMA)
                    ident = pa.tile([C, C], f16)
                    masks.make_identity(nc, ident[:])
                    wmps = paps.tile([C, C], f32, tag="warm", name="wmps",
                                     bufs=1)
                    for _ in range(140):
                        nc.tensor.matmul(wmps[:], ident[:], ident[:],
                                         start=True, stop=True,
                                         skip_group_check=True)

                    # conv-critical DMAs first: HWDGE is one serial queue,
                    # and conv1 must run gapless to keep the PE p-state up
                    xpad = pa.tile([C, 98, 98], f16)
                    w1sb = pa.tile([C, 9 * C], f16)
                    nc.sync.dma_start(xpad[:, 0:18, :], xpad_io[:, 0:18, :])
                    nc.sync.dma_start(w1sb[:], w1_io[:])
                    nc.sync.dma_start(xpad[:, 18:50, :], xpad_io[:, 18:50, :])
                    nc.sync.dma_start(xpad[:, 50:98, :], xpad_io[:, 50:98, :])
                    w2sb = pa.tile([C, 9 * C], f16)
                    nc.sync.dma_start(w2sb[:], w2_io[:])
                    w3sb = pa.tile([C, 9 * C], f16)
                    nc.sync.dma_start(w3sb[:], w3_io[:])
                    twT16 = pa.tile([C, CI], f16)
                    nc.sync.dma_start(twT16[:], twT_io[:])
                    pwT16 = pa.tile([C, CI], f16)
                    nc.sync.dma_start(pwT16[:], pwT_io[:])
                    gwsb = pa.tile([CI, C], f32)
                    nc.sync.dma_start(gwsb[:], gw_io[:])
                    WwTsb = pa.tile([CI, C], f32)
                    nc.sync.dma_start(WwTsb[:], WwT_io[:])
                    krCa = pa.tile([72, CH], f16)
                    nc.sync.dma_start(krCa[:], krC_io[0:72, :])
                    krCb = pa.tile([72, CH], f16)
                    nc.sync.dma_start(krCb[:], krC_io[72:144, :])
                    xchsb = pa.tile([C, CH], f16)
                    nc.sync.dma_start(xchsb[:], xch_io[:])

                    # conv1: 96x96 -> 48x48, stride 2, pad 1, lrelu(0.2)
                    y1p = pa.tile([C, 50, 50], f16)
                    nc.gpsimd.memset(y1p[:], 0.0)
                    for g in range(6):
                        ps1 = paps.tile([C, 8, 48], f32, tag="cv", name="ps1")
                        for t in range(9):
                            dy, dx = t // 3, t % 3
                            nc.tensor.matmul(
                                ps1[:], w1sb[:, t * C:(t + 1) * C],
                                xpad[:, 16 * g + dy: 16 * g + dy + 16: 2,
                                     dx: dx + 96: 2],
                                start=(t == 0), stop=(t == 8))
                        # lrelu(x) = 0.6*x + 0.4*|x| (only one PSUM input
                        # allowed per DVE op; ACT is idle during the convs)
                        ab1 = pa.tile([C, 8 * 48], f32, tag="ab1", name="ab1",
                                      bufs=2)
                        nc.scalar.activation(ab1[:], ps1[:], AF.Abs,
                                             scale=0.4)
                        nc.vector.scalar_tensor_tensor(
                            y1p[:, 1 + 8 * g: 9 + 8 * g, 1:49], ps1[:], 0.6,
                            ab1[:], op0=ALU.mult, op1=ALU.add)

                    # conv2: 48x48 -> 24x24
                    y2p = pa.tile([C, 26, 26], f16)
                    nc.gpsimd.memset(y2p[:], 0.0)
                    for g in range(2):
                        ps2 = paps.tile([C, 12, 24], f32, tag="cv", name="ps2")
                        for t in range(9):
                            dy, dx = t // 3, t % 3
                            nc.tensor.matmul(
                                ps2[:], w2sb[:, t * C:(t + 1) * C],
                                y1p[:, 24 * g + dy: 24 * g + dy + 24: 2,
                                    dx: dx + 48: 2],
                                start=(t == 0), stop=(t == 8))
                        ab2 = pa.tile([C, 12 * 24], f32, tag="ab2", name="ab2",
                                      bufs=2)
                        nc.scalar.activation(ab2[:], ps2[:], AF.Abs,
                                             scale=0.4)
                        nc.vector.scalar_tensor_tensor(
                            y2p[:, 1 + 12 * g: 13 + 12 * g, 1:25], ps2[:], 0.6,
                            ab2[:], op0=ALU.mult, op1=ALU.add)

                    # conv3: 24x24 -> 12x12 (no activation), then PE
                    # transpose into y3T[(row, col), c] halves
                    ps3 = paps.tile([C, 12, 12], f32, tag="cv", name="ps3")
                    for t in range(9):
                        dy, dx = t // 3, t % 3
                        nc.tensor.matmul(
                            ps3[:], w3sb[:, t * C:(t + 1) * C],
                            y2p[:, dy: dy + 24: 2, dx: dx + 24: 2],
                            start=(t == 0), stop=(t == 8))
                    y3f = pa.tile([C, 144], f16)
                    nc.vector.tensor_copy(y3f[:], ps3[:])
                    for hh, y3t in ((0, y3Ta), (1, y3Tb)):
                        pst = paps.tile([72, C], f16, tag="cv", name="pst")
                        nc.tensor.transpose(
                            pst[:], y3f[:, 72 * hh:72 * (hh + 1)], ident[:])
                        nc.vector.tensor_copy(y3t[:], pst[:])

                    # E^T = gw^T WwT [C, C]
                    eps = paps.tile([C, 512], f32, tag="prj", name="eps",
                                    bufs=3)
                    nc.tensor.matmul(eps[:, 0:C], gwsb[:], WwTsb[:],
                                     start=True, stop=True)
                    ET16 = hp.tile([C, C], f16)
                    nc.vector.tensor_copy(ET16[:], eps[:, 0:C])

                    # gate pipeline: all krons first (kron -> sigmoid ->
                    # fp16 gate-mul per sub), then the phi chain (it feeds
                    # the AllGather = the pass-1 critical path), then theta
                    gtc = pa.tile([C, CH], f16)
                    for o0, w in GSUBS:
                        kps = paps.tile([C, 512], f32, tag="prj",
                                        name="kps", bufs=3)
                        nc.tensor.matmul(kps[:, 0:w], y3Ta[:],
                                         krCa[:, o0:o0 + w],
                                         start=True, stop=False)
                        nc.tensor.matmul(kps[:, 0:w], y3Tb[:],
                                         krCb[:, o0:o0 + w],
                                         start=False, stop=True)
                        nc.scalar.activation(gtc[:, o0:o0 + w],
                                             kps[:, 0:w], AF.Sigmoid)
                        nc.vector.tensor_mul(xgc16[:, o0:o0 + w],
                                             gtc[:, o0:o0 + w],
                                             xchsb[:, o0:o0 + w])
                    # preload the Exp table during the gate pipeline: the read
                    # of gtc pins this after sigmoid0 (it cannot be hoisted
                    # to t=0 where the sigmoid load would evict it again)
                    tld1 = pa.tile([C, 1], f32)
                    nc.scalar.activation(tld1[:], gtc[:, 0:1], AF.Exp)
                    for o0, w in GSUBS:
                        pps = paps.tile([C, 512], f32, tag="prj",
                                        name="pps", bufs=3)
                        nc.tensor.matmul(pps[0:CI, 0:w], pwT16[:],
                                         xgc16[:, o0:o0 + w],
                                         start=True, stop=True)
                        nc.vector.tensor_copy(phiown[:, o0:o0 + w],
                                              pps[0:CI, 0:w])
                    for o0, w in GSUBS:
                        tps = paps.tile([C, 512], f32, tag="prj",
                                        name="tps", bufs=3)
                        nc.tensor.matmul(tps[0:CI, 0:w], twT16[:],
                                         xgc16[:, o0:o0 + w],
                                         start=True, stop=True)
                        # (GPSIMD cannot read PSUM on HW: copies on DVE)
                        nc.vector.tensor_copy(th16[:, o0:o0 + w],
                                              tps[0:CI, 0:w])

                    # own G^T tiles [128, 9*C] (AllGathered later)
                    gps = paps.tile([128, TPC * C], f32, tag="gps",
                                    name="gps", bufs=1)
                    for u in range(TPC):
                        nc.tensor.matmul(gps[:, u * C:(u + 1) * C],
                                         xgc16[:, u * 128:(u + 1) * 128],
                                         ET16[:], start=True, stop=True)
                    nc.vector.tensor_copy(gown[:], gps[:])

                # ====== PASS 1 with seg-interleaved fp8 PASS 2 + ARs ======
                with tc.tile_pool(name="p1ps", bufs=2, space="PSUM") as p1ps, \
                     tc.tile_pool(name="p2ps", bufs=2, space="PSUM") as p2ps, \
                     tc.tile_pool(name="p2", bufs=1) as p2:
                    # AllGathers emitted inside this scope so no pool-close
                    # barrier or clock alignment gates pass-1 on them.
                    # single-mode convention: ONE DRAM hop stands in for
                    # upload+collective; landing DMAs are modeled in full.
                    # per-sub uploads pipeline behind the phiown copies so
                    # the first landed piece arrives ~3us earlier (Shared
                    # phout allows only one writer, so single mode lands
                    # the r=0 pieces straight from phin - same hop count)
                    for o0, w in GSUBS:
                        nc.sync.dma_start(phin[:, o0:o0 + w],
                                          phiown[:, o0:o0 + w])
                    if not single:
                        nc.gpsimd.collective_compute(
                            "AllGather", ALU.bypass, replica_groups=groups,
                            ins=[phin.opt()], outs=[phout.opt()])
                    r0src = phin if single else phout[0]
                    # land r=0 in 2 pieces: cols 0-512 unblock pass-1
                    # instrs 0-2, the rest covers tiles 4-8
                    nc.sync.dma_start(phi16[:, 0:384], r0src[:, 0:384])
                    nc.sync.dma_start(phi16[:, 384:CH], r0src[:, 384:CH])
                    nc.sync.dma_start(
                        phi16[:, CH:].rearrange("c (r n) -> c r n",
                                                r=NCORES - 1),
                        phout[1:, :, :].rearrange("r c n -> c r n"))

                    # warm the PE through the AG landing wait with fake
                    # pass-1 tiles read from phiown (already in SBUF)
                    for _ in range(3):
                        wfps = p1ps.tile([128, EIW], f32, tag="fps",
                                         name="fps")
                        for o0 in range(0, EIW, 512):
                            nc.tensor.matmul(wfps[:, o0:o0 + 512],
                                             phiown[:, 0:128],
                                             th16[:, 0:512],
                                             start=True, stop=True)

                    def emit_G_ag():
                        if single:
                            nc.sync.dma_start(gout[0, :, :], gown[:])
                        else:
                            nc.sync.dma_start(gin[:], gown[:])
                            nc.gpsimd.collective_compute(
                                "AllGather", ALU.bypass,
                                replica_groups=groups,
                                ins=[gin.opt()], outs=[gout.opt()])
                        nc.sync.dma_start(
                            G16[:].rearrange("p (r n) -> p r n", r=NCORES),
                            gout[:].rearrange("r p n -> p r n"))

                    def pass1_instr(i):
                        # one exp instruction = up to 1.33 m-tiles; the fp8
                        # cache is contiguous so the exp span can cross
                        # m-tile boundaries; Z is per-m-tile off the cache,
                        # except single-tile-aligned instrs which use the
                        # ACT f32 accumulator directly
                        c0, wd = INSTRS[i]
                        fps = p1ps.tile([128, wd], f32, tag="fps",
                                        name="fps")
                        edges = {0, wd}
                        for b in range(512, wd, 512):
                            edges.add(b)
                        jlo, jhi = c0 // CH, (c0 + wd - 1) // CH
                        for j in range(jlo, jhi + 1):
                            if c0 < j * CH < c0 + wd:
                                edges.add(j * CH - c0)
                        edges = sorted(edges)
                        for a, b in zip(edges[:-1], edges[1:]):
                            j = (c0 + a) // CH
                            ta = c0 + a - j * CH
                            nc.tensor.matmul(fps[:, a:b],
                                             phi16[:, j * 128:(j + 1) * 128],
                                             th16[:, ta:ta + (b - a)],
                                             start=True, stop=True)
                        aligned = (wd == CH and c0 % CH == 0)
                        if aligned:
                            j = c0 // CH
                            k, j0 = _seg_of(j)
                            nc.scalar.activation(
                                s8[:, c0:c0 + wd], fps[:], AF.Exp,
                                bias=nb5[:], scale=1.0,
                                accum_out=zsumk[k][:, j - j0:j - j0 + 1])
                            return
                        nc.scalar.activation(s8[:, c0:c0 + wd], fps[:],
                                             AF.Exp, bias=nb5[:], scale=1.0)
                        # Z for every m-tile completed by this instruction
                        for j in range(jlo, jhi + 1):
                            if (j + 1) * CH <= c0 + wd:
                                k, j0 = _seg_of(j)
                                zcol = zsumk[k][:, j - j0:j - j0 + 1]
                                nc.vector.tensor_scalar(
                                    zdeadV[:], s83[:, j, :], 1.0, 0.0,
                                    op0=ALU.mult, op1=ALU.add,
                                    accum_out=zcol)

                    def allreduce(k):
                        # single-mode convention (as for the AllGathers):
                        # one DRAM hop stands in for upload+collective
                        if single:
                            nc.sync.dma_start(zout[k][:], zsumk[k][:])
                        else:
                            nc.sync.dma_start(zin[k][:], zsumk[k][:])
                            nc.gpsimd.collective_compute(
                                "AllReduce", ALU.add,
                                replica_groups=groups,
                                ins=[zin[k].opt()], outs=[zout[k].opt()])
                        nc.sync.dma_start(zredk[k][:], zout[k][:])

                    def scale_G(k):
                        j0, j1 = SEGS[k]
                        ln = j1 - j0
                        zf = p2.tile([128, 22], f32, tag="zf", name="zf",
                                     bufs=2)
                        # 1/(Z/GSCALE) = GSCALE/Z
                        nc.vector.tensor_scalar(zf[:, 0:ln], zredk[k][:],
                                                1.0 / GSCALE, None,
                                                op0=ALU.mult)
                        rz = p2.tile([128, 22], f32, tag="rz", name="rz",
                                     bufs=2)
                        nc.vector.reciprocal(rz[:, 0:ln], zf[:, 0:ln])
                        rzb = rz[:, 0:ln].unsqueeze(-1).to_broadcast(
                            (128, ln, C))
                        nc.vector.tensor_mul(G3[:, j0:j1, :],
                                             G3[:, j0:j1, :], rzb)
                        nc.vector.tensor_copy(G83[:, j0:j1, :],
                                              G3[:, j0:j1, :])
                        if RESID:
                            # split G into fp8 high + fp8 residual parts
                            rt = p2.tile([128, 22 * C], f16, tag="rt",
                                         name="rt", bufs=2)
                            rt3 = rt[:].rearrange("p (j c) -> p j c", c=C)
                            nc.vector.tensor_sub(rt3[:, 0:ln, :],
                                                 G3[:, j0:j1, :],
                                                 G83[:, j0:j1, :])
                            nc.vector.tensor_copy(R83[:, j0:j1, :],
                                                  rt3[:, 0:ln, :])

                    # pass-2 work units: (k, ci, u); per-segment PSUM
                    # accumulation, DVE adds across segments into outsb
                    units = []
                    for k in range(len(SEGS)):
                        j0, j1 = SEGS[k]
                        for ci in range(len(YSUBS)):
                            for u in range((j1 - j0) // 2):
                                units.append((k, ci, u))
                    emitted = 0
                    cur_ps = {}

                    def emit_unit():
                        nonlocal emitted
                        k, ci, u = units[emitted]
                        j0, j1 = SEGS[k]
                        o0, w = YSUBS[ci]
                        npr = (j1 - j0) // 2
                        jj = j0 + 2 * u
                        if u == 0:
                            cur_ps[ci] = p2ps.tile([C, 512], f32, tag="yps",
                                                   name="yps")
                        yp = cur_ps[ci]
                        nc.tensor.matmul(
                            yp[:, 0:w], G83[:, jj:jj + 2, :],
                            s83[:, jj:jj + 2, o0:o0 + w],
                            start=(u == 0), stop=(not RESID and u == npr - 1),
                            perf_mode=DR, skip_group_check=True)
                        if RESID:
                            nc.tensor.matmul(
                                yp[:, 0:w], R83[:, jj:jj + 2, :],
                                s83[:, jj:jj + 2, o0:o0 + w],
                                start=False, stop=(u == npr - 1),
                                perf_mode=DR, skip_group_check=True)
                        if u == npr - 1:
                            osl = outsb[:, o0:o0 + w]
                            if k == 0:
                                nc.vector.tensor_copy(osl, yp[:, 0:w])
                            else:
                                nc.vector.tensor_add(osl, osl, yp[:, 0:w])
                            if k == 3:
                                # pre-fold GSCALE + gated residual for segs
                                # 0-3 NOW (hidden in the loop) so the final
                                # drain read-out is a single stt per sub
                                nc.vector.scalar_tensor_tensor(
                                    ofold[:, o0:o0 + w], osl, 1.0 / GSCALE,
                                    xgc16[:, o0:o0 + w],
                                    op0=ALU.mult, op1=ALU.add)
                        emitted += 1

                    # m-tile j's exp completes during exp-instr ei(j)
                    def ei_of(j):
                        end = (j + 1) * CH
                        for i, (c0, wd) in enumerate(INSTRS):
                            if c0 + wd >= end:
                                return i
                        raise ValueError(j)

                    seg_ei = [ei_of(s[1] - 1) for s in SEGS]
                    seg_units = [sum(1 for x in units if x[0] <= k)
                                 for k in range(len(SEGS))]
                    avail = [0]

                    def pump(i):
                        if i == 2:
                            emit_G_ag()
                        for k in range(len(SEGS)):
                            if i == seg_ei[k]:
                                allreduce(k)
                                scale_G(k)
                            if (k < len(SEGS) - 2
                                    and i == seg_ei[k] + MARGINS[k]):
                                # last 2 segs drain after the loop, behind
                                # the PE warm-up (parked units would block
                                # the warm-up and drop the p-state)
                                avail[0] = seg_units[k]
                        budget = (BUDGET[0] if i < 30 else
                                  3 if i >= 52 else BUDGET[1])
                        while emitted < avail[0] and budget > 0:
                            emit_unit()
                            budget -= 1

                    for i in range(len(INSTRS)):
                        pass1_instr(i)
                        pump(i)
                    # keep the PE p-state warm through the final Z-AR wait:
                    # re-run an already-satisfied pair into a scratch bank
                    wps = p2ps.tile([C, 512], f32, tag="yps", name="wps")
                    for _ in range(16):
                        nc.tensor.matmul(wps[:], G83[:, 0:2, :],
                                         s83[:, 0:2, 0:512],
                                         start=True, stop=True, perf_mode=DR,
                                         skip_group_check=True)
                    # segs <=3 stragglers, then segs 4+5 merged per output
                    # sub: one PSUM accumulation spanning both, and the
                    # final read-out is a single stt (yp/GSCALE + ofold)
                    while emitted < seg_units[3]:
                        emit_unit()
                    prs = [(k, u) for k in (4, 5)
                           for u in range((SEGS[k][1] - SEGS[k][0]) // 2)]
                    for ci, (o0, w) in enumerate(YSUBS):
                        yp = p2ps.tile([C, 512], f32, tag="yps", name="yps")
                        for pi, (k, u) in enumerate(prs):
                            jj = SEGS[k][0] + 2 * u
                            first = pi == 0
                            last = pi == len(prs) - 1
                            nc.tensor.matmul(
                                yp[:, 0:w], G83[:, jj:jj + 2, :],
                                s83[:, jj:jj + 2, o0:o0 + w],
                                start=first, stop=(not RESID and last),
                                perf_mode=DR, skip_group_check=True)
                            if RESID:
                                nc.tensor.matmul(
                                    yp[:, 0:w], R83[:, jj:jj + 2, :],
                                    s83[:, jj:jj + 2, o0:o0 + w],
                                    start=False, stop=last,
                                    perf_mode=DR, skip_group_check=True)
                        osl = outsb[:, o0:o0 + w]
                        nc.vector.scalar_tensor_tensor(
                            osl, yp[:, 0:w], 1.0 / GSCALE,
                            ofold[:, o0:o0 + w], op0=ALU.mult, op1=ALU.add)
                        nc.sync.dma_start(out_io[:, o0:o0 + w], osl)

    nc.compile()
    return nc


def get_program():
    if "nc" not in _compiled:
        _compiled["nc"] = _build()
    return _compiled["nc"]


def _bilinear_kron():
    """K[(k,j), (R,Cc)] = uv[R,k]*uv[Cc,j] for x8 bilinear upsample 12->96
    (align_corners=False, edge-clamped), split into two 72-row halves."""
    uv = np.zeros((96, 12), np.float64)
    for R in range(96):
        t = (R + 0.5) / 8.0 - 0.5
        k0 = int(np.floor(t))
        fr = t - k0
        for k, wt in ((k0, 1.0 - fr), (k0 + 1, fr)):
            kc = min(max(k, 0), 11)
            uv[R, kc] += wt
    K = np.einsum("Rk,Cj->kjRC", uv, uv).reshape(144, 9216)
    return np.ascontiguousarray(K).astype(np.float16)


def make_in_maps(inputs):
    f16 = np.float16
    x = np.asarray(inputs["x"], np.float32).reshape(C, H, W)
    xflat = np.ascontiguousarray(x.reshape(C, N))
    xpad = np.zeros((C, 98, 98), f16)
    xpad[:, 1:97, 1:97] = x.astype(f16)
    krF = _bilinear_kron()

    def conv_w(w):
        # [o, i, dy, dx] -> [i, (dy dx), o]
        return np.ascontiguousarray(
            np.asarray(w, np.float32).transpose(1, 2, 3, 0).reshape(C, 9 * C)
        ).astype(f16)

    base = {
        "xpad": xpad,
        "w1": conv_w(inputs["d1_w"]),
        "w2": conv_w(inputs["d2_w"]),
        "w3": conv_w(inputs["d3_w"]),
        "twT": np.ascontiguousarray(
            np.asarray(inputs["th_w"], np.float32)[:, :, 0, 0].T).astype(f16),
        "pwT": np.ascontiguousarray(
            np.asarray(inputs["ph_w"], np.float32)[:, :, 0, 0].T).astype(f16),
        "gw": np.ascontiguousarray(
            np.asarray(inputs["g_w"], np.float32)[:, :, 0, 0]),
        "WwT": np.ascontiguousarray(
            np.asarray(inputs["W_w"], np.float32)[:, :, 0, 0].T),
    }
    in_maps = []
    for k in range(NCORES):
        m = dict(base)
        m["xch"] = np.ascontiguousarray(
            xflat[:, k * CH:(k + 1) * CH]).astype(f16)
        m["krC"] = np.ascontiguousarray(krF[:, k * CH:(k + 1) * CH])
        in_maps.append(m)
    return in_maps


def kernel(**inputs):
    from concourse import bass_utils

    nc = get_program()
    in_maps = make_in_maps(inputs)
    res = bass_utils.run_bass_kernel_spmd(nc, in_maps,
                                          core_ids=list(range(NCORES)))
    out = np.concatenate([res.results[k]["out"] for k in range(NCORES)], axis=1)
    return out.reshape(1, C, H, W).astype(np.float32)


# revision 114
# speedup vs baseline: 1.0254x; 1.0007x over previous
"""Trainium2 Bass kernel for AttentiveNonLocalBlock2D (AllGather design).

Sequence-parallel over N=H*W across 8 cores, per the sharding hint's
tensor-parallel scheme: each core computes the gate + projections ONLY for
its own 1152-pixel chunk, then phi [32,1152] and G^T [128,9*64] are
AllGathered (DRAM-staged collectives) to form the full phi [32,9216] /
G [128,72*64] every core needs for its n-slice of the attention.

Per core:
  Phase A (one pool scope, no mid barriers): identity-matmul PE p-state
    warm-up under the input DMAs; 3x stride-2 conv gating unit (fp16 PE,
    lrelu = 0.6x+0.4|x| via ACT Abs + DVE stt); conv3 emits pre-transposed
    y3T halves; bilinear-upsample columns for the OWN chunk only via the
    per-core krC input (y3T^T @ krC) -> sigmoid -> fp16 gate-mul ->
    phi/theta/G^T projections.  The AllGathers + Exp table load launch
    outside the phase-A pools so no close-barrier gates pass-1 on them.
  Pass 1: 55 exp instructions (52x1536 cols = 1.33 m-tiles each, then
    tile-aligned 768/1152/1152 tails; the fp8 cache is contiguous so spans
    may cross m-tile boundaries): PE score matmuls fT = phi_tile^T
    theta_chunk into a 2-buffer PSUM ring, ACT exp(f - 2.5) written
    straight to a float8e5 cache (e5m2: wide range so the softmax
    denominator doesn't lose its tail to subnormal flushing; e4m3 loses
    ~10% of Z's mass).  Z[m] partials via DVE dead-store tensor_scalar
    accumulation over the cache (2x SBUF mode), except the final two
    tile-aligned instrs which use ACT's f32 accumulator (shortest path
    into the last Z-AllReduce); Z is AllReduced in 6 segments.
  Pass 2: per segment G is scaled by GSCALE/Z and split into fp8e4 high +
    residual parts; fp8 DoubleRow matmuls (2 m-tiles/instr, 0.5 cy/col,
    e4 stationary x e5 moving) accumulate into per-segment PSUM banks,
    pace-interleaved between later pass-1 instrs (margins keep not-ready
    units from parking at the PE queue head, which would starve ACT);
    DVE adds across segments, final read-out divides by GSCALE and adds
    the gated residual.  The last two segments drain after the loop behind
    a PE warm-up burst that keeps the p-state up through the final Z
    AllReduce latency.
  Host concatenates the per-core n-chunks.

Single-device build (the TimelineSim timing variant) replaces each
collective with one DRAM-hop DMA (upload straight to the gathered buffer);
landing DMAs are modeled in full.
"""

import sys

if "/opt/trn_rl_repo" not in sys.path:
    sys.path.insert(0, "/opt/trn_rl_repo")

import numpy as np

NCORES = 8
C, CI, H, W = 64, 32, 96, 96
N = H * W            # 9216
CH = N // NCORES     # 1152 pixels per core
MT = N // 128        # 72 m-tiles of 128
TPC = MT // NCORES   # 9 own m-tiles per core
EXP_BIAS = -2.5      # keeps exp(f+bias) <= ~16k < 57344 (e5m2 max) while
                     # minimizing subnormal flushing of tiny softmax terms
GSCALE = 64.0 * float(np.exp(-2.5 + 7.5))
                     # pre-scale so G*GSCALE/Z clears the e4m3 subnormal
                     # floor; tracks EXP_BIAS (Z scales with exp(bias))
SEGS = ((0, 22), (22, 40), (40, 54), (54, 64), (64, 70), (70, 72))
EIW = 1536           # exp-instruction width (cols)
# 52 x 1536-col instrs, then tile-aligned tails (768, 1152, 1152): the last
# two instrs cover exactly tiles 70 / 71 so their Z comes from the ACT f32
# accumulator (saves the DVE round trip on the final Z-AllReduce chain)
INSTRS = tuple([(i * EIW, EIW) for i in range(52)]
               + [(52 * EIW, 768), (70 * CH, CH), (71 * CH, CH)])
MARGINS = (7, 8, 7, 4, 99, 99)  # exp-instrs between AR issue and pass-2
                                # emit; last two segs drain after the loop
BUDGET = (3, 4)      # pass-2 units per exp instr (early, late)
RESID = True         # add an fp8 residual pass for G (extra accuracy)
# n-chunk subtiles for the two PSUM ring halves (bank-boundary aligned)
SUBS0 = ((0, 512), (512, 512), (1024, 128))
SUBS1 = ((0, 384), (384, 512), (896, 256))
YSUBS = ((0, 512), (512, 512), (1024, 128))  # pass-2 output subtiles
GSUBS = ((0, 384), (384, 512), (896, 256))   # gate-pipeline subtiles: the
                                             # first 384 cols feed the phi
                                             # chain for exp-instr 0

_compiled = {}


def _zmode(j):
    """Z accumulation engine per tile: DVE only (the dead-store
    tensor_scalar opcode does not exist on GPSIMD, and ACT's accumulator
    cannot be used because exp instructions span m-tile boundaries)."""
    return "dve"


def _seg_of(j):
    for k, (j0, j1) in enumerate(SEGS):
        if j0 <= j < j1:
            return k, j0
    raise ValueError(j)


def _build(single=False):
    import concourse.bacc as bacc
    import concourse.bass as bass
    import concourse.mybir as mybir
    import concourse.tile as tile
    from concourse import masks

    f16 = mybir.dt.float16
    f32 = mybir.dt.float32
    f8 = mybir.dt.float8e4
    f8w = mybir.dt.float8e5   # exp cache: wide range so tiny softmax terms
                              # aren't flushed (Z would lose ~10% of its mass)
    DR = mybir.MatmulPerfMode.DoubleRow
    AF = mybir.ActivationFunctionType
    ALU = mybir.AluOpType

    nc = bacc.Bacc("TRN2", target_bir_lowering=False, debug=False,
                   num_devices=1 if single else NCORES)

    xpad_io = nc.dram_tensor("xpad", [C, 98, 98], f16, kind="ExternalInput")
    w1_io = nc.dram_tensor("w1", [C, 9 * C], f16, kind="ExternalInput")
    w2_io = nc.dram_tensor("w2", [C, 9 * C], f16, kind="ExternalInput")
    w3_io = nc.dram_tensor("w3", [C, 9 * C], f16, kind="ExternalInput")
    twT_io = nc.dram_tensor("twT", [C, CI], f16, kind="ExternalInput")
    pwT_io = nc.dram_tensor("pwT", [C, CI], f16, kind="ExternalInput")
    gw_io = nc.dram_tensor("gw", [CI, C], f32, kind="ExternalInput")
    WwT_io = nc.dram_tensor("WwT", [CI, C], f32, kind="ExternalInput")
    xch_io = nc.dram_tensor("xch", [C, CH], f16, kind="ExternalInput")
    krC_io = nc.dram_tensor("krC", [144, CH], f16, kind="ExternalInput")
    out_io = nc.dram_tensor("out", [C, CH], f32, kind="ExternalOutput")

    groups = [list(range(NCORES))]

    with tile.TileContext(nc) as tc:
        with tc.tile_pool(name="persist", bufs=1) as pp, \
             tc.tile_pool(name="dram", bufs=1, space="DRAM") as dp:
            # per-segment Z tiles so the AR DMA reads never alias later writes
            zsumk = [pp.tile([128, j1 - j0], f32, name=f"zsum{k}")
                     for k, (j0, j1) in enumerate(SEGS)]
            zredk = [pp.tile([128, j1 - j0], f32, name=f"zred{k}")
                     for k, (j0, j1) in enumerate(SEGS)]
            nb5 = pp.tile([128, 1], f32)
            nc.gpsimd.memset(nb5[:], EXP_BIAS)
            zin = [dp.tile([128, j1 - j0], f32, name=f"zin{k}")
                   for k, (j0, j1) in enumerate(SEGS)]
            zout = [dp.tile([128, j1 - j0], f32, addr_space="Shared",
                            name=f"zout{k}")
                    for k, (j0, j1) in enumerate(SEGS)]
            phin = dp.tile([CI, CH], f16, name="phin")
            phout = dp.tile([NCORES, CI, CH], f16, addr_space="Shared",
                            name="phout")
            gin = dp.tile([128, TPC * C], f16, name="gin")
            gout = dp.tile([NCORES, 128, TPC * C], f16, addr_space="Shared",
                           name="gout")

            with tc.tile_pool(name="hand", bufs=1) as hp:
                phi16 = hp.tile([CI, N], f16)
                th16 = hp.tile([CI, CH], f16)
                G16 = hp.tile([128, MT * C], f16)
                G3 = G16[:].rearrange("p (j c) -> p j c", c=C)
                G8 = hp.tile([128, MT * C], f8)
                G83 = G8[:].rearrange("p (j c) -> p j c", c=C)
                R8 = hp.tile([128, MT * C], f8)
                R83 = R8[:].rearrange("p (j c) -> p j c", c=C)
                xgc16 = hp.tile([C, CH], f16)
                outsb = hp.tile([C, CH], f32)
                ofold = hp.tile([C, CH], f32)  # segs 0-3 sum, pre-folded
                # (exp-table load is implicit before the first pass-1 exp;
                # it hides behind the phi AllGather landing wait)
                zdeadV = hp.tile([128, CH], f8w)  # dead stores for Z accum
                zdeadP = hp.tile([128, CH], f8w)  # (same dtype as the cache)
                phiown = hp.tile([CI, CH], f16)
                gown = hp.tile([128, TPC * C], f16)
                s8 = hp.tile([128, MT * CH], f8w)
                s83 = s8[:].rearrange("p (j n) -> p j n", n=CH)

                # ==================== PHASE A ====================
                # single merged pool scope: no mid-phase close barrier
                # between the convs and the gate/projection pipeline
                with tc.tile_pool(name="pa", bufs=1) as pa, \
                     tc.tile_pool(name="paps", bufs=2, space="PSUM") as paps:
                    y3Ta = pa.tile([72, C], f16)
                    y3Tb = pa.tile([72, C], f16)
                    # preload the Sigmoid table while input DMAs fly
                    tld0 = pa.tile([128, 1], f32)
                    nc.scalar.activation(tld0[:], nb5[:], AF.Sigmoid)
                    # ramp the PE p-state during the input-DMA wait so conv1
                    # runs at full speed from its first matmul (identity
                    # needs no DMA)
                    ident = pa.tile([C, C], f16)
                    masks.make_identity(nc, ident[:])
                    wmps = paps.tile([C, C], f32, tag="warm", name="wmps",
                                     bufs=1)
                    for _ in range(140):
                        nc.tensor.matmul(wmps[:], ident[:], ident[:],
                                         start=True, stop=True,
                                         skip_group_check=True)

                    # conv-critical DMAs first: HWDGE is one serial queue,
                    # and conv1 must run gapless to keep the PE p-state up
                    xpad = pa.tile([C, 98, 98], f16)
                    w1sb = pa.tile([C, 9 * C], f16)
                    nc.sync.dma_start(xpad[:, 0:18, :], xpad_io[:, 0:18, :])
                    nc.sync.dma_start(w1sb[:], w1_io[:])
                    nc.sync.dma_start(xpad[:, 18:50, :], xpad_io[:, 18:50, :])
                    nc.sync.dma_start(xpad[:, 50:98, :], xpad_io[:, 50:98, :])
                    w2sb = pa.tile([C, 9 * C], f16)
                    nc.sync.dma_start(w2sb[:], w2_io[:])
                    w3sb = pa.tile([C, 9 * C], f16)
                    nc.sync.dma_start(w3sb[:], w3_io[:])
                    twT16 = pa.tile([C, CI], f16)
                    nc.sync.dma_start(twT16[:], twT_io[:])
                    pwT16 = pa.tile([C, CI], f16)
                    nc.sync.dma_start(pwT16[:], pwT_io[:])
                    gwsb = pa.tile([CI, C], f32)
                    nc.sync.dma_start(gwsb[:], gw_io[:])
                    WwTsb = pa.tile([CI, C], f32)
                    nc.sync.dma_start(WwTsb[:], WwT_io[:])
                    krCa = pa.tile([72, CH], f16)
                    nc.sync.dma_start(krCa[:], krC_io[0:72, :])
                    krCb = pa.tile([72, CH], f16)
                    nc.sync.dma_start(krCb[:], krC_io[72:144, :])
                    xchsb = pa.tile([C, CH], f16)
                    nc.sync.dma_start(xchsb[:], xch_io[:])

                    # conv1: 96x96 -> 48x48, stride 2, pad 1, lrelu(0.2)
                    y1p = pa.tile([C, 50, 50], f16)
                    nc.gpsimd.memset(y1p[:], 0.0)
                    for g in range(6):
                        ps1 = paps.tile([C, 8, 48], f32, tag="cv", name="ps1")
                        for t in range(9):
                            dy, dx = t // 3, t % 3
                            nc.tensor.matmul(
                                ps1[:], w1sb[:, t * C:(t + 1) * C],
                                xpad[:, 16 * g + dy: 16 * g + dy + 16: 2,
                                     dx: dx + 96: 2],
                                start=(t == 0), stop=(t == 8))
                        # lrelu(x) = 0.6*x + 0.4*|x| (only one PSUM input
                        # allowed per DVE op; ACT is idle during the convs)
                        ab1 = pa.tile([C, 8 * 48], f32, tag="ab1", name="ab1",
                                      bufs=2)
                        nc.scalar.activation(ab1[:], ps1[:], AF.Abs,
                                             scale=0.4)
                        nc.vector.scalar_tensor_tensor(
                            y1p[:, 1 + 8 * g: 9 + 8 * g, 1:49], ps1[:], 0.6,
                            ab1[:], op0=ALU.mult, op1=ALU.add)

                    # conv2: 48x48 -> 24x24
                    y2p = pa.tile([C, 26, 26], f16)
                    nc.gpsimd.memset(y2p[:], 0.0)
                    for g in range(2):
                        ps2 = paps.tile([C, 12, 24], f32, tag="cv", name="ps2")
                        for t in range(9):
                            dy, dx = t // 3, t % 3
                            nc.tensor.matmul(
                                ps2[:], w2sb[:, t * C:(t + 1) * C],
                                y1p[:, 24 * g + dy: 24 * g + dy + 24: 2,
                                    dx: dx + 48: 2],
                                start=(t == 0), stop=(t == 8))
                        ab2 = pa.tile([C, 12 * 24], f32, tag="ab2", name="ab2",
                                      bufs=2)
                        nc.scalar.activation(ab2[:], ps2[:], AF.Abs,
                                             scale=0.4)
                        nc.vector.scalar_tensor_tensor(
                            y2p[:, 1 + 12 * g: 13 + 12 * g, 1:25], ps2[:], 0.6,
                            ab2[:], op0=ALU.mult, op1=ALU.add)

                    # conv3: 24x24 -> 12x12 (no activation), then PE
                    # transpose into y3T[(row, col), c] halves
                    ps3 = paps.tile([C, 12, 12], f32, tag="cv", name="ps3")
                    for t in range(9):
                        dy, dx = t // 3, t % 3
                        nc.tensor.matmul(
                            ps3[:], w3sb[:, t * C:(t + 1) * C],
                            y2p[:, dy: dy + 24: 2, dx: dx + 24: 2],
                            start=(t == 0), stop=(t == 8))
                    y3f = pa.tile([C, 144], f16)
                    nc.vector.tensor_copy(y3f[:], ps3[:])
                    for hh, y3t in ((0, y3Ta), (1, y3Tb)):
                        pst = paps.tile([72, C], f16, tag="cv", name="pst")
                        nc.tensor.transpose(
                            pst[:], y3f[:, 72 * hh:72 * (hh + 1)], ident[:])
                        nc.vector.tensor_copy(y3t[:], pst[:])

                    # E^T = gw^T WwT [C, C]
                    eps = paps.tile([C, 512], f32, tag="prj", name="eps",
                                    bufs=3)
                    nc.tensor.matmul(eps[:, 0:C], gwsb[:], WwTsb[:],
                                     start=True, stop=True)
                    ET16 = hp.tile([C, C], f16)
                    nc.vector.tensor_copy(ET16[:], eps[:, 0:C])

                    # gate pipeline: all krons first (kron -> sigmoid ->
                    # fp16 gate-mul per sub), then the phi chain (it feeds
                    # the AllGather = the pass-1 critical path), then theta
                    gtc = pa.tile([C, CH], f16)
                    for o0, w in GSUBS:
                        kps = paps.tile([C, 512], f32, tag="prj",
                                        name="kps", bufs=3)
                        nc.tensor.matmul(kps[:, 0:w], y3Ta[:],
                                         krCa[:, o0:o0 + w],
                                         start=True, stop=False)
                        nc.tensor.matmul(kps[:, 0:w], y3Tb[:],
                                         krCb[:, o0:o0 + w],
                                         start=False, stop=True)
                        nc.scalar.activation(gtc[:, o0:o0 + w],
                                             kps[:, 0:w], AF.Sigmoid)
                        nc.vector.tensor_mul(xgc16[:, o0:o0 + w],
                                             gtc[:, o0:o0 + w],
                                             xchsb[:, o0:o0 + w])
                    # preload the Exp table during the gate pipeline: the read
                    # of gtc pins this after sigmoid0 (it cannot be hoisted
                    # to t=0 where the sigmoid load would evict it again)
                    tld1 = pa.tile([C, 1], f32)
                    nc.scalar.activation(tld1[:], gtc[:, 0:1], AF.Exp)
                    for o0, w in GSUBS:
                        pps = paps.tile([C, 512], f32, tag="prj",
                                        name="pps", bufs=3)
                        nc.tensor.matmul(pps[0:CI, 0:w], pwT16[:],
                                         xgc16[:, o0:o0 + w],
                                         start=True, stop=True)
                        nc.vector.tensor_copy(phiown[:, o0:o0 + w],
                                              pps[0:CI, 0:w])
                    for o0, w in GSUBS:
                        tps = paps.tile([C, 512], f32, tag="prj",
                                        name="tps", bufs=3)
                        nc.tensor.matmul(tps[0:CI, 0:w], twT16[:],
                                         xgc16[:, o0:o0 + w],
                                         start=True, stop=True)
                        # (GPSIMD cannot read PSUM on HW: copies on DVE)
                        nc.vector.tensor_copy(th16[:, o0:o0 + w],
                                              tps[0:CI, 0:w])

                    # own G^T tiles [128, 9*C] (AllGathered later)
                    gps = paps.tile([128, TPC * C], f32, tag="gps",
                                    name="gps", bufs=1)
                    for u in range(TPC):
                        nc.tensor.matmul(gps[:, u * C:(u + 1) * C],
                                         xgc16[:, u * 128:(u + 1) * 128],
                                         ET16[:], start=True, stop=True)
                    nc.vector.tensor_copy(gown[:], gps[:])

                # ====== PASS 1 with seg-interleaved fp8 PASS 2 + ARs ======
                with tc.tile_pool(name="p1ps", bufs=2, space="PSUM") as p1ps, \
                     tc.tile_pool(name="p2ps", bufs=2, space="PSUM") as p2ps, \
                     tc.tile_pool(name="p2", bufs=1) as p2:
                    # AllGathers emitted inside this scope so no pool-close
                    # barrier or clock alignment gates pass-1 on them.
                    # single-mode convention: ONE DRAM hop stands in for
                    # upload+collective; landing DMAs are modeled in full.
                    # per-sub uploads pipeline behind the phiown copies so
                    # the first landed piece arrives ~3us earlier (Shared
                    # phout allows only one writer, so single mode lands
                    # the r=0 pieces straight from phin - same hop count)
                    for o0, w in GSUBS:
                        nc.sync.dma_start(phin[:, o0:o0 + w],
                                          phiown[:, o0:o0 + w])
                    if not single:
                        nc.gpsimd.collective_compute(
                            "AllGather", ALU.bypass, replica_groups=groups,
                            ins=[phin.opt()], outs=[phout.opt()])
                    r0src = phin if single else phout[0]
                    # land r=0 in 2 pieces: cols 0-512 unblock pass-1
                    # instrs 0-2, the rest covers tiles 4-8
                    nc.sync.dma_start(phi16[:, 0:384], r0src[:, 0:384])
                    nc.sync.dma_start(phi16[:, 384:CH], r0src[:, 384:CH])
                    nc.sync.dma_start(
                        phi16[:, CH:].rearrange("c (r n) -> c r n",
                                                r=NCORES - 1),
                        phout[1:, :, :].rearrange("r c n -> c r n"))

                    # warm the PE through the AG landing wait with fake
                    # pass-1 tiles read from phiown (already in SBUF)
                    for _ in range(3):
                        wfps = p1ps.tile([128, EIW], f32, tag="fps",
                                         name="fps")
                        for o0 in range(0, EIW, 512):
                            nc.tensor.matmul(wfps[:, o0:o0 + 512],
                                             phiown[:, 0:128],
                                             th16[:, 0:512],
                                             start=True, stop=True)

                    def emit_G_ag():
                        if single:
                            nc.sync.dma_start(gout[0, :, :], gown[:])
                        else:
                            nc.sync.dma_start(gin[:], gown[:])
                            nc.gpsimd.collective_compute(
                                "AllGather", ALU.bypass,
                                replica_groups=groups,
                                ins=[gin.opt()], outs=[gout.opt()])
                        nc.sync.dma_start(
                            G16[:].rearrange("p (r n) -> p r n", r=NCORES),
                            gout[:].rearrange("r p n -> p r n"))

                    def pass1_instr(i):
                        # one exp instruction = up to 1.33 m-tiles; the fp8
                        # cache is contiguous so the exp span can cross
                        # m-tile boundaries; Z is per-m-tile off the cache,
                        # except single-tile-aligned instrs which use the
                        # ACT f32 accumulator directly
                        c0, wd = INSTRS[i]
                        fps = p1ps.tile([128, wd], f32, tag="fps",
                                        name="fps")
                        edges = {0, wd}
                        for b in range(512, wd, 512):
                            edges.add(b)
                        jlo, jhi = c0 // CH, (c0 + wd - 1) // CH
                        for j in range(jlo, jhi + 1):
                            if c0 < j * CH < c0 + wd:
                                edges.add(j * CH - c0)
                        edges = sorted(edges)
                        for a, b in zip(edges[:-1], edges[1:]):
                            j = (c0 + a) // CH
                            ta = c0 + a - j * CH
                            nc.tensor.matmul(fps[:, a:b],
                                             phi16[:, j * 128:(j + 1) * 128],
                                             th16[:, ta:ta + (b - a)],
                                             start=True, stop=True)
                        aligned = (wd == CH and c0 % CH == 0)
                        if aligned:
                            j = c0 // CH
                            k, j0 = _seg_of(j)
                            nc.scalar.activation(
                                s8[:, c0:c0 + wd], fps[:], AF.Exp,
                                bias=nb5[:], scale=1.0,
                                accum_out=zsumk[k][:, j - j0:j - j0 + 1])
                            return
                        nc.scalar.activation(s8[:, c0:c0 + wd], fps[:],
                                             AF.Exp, bias=nb5[:], scale=1.0)
                        # Z for every m-tile completed by this instruction
                        for j in range(jlo, jhi + 1):
                            if (j + 1) * CH <= c0 + wd:
                                k, j0 = _seg_of(j)
                                zcol = zsumk[k][:, j - j0:j - j0 + 1]
                                nc.vector.tensor_scalar(
                                    zdeadV[:], s83[:, j, :], 1.0, 0.0,
                                    op0=ALU.mult, op1=ALU.add,
                                    accum_out=zcol)

                    def allreduce(k):
                        # single-mode convention (as for the AllGathers):
                        # one DRAM hop stands in for upload+collective
                        if single:
                            nc.sync.dma_start(zout[k][:], zsumk[k][:])
                        else:
                            nc.sync.dma_start(zin[k][:], zsumk[k][:])
                            nc.gpsimd.collective_compute(
                                "AllReduce", ALU.add,
                                replica_groups=groups,
                                ins=[zin[k].opt()], outs=[zout[k].opt()])
                        nc.sync.dma_start(zredk[k][:], zout[k][:])

                    def scale_G(k):
                        j0, j1 = SEGS[k]
                        ln = j1 - j0
                        zf = p2.tile([128, 22], f32, tag="zf", name="zf",
                                     bufs=2)
                        # 1/(Z/GSCALE) = GSCALE/Z
                        nc.vector.tensor_scalar(zf[:, 0:ln], zredk[k][:],
                                                1.0 / GSCALE, None,
                                                op0=ALU.mult)
                        rz = p2.tile([128, 22], f32, tag="rz", name="rz",
                                     bufs=2)
                        nc.vector.reciprocal(rz[:, 0:ln], zf[:, 0:ln])
                        rzb = rz[:, 0:ln].unsqueeze(-1).to_broadcast(
                            (128, ln, C))
                        nc.vector.tensor_mul(G3[:, j0:j1, :],
                                             G3[:, j0:j1, :], rzb)
                        nc.vector.tensor_copy(G83[:, j0:j1, :],
                                              G3[:, j0:j1, :])
                        if RESID:
                            # split G into fp8 high + fp8 residual parts
                            rt = p2.tile([128, 22 * C], f16, tag="rt",
                                         name="rt", bufs=2)
                            rt3 = rt[:].rearrange("p (j c) -> p j c", c=C)
                            nc.vector.tensor_sub(rt3[:, 0:ln, :],
                                                 G3[:, j0:j1, :],
                                                 G83[:, j0:j1, :])
                            nc.vector.tensor_copy(R83[:, j0:j1, :],
                                                  rt3[:, 0:ln, :])

                    # pass-2 work units: (k, ci, u); per-segment PSUM
                    # accumulation, DVE adds across segments into outsb
                    units = []
                    for k in range(len(SEGS)):
                        j0, j1 = SEGS[k]
                        for ci in range(len(YSUBS)):
                            for u in range((j1 - j0) // 2):
                                units.append((k, ci, u))
                    emitted = 0
                    cur_ps = {}

                    def emit_unit():
                        nonlocal emitted
                        k, ci, u = units[emitted]
                        j0, j1 = SEGS[k]
                        o0, w = YSUBS[ci]
                        npr = (j1 - j0) // 2
                        jj = j0 + 2 * u
                        if u == 0:
                            cur_ps[ci] = p2ps.tile([C, 512], f32, tag="yps",
                                                   name="yps")
                        yp = cur_ps[ci]
                        nc.tensor.matmul(
                            yp[:, 0:w], G83[:, jj:jj + 2, :],
                            s83[:, jj:jj + 2, o0:o0 + w],
                            start=(u == 0), stop=(not RESID and u == npr - 1),
                            perf_mode=DR, skip_group_check=True)
                        if RESID:
                            nc.tensor.matmul(
                                yp[:, 0:w], R83[:, jj:jj + 2, :],
                                s83[:, jj:jj + 2, o0:o0 + w],
                                start=False, stop=(u == npr - 1),
                                perf_mode=DR, skip_group_check=True)
                        if u == npr - 1:
                            osl = outsb[:, o0:o0 + w]
                            if k == 0:
                                nc.vector.tensor_copy(osl, yp[:, 0:w])
                            else:
                                nc.vector.tensor_add(osl, osl, yp[:, 0:w])
                            if k == 3:
                                # pre-fold GSCALE + gated residual for segs
                                # 0-3 NOW (hidden in the loop) so the final
                                # drain read-out is a single stt per sub
                                nc.vector.scalar_tensor_tensor(
                                    ofold[:, o0:o0 + w], osl, 1.0 / GSCALE,
                                    xgc16[:, o0:o0 + w],
                                    op0=ALU.mult, op1=ALU.add)
                        emitted += 1

                    # m-tile j's exp completes during exp-instr ei(j)
                    def ei_of(j):
                        end = (j + 1) * CH
                        for i, (c0, wd) in enumerate(INSTRS):
                            if c0 + wd >= end:
                                return i
                        raise ValueError(j)

                    seg_ei = [ei_of(s[1] - 1) for s in SEGS]
                    seg_units = [sum(1 for x in units if x[0] <= k)
                                 for k in range(len(SEGS))]
                    avail = [0]

                    def pump(i):
                        if i == 2:
                            emit_G_ag()
                        for k in range(len(SEGS)):
                            if i == seg_ei[k]:
                                allreduce(k)
                                scale_G(k)
                            if (k < len(SEGS) - 2
                                    and i == seg_ei[k] + MARGINS[k]):
                                # last 2 segs drain after the loop, behind
                                # the PE warm-up (parked units would block
                                # the warm-up and drop the p-state)
                                avail[0] = seg_units[k]
                        budget = (BUDGET[0] if i < 30 else
                                  3 if i >= 52 else
                                  3 if 38 <= i <= 43 else BUDGET[1])
                        while emitted < avail[0] and budget > 0:
                            emit_unit()
                            budget -= 1

                    for i in range(len(INSTRS)):
                        pass1_instr(i)
                        pump(i)
                    # keep the PE p-state warm through the final Z-AR wait:
                    # re-run an already-satisfied pair into a scratch bank
                    wps = p2ps.tile([C, 512], f32, tag="yps", name="wps")
                    for _ in range(16):
                        nc.tensor.matmul(wps[:], G83[:, 0:2, :],
                                         s83[:, 0:2, 0:512],
                                         start=True, stop=True, perf_mode=DR,
                                         skip_group_check=True)
                    # segs <=3 stragglers, then segs 4+5 merged per output
                    # sub: one PSUM accumulation spanning both, and the
                    # final read-out is a single stt (yp/GSCALE + ofold)
                    while emitted < seg_units[3]:
                        emit_unit()
                    prs = [(k, u) for k in (4, 5)
                           for u in range((SEGS[k][1] - SEGS[k][0]) // 2)]
                    for ci, (o0, w) in enumerate(YSUBS):
                        yp = p2ps.tile([C, 512], f32, tag="yps", name="yps")
                        for pi, (k, u) in enumerate(prs):
                            jj = SEGS[k][0] + 2 * u
                            first = pi == 0
                            last = pi == len(prs) - 1
                            nc.tensor.matmul(
                                yp[:, 0:w], G83[:, jj:jj + 2, :],
                                s83[:, jj:jj + 2, o0:o0 + w],
                                start=first, stop=(not RESID and last),
                                perf_mode=DR, skip_group_check=True)
                            if RESID:
                                nc.tensor.matmul(
                                    yp[:, 0:w], R83[:, jj:jj + 2, :],
                                    s83[:, jj:jj + 2, o0:o0 + w],
                                    start=False, stop=last,
                                    perf_mode=DR, skip_group_check=True)
                        osl = outsb[:, o0:o0 + w]
                        nc.vector.scalar_tensor_tensor(
                            osl, yp[:, 0:w], 1.0 / GSCALE,
                            ofold[:, o0:o0 + w], op0=ALU.mult, op1=ALU.add)
                        nc.sync.dma_start(out_io[:, o0:o0 + w], osl)

    nc.compile()
    return nc


def get_program():
    if "nc" not in _compiled:
        _compiled["nc"] = _build()
    return _compiled["nc"]


def _bilinear_kron():
    """K[(k,j), (R,Cc)] = uv[R,k]*uv[Cc,j] for x8 bilinear upsample 12->96
    (align_corners=False, edge-clamped), split into two 72-row halves."""
    uv = np.zeros((96, 12), np.float64)
    for R in range(96):
        t = (R + 0.5) / 8.0 - 0.5
        k0 = int(np.floor(t))
        fr = t - k0
        for k, wt in ((k0, 1.0 - fr), (k0 + 1, fr)):
            kc = min(max(k, 0), 11)
            uv[R, kc] += wt
    K = np.einsum("Rk,Cj->kjRC", uv, uv).reshape(144, 9216)
    return np.ascontiguousarray(K).astype(np.float16)


def make_in_maps(inputs):
    f16 = np.float16
    x = np.asarray(inputs["x"], np.float32).reshape(C, H, W)
    xflat = np.ascontiguousarray(x.reshape(C, N))
    xpad = np.zeros((C, 98, 98), f16)
    xpad[:, 1:97, 1:97] = x.astype(f16)
    krF = _bilinear_kron()

    def conv_w(w):
        # [o, i, dy, dx] -> [i, (dy dx), o]
        return np.ascontiguousarray(
            np.asarray(w, np.float32).transpose(1, 2, 3, 0).reshape(C, 9 * C)
        ).astype(f16)

    base = {
        "xpad": xpad,
        "w1": conv_w(inputs["d1_w"]),
        "w2": conv_w(inputs["d2_w"]),
        "w3": conv_w(inputs["d3_w"]),
        "twT": np.ascontiguousarray(
            np.asarray(inputs["th_w"], np.float32)[:, :, 0, 0].T).astype(f16),
        "pwT": np.ascontiguousarray(
            np.asarray(inputs["ph_w"], np.float32)[:, :, 0, 0].T).astype(f16),
        "gw": np.ascontiguousarray(
            np.asarray(inputs["g_w"], np.float32)[:, :, 0, 0]),
        "WwT": np.ascontiguousarray(
            np.asarray(inputs["W_w"], np.float32)[:, :, 0, 0].T),
    }
    in_maps = []
    for k in range(NCORES):
        m = dict(base)
        m["xch"] = np.ascontiguousarray(
            xflat[:, k * CH:(k + 1) * CH]).astype(f16)
        m["krC"] = np.ascontiguousarray(krF[:, k * CH:(k + 1) * CH])
        in_maps.append(m)
    return in_maps


def kernel(**inputs):
    from concourse import bass_utils

    nc = get_program()
    in_maps = make_in_maps(inputs)
    res = bass_utils.run_bass_kernel_spmd(nc, in_maps,
                                          core_ids=list(range(NCORES)))
    out = np.concatenate([res.results[k]["out"] for k in range(NCORES)], axis=1)
    return out.reshape(1, C, H, W).astype(np.float32)
